# revision 13
# baseline (speedup 1.0000x reference)
"""Self-contained Trainium2 (Bass/Tile) kernel for the DeviceGAT problem.

Computes, on 8 NeuronCores, the GAT layer + LayerNorm + two decoder MLPs of
reference.py, exploiting the deterministic graph structure:
  - edges = dense local clique (1024 local nodes, no self edges)
            + local->remote pairs (i -> L+i)
            + remote->local pairs (L+i -> i)
            + self loops for all 2048 nodes
  - so each local dst d has in-edges from all 1024 local nodes (incl. itself
    via the self loop) plus remote node L+d; each remote dst L+i has in-edges
    {i, L+i}.

Sharding: destination rows are sharded 8 ways (128 local dst rows + the
matching 128 remote dst rows per core).  Node features / params replicated.

Scores are built directly in transposed [src, dst] layout (a_dst broadcast by
a tiny selector matmul, a_src as the per-partition activation bias), so the
attention needs NO on-chip transposes.  exp(scores) is written to HBM
unnormalized together with the per-(dst, head) softmax reciprocal R; the host
applies R while permuting the dense block into the reference per-edge order.
The aggregation matmul contracts src on the partition axis (exb^T @ [x | 1])
in bf16, producing both the weighted message sum and the softmax denominator
for all four heads in one PSUM tile; per-head epilogue work is batched into a
handful of broadcast-AP vector ops.
"""

import os
import sys
import types
import numpy as np

# ---------------------------------------------------------------------------
# Problem constants (from the reference problem definition; deterministic).
L = 1024
N = 2 * L
MAPPED = 32
HID = 64
HEADS = 4
OUT = HID * HEADS        # 256
IN_DIM = 128
NEG_SLOPE = 0.2
EPS = 1e-5
NCORES = 8
SH = L // NCORES         # 128 dst rows per core
NLOC_T = L // 128        # 8 local node tiles


def _ensure_axon_hooks_stub():
    """run_bass_kernel_spmd(trace=True) imports antenv.axon_hooks; provide a
    graceful stub when the image lacks it so tracing degrades instead of
    crashing.  (Harness runs trace=False and never hits this, but be safe.)"""
    try:
        import antenv.axon_hooks  # noqa: F401
        return
    except Exception:
        pass
    try:
        import antenv
    except Exception:
        antenv = types.ModuleType("antenv")
        sys.modules["antenv"] = antenv
    mod = types.ModuleType("antenv.axon_hooks")
    mod._HOOK = None

    def set_axon_ntff_profile_hook(hook):
        mod._HOOK = hook

    def get_axon_ntff_profile_hook():
        if mod._HOOK is not None:
            return mod._HOOK
        so = "/opt/axon/libaxon_pjrt.so"
        if os.path.exists(so):
            import contextlib
            import ctypes

            try:
                lib = ctypes.CDLL(so)
            except OSError:
                return None
            if not hasattr(lib, "axon_start_nrt_profile"):
                return None
            lib.axon_start_nrt_profile.argtypes = [
                ctypes.POINTER(ctypes.c_int64),
                ctypes.c_size_t,
            ]
            lib.axon_start_nrt_profile.restype = ctypes.c_int64
            lib.axon_stop_nrt_profile.argtypes = [ctypes.c_char_p]
            lib.axon_stop_nrt_profile.restype = ctypes.c_int64

            @contextlib.contextmanager
            def _hook(output_dir, device_ids):
                import jax

                jax.devices()
                if device_ids:
                    ids = (ctypes.c_int64 * len(device_ids))(*device_ids)
                    rc = lib.axon_start_nrt_profile(ids, len(device_ids))
                else:
                    rc = lib.axon_start_nrt_profile(None, 0)
                if rc != 0:
                    raise RuntimeError(f"axon_start_nrt_profile rc={rc}")
                try:
                    yield
                finally:
                    n = lib.axon_stop_nrt_profile(str(output_dir).encode())
                    print(f"profile: {n} file(s) in {output_dir}", file=sys.stderr)

            mod._HOOK = _hook
            return mod._HOOK
        return None

    mod.set_axon_ntff_profile_hook = set_axon_ntff_profile_hook
    mod.get_axon_ntff_profile_hook = get_axon_ntff_profile_hook
    sys.modules["antenv.axon_hooks"] = mod


# ---------------------------------------------------------------------------
# Bass kernel builder


def _build_nc():
    from contextlib import ExitStack

    import concourse.tile as tile
    from concourse import bacc, mybir
    from concourse.masks import make_identity

    f32 = mybir.dt.float32
    bf16 = mybir.dt.bfloat16
    AF = mybir.ActivationFunctionType
    MUL = mybir.AluOpType.mult
    ADD = mybir.AluOpType.add
    WEXT = OUT + 2 * HEADS  # 264: [Wg | Wsrc | Wdst]

    nc = bacc.Bacc(
        trn_type="TRN2", target_bir_lowering=False, debug=False, num_devices=NCORES
    )

    # ---- I/O ----
    nfT = nc.dram_tensor("nfT", [MAPPED, N], f32, kind="ExternalInput")
    nfT_loc = nc.dram_tensor("nfT_loc", [MAPPED, SH], f32, kind="ExternalInput")
    nfT_rem = nc.dram_tensor("nfT_rem", [MAPPED, SH], f32, kind="ExternalInput")
    Wg = nc.dram_tensor("Wg", [MAPPED, WEXT], f32, kind="ExternalInput")
    bgat = nc.dram_tensor("bgat", [1, OUT], f32, kind="ExternalInput")
    decs = {}
    for tag in ("l", "r"):
        decs[tag] = {
            "W1": nc.dram_tensor(f"W1{tag}", [OUT, 128], f32, kind="ExternalInput"),
            "b1": nc.dram_tensor(f"b1{tag}", [128, 128], f32, kind="ExternalInput"),
            "W2": nc.dram_tensor(f"W2{tag}", [128, 64], f32, kind="ExternalInput"),
            "b2": nc.dram_tensor(f"b2{tag}", [128, 64], f32, kind="ExternalInput"),
            "W3": nc.dram_tensor(f"W3{tag}", [64, 128], f32, kind="ExternalInput"),
            "b3": nc.dram_tensor(f"b3{tag}", [128, 128], f32, kind="ExternalInput"),
        }

    # dense unnormalized exp(scores), transposed layout, raw per-head blocks
    # [p, t, d]: src node s = t*128+p, dst column d
    alpha_t = nc.dram_tensor("alpha_t", [128, HEADS * L], f32, kind="ExternalOutput")
    # a_rl | a_lr | a_rr | R  (R = per-(d,h) softmax reciprocal)
    asmall = nc.dram_tensor("asmall", [SH, 16], f32, kind="ExternalOutput")
    rec_l = nc.dram_tensor("rec_l", [2 * SH, IN_DIM], f32, kind="ExternalOutput")
    rec_r = nc.dram_tensor("rec_r", [2 * SH, IN_DIM], f32, kind="ExternalOutput")

    with tile.TileContext(nc) as tc, ExitStack() as ctx:
        consts = ctx.enter_context(tc.tile_pool(name="consts", bufs=1))
        big = ctx.enter_context(tc.tile_pool(name="big", bufs=2))
        persist = ctx.enter_context(tc.tile_pool(name="persist", bufs=1))
        small = ctx.enter_context(tc.tile_pool(name="small", bufs=2))
        psB = ctx.enter_context(tc.tile_pool(name="psB", bufs=2, space="PSUM"))
        psG = ctx.enter_context(tc.tile_pool(name="psG", bufs=1, space="PSUM"))
        psT = ctx.enter_context(tc.tile_pool(name="psT", bufs=2, space="PSUM"))
        psX = ctx.enter_context(tc.tile_pool(name="psX", bufs=2, space="PSUM"))

        # ---- load constants ----
        sb_nfT = consts.tile([MAPPED, N], f32)
        nc.sync.dma_start(sb_nfT[:], nfT[:])
        sb_nfT_loc = consts.tile([MAPPED, SH], f32)
        nc.sync.dma_start(sb_nfT_loc[:], nfT_loc[:])
        sb_nfT_rem = consts.tile([MAPPED, SH], f32)
        nc.sync.dma_start(sb_nfT_rem[:], nfT_rem[:])
        sb_Wg = consts.tile([MAPPED, WEXT], f32)
        nc.sync.dma_start(sb_Wg[:], Wg[:])
        sbd = {}
        for tag in ("l", "r"):
            d = decs[tag]
            sbd[tag] = {
                "W1": consts.tile([128, 2, 128], f32, tag=f"W1{tag}", name=f"sbW1{tag}"),
                "b1": consts.tile([128, 128], f32, tag=f"b1{tag}", name=f"sbb1{tag}"),
                "W2": consts.tile([128, 64], f32, tag=f"W2{tag}", name=f"sbW2{tag}"),
                "b2": consts.tile([128, 64], f32, tag=f"b2{tag}", name=f"sbb2{tag}"),
                "W3": consts.tile([64, 128], f32, tag=f"W3{tag}", name=f"sbW3{tag}"),
                "b3": consts.tile([128, 128], f32, tag=f"b3{tag}", name=f"sbb3{tag}"),
            }
            nc.sync.dma_start(
                sbd[tag]["W1"][:], d["W1"].rearrange("(a p) n -> p a n", p=128)
            )
            for k in ("b1", "W2", "b2", "W3", "b3"):
                nc.sync.dma_start(sbd[tag][k][:], d[k][:])

        eps_col = consts.tile([128, 1], f32)
        nc.vector.memset(eps_col[:], EPS)
        ident = consts.tile([128, 128], f32)
        make_identity(nc, ident[:])
        # head-selector: sel4[k, h*128+m] = (k == h); used as k=4 lhsT to
        # broadcast row h of a [4, x] tile across 128 partitions
        sel4 = consts.tile([HEADS, HEADS * 128], f32)
        nc.gpsimd.memset(sel4[:], 0.0)
        sel4v = sel4.rearrange("p (a b) -> p a b", a=HEADS)
        nc.gpsimd.affine_select(
            out=sel4v,
            in_=sel4v,
            compare_op=mybir.AluOpType.not_equal,
            fill=1.0,
            base=0,
            pattern=[[-1, HEADS], [0, 128]],
            channel_multiplier=1,
        )
        # Bgat broadcast tile (for h_rem / h_loc bias)
        sb_Bg = persist.tile([128, OUT], f32, tag="bg")
        nc.gpsimd.dma_start(sb_Bg[:], bgat[:].to_broadcast((128, OUT)))

        # ---- projections on PE: x|a_src|a_dst per node tile ----
        # xbf_all[p, t, h, 0:64] = x, [..., 64] = 1.0  (bf16, agg rhs)
        xbf_all = persist.tile([128, NLOC_T, HEADS, HID + 1], bf16, tag="xbf")
        nc.vector.memset(xbf_all[:], 1.0)
        A_t = []    # f32 [128, 8]: a_src cols 0:4, a_dst cols 4:8
        for t in range(NLOC_T):
            ps = psX.tile([128, WEXT], f32, tag="psx")
            nc.tensor.matmul(ps[:], sb_nfT[:, t * 128 : (t + 1) * 128], sb_Wg[:])
            nc.vector.tensor_copy(
                xbf_all[:, t, :, 0:HID],
                ps[:, 0:OUT].rearrange("p (h c) -> p h c", h=HEADS),
            )
            at = persist.tile([128, 2 * HEADS], f32, tag=f"at{t}", name=f"at{t}")
            nc.scalar.copy(at[:], ps[:, OUT:WEXT])
            A_t.append(at)

        ps = psX.tile([128, WEXT], f32, tag="psx")
        nc.tensor.matmul(ps[:], sb_nfT_rem[:], sb_Wg[:])
        x_rem = persist.tile([128, OUT], f32, tag="xrem")
        nc.scalar.copy(x_rem[:], ps[:, 0:OUT])
        A_rem = persist.tile([128, 2 * HEADS], f32, tag="arem")
        nc.vector.tensor_copy(A_rem[:], ps[:, OUT:WEXT])

        ps = psX.tile([128, WEXT], f32, tag="psx")
        nc.tensor.matmul(ps[:], sb_nfT_loc[:], sb_Wg[:])
        x_shard = persist.tile([128, OUT], f32, tag="xshard")
        nc.vector.tensor_copy(x_shard[:], ps[:, 0:OUT])
        A_loc = persist.tile([128, 2 * HEADS], f32, tag="aloc")
        nc.scalar.copy(A_loc[:], ps[:, OUT:WEXT])

        # a_dst of the shard as rows: [4, 128]
        psd = psX.tile([128, WEXT], f32, tag="psx")
        nc.tensor.matmul(
            psd[0:HEADS, 0:SH], sb_Wg[:, OUT + HEADS : WEXT], sb_nfT_loc[:]
        )
        a_dstT = consts.tile([HEADS, SH], f32)
        nc.scalar.copy(a_dstT[:], psd[0:HEADS, 0:SH])

        # ---- special-edge scores: e_rl | e_lr | e_rr  [128, 12] ----
        E3 = persist.tile([128, 12], f32, tag="E3")
        nc.vector.tensor_add(E3[:, 0:4], A_rem[:, 0:4], A_loc[:, 4:8])
        nc.vector.tensor_add(E3[:, 4:8], A_loc[:, 0:4], A_rem[:, 4:8])
        nc.vector.tensor_add(E3[:, 8:12], A_rem[:, 0:4], A_rem[:, 4:8])
        LR3 = persist.tile([128, 12], f32, tag="LR3")
        nc.scalar.activation(LR3[:], E3[:], AF.Prelu, alpha=NEG_SLOPE)
        EXP3 = persist.tile([128, 12], f32, tag="EXP3")
        nc.scalar.activation(EXP3[:], LR3[:], AF.Exp)

        # remote-dst softmax (2 edges) and h_rem (batched broadcast ops)
        asm = persist.tile([128, 16], f32, tag="asm")  # a_rl | a_lr | a_rr | R
        Zr = persist.tile([128, 4], f32, tag="Zr")
        nc.vector.tensor_add(Zr[:], EXP3[:, 4:8], EXP3[:, 8:12])
        Rr = persist.tile([128, 4], f32, tag="Rr")
        nc.vector.reciprocal(Rr[:], Zr[:])
        nc.vector.tensor_mul(asm[:, 4:8], EXP3[:, 4:8], Rr[:])
        nc.vector.tensor_mul(asm[:, 8:12], EXP3[:, 8:12], Rr[:])

        x_shard_v = x_shard.rearrange("p (h c) -> p h c", h=HEADS)
        x_rem_v = x_rem.rearrange("p (h c) -> p h c", h=HEADS)
        hrem_t1 = persist.tile([128, OUT], f32, tag="hrem_t1")
        nc.vector.tensor_mul(
            hrem_t1.rearrange("p (h c) -> p h c", h=HEADS),
            x_shard_v, asm[:, 4:8].to_broadcast((128, HEADS, HID)),
        )
        hrem_t2 = persist.tile([128, OUT], f32, tag="hrem_t2")
        nc.vector.tensor_mul(
            hrem_t2.rearrange("p (h c) -> p h c", h=HEADS),
            x_rem_v, asm[:, 8:12].to_broadcast((128, HEADS, HID)),
        )
        h_rem = persist.tile([128, OUT], f32, tag="h_rem")
        nc.vector.tensor_add(h_rem[:], hrem_t1[:], hrem_t2[:])
        nc.vector.tensor_add(h_rem[:], h_rem[:], sb_Bg[:])

        # ---- layernorm helper (stats DVE, sqrt ACT, apply DVE) ----
        BNS = nc.vector.BN_STATS_DIM
        BNA = nc.vector.BN_AGGR_DIM

        def layer_norm(x_t, width, tagp):
            st = small.tile([128, BNS], f32, tag="bnst")
            nc.vector.bn_stats(st[:], x_t[:, 0:width])
            mv = small.tile([128, BNA], f32, tag="bnmv")
            nc.vector.bn_aggr(mv[:], st[:])
            sdev = small.tile([128, 1], f32, tag="sdev")
            nc.scalar.activation(sdev[:], mv[:, 1:2], AF.Sqrt, bias=eps_col[:])
            rstd = small.tile([128, 1], f32, tag="rstd")
            nc.vector.reciprocal(rstd[:], sdev[:])
            negmr = small.tile([128, 1], f32, tag="negmr")
            nc.vector.tensor_scalar(
                negmr[:], mv[:, 0:1], rstd[:], -1.0, op0=MUL, op1=MUL
            )
            out_t = big.tile([128, width], f32, tag=tagp)
            nc.vector.tensor_scalar(
                out_t[:], x_t[:, 0:width], rstd[:], negmr[:], op0=MUL, op1=ADD
            )
            return out_t

        # ---- decoder machinery (f32) ----
        rec_sb = {
            "l": persist.tile([128, 2, IN_DIM], f32, tag="recl", name="recl"),
            "r": persist.tile([128, 2, IN_DIM], f32, tag="recr", name="recr"),
        }

        def transpose_h(hn_t, nt):
            psTt = psT.tile([128, 4, 128], f32, tag="psT")
            nc.tensor.transpose(psTt[:, 0, :], hn_t[:, 0:128], ident[:])
            nc.tensor.transpose(psTt[:, 1, :], hn_t[:, 128:256], ident[:])
            ha = big.tile([128, 128], f32, tag=f"hTa{nt}", name=f"hTa{nt}")
            hb = big.tile([128, 128], f32, tag=f"hTb{nt}", name=f"hTb{nt}")
            if nt == 0:
                nc.scalar.copy(ha[:], psTt[:, 0, :])
                nc.vector.tensor_copy(hb[:], psTt[:, 1, :])
            else:
                nc.vector.tensor_copy(ha[:], psTt[:, 0, :])
                nc.scalar.copy(hb[:], psTt[:, 1, :])
            return ha, hb

        def run_decoders_nt(nt, ha, hb):
            for di, tag in enumerate(("l", "r")):
                p = sbd[tag]
                rec_t = rec_sb[tag]
                ps1 = psX.tile([128, WEXT], f32, tag="psx")
                nc.tensor.matmul(ps1[:, 0:128], ha[:], p["W1"][:, 0, :],
                                 start=True, stop=False)
                nc.tensor.matmul(ps1[:, 0:128], hb[:], p["W1"][:, 1, :],
                                 start=False, stop=True)
                s1 = big.tile([128, 128], f32, tag="s1")
                nc.vector.tensor_add(s1[:], ps1[:, 0:128], p["b1"][:])
                r1 = big.tile([128, 128], f32, tag="r1")
                nc.scalar.activation(r1[:], s1[:], AF.Relu)
                n1 = layer_norm(r1, 128, "n1")
                psn = psT.tile([128, 4, 128], f32, tag="psT")
                nc.tensor.transpose(psn[:, 0, :], n1[:], ident[:])
                n1T = big.tile([128, 128], f32, tag="n1T")
                if (nt + di) % 2 == 0:
                    nc.scalar.copy(n1T[:], psn[:, 0, :])
                else:
                    nc.vector.tensor_copy(n1T[:], psn[:, 0, :])

                ps2 = psX.tile([128, WEXT], f32, tag="psx")
                nc.tensor.matmul(ps2[:, 0:64], n1T[:], p["W2"][:],
                                 start=True, stop=True)
                s2 = big.tile([128, 64], f32, tag="s2")
                nc.vector.tensor_add(s2[:], ps2[:, 0:64], p["b2"][:])
                r2 = big.tile([128, 64], f32, tag="r2")
                nc.scalar.activation(r2[:], s2[:], AF.Relu)
                n2 = layer_norm(r2, 64, "n2")
                psn2 = psT.tile([128, 4, 128], f32, tag="psT")
                nc.tensor.transpose(psn2[0:64, 0, :], n2[:, 0:64], ident[:])
                n2T = big.tile([64, 128], f32, tag="n2T")
                if (nt + di) % 2 == 0:
                    nc.vector.tensor_copy(n2T[:], psn2[0:64, 0, :])
                else:
                    nc.scalar.copy(n2T[:], psn2[0:64, 0, :])

                ps3 = psX.tile([128, WEXT], f32, tag="psx")
                nc.tensor.matmul(ps3[:, 0:IN_DIM], n2T[:], p["W3"][:],
                                 start=True, stop=True)
                nc.vector.tensor_add(rec_t[:, nt, :], ps3[:, 0:IN_DIM], p["b3"][:])

        # remote half of the decoders runs during the attention head loop
        hn_rem = layer_norm(h_rem, OUT, "hn")
        ha1, hb1 = transpose_h(hn_rem, 1)
        run_decoders_nt(1, ha1, hb1)

        # ---- dense attention per head (transposed [s, d] layout) ----
        # all 4 heads accumulate into one PSUM tile: [:, h, 0:64] = msg sum,
        # [:, h, 64] = sum(exp)
        psagg = psG.tile([128, HEADS, HID + 1], f32, tag="agg")
        for h in range(HEADS):
            # Bh[s_p, d] = a_dst[d]  (selector matmul broadcast)
            psBh = psB.tile([128, SH], f32, tag="Bh")
            nc.tensor.matmul(psBh[:], sel4[:, h * 128 : (h + 1) * 128], a_dstT[:])
            # e_T[p, t, d] = prelu(a_src[t*128+p] + a_dst[d])
            e_T = big.tile([128, NLOC_T, SH], f32, tag="eT")
            for t in range(NLOC_T):
                nc.scalar.activation(
                    e_T[:, t, :], psBh[:], AF.Prelu,
                    bias=A_t[t][:, h : h + 1], alpha=NEG_SLOPE,
                )
            ex_T = big.tile([128, NLOC_T, SH], f32, tag="exT")
            nc.scalar.activation(
                ex_T.rearrange("p a b -> p (a b)"),
                e_T.rearrange("p a b -> p (a b)"),
                AF.Exp,
            )
            # unnormalized exp block straight to HBM (host normalizes)
            nc.sync.dma_start(
                alpha_t[:, h * L : (h + 1) * L],
                ex_T.rearrange("p a b -> p (a b)"),
            )
            exb = big.tile([128, NLOC_T, SH], bf16, tag="exb")
            nc.gpsimd.tensor_copy(exb[:], ex_T[:])

            # aggregation + Z in one accumulation: exb[s,:]^T @ [x_h | 1]
            for t in range(NLOC_T):
                nc.tensor.matmul(
                    psagg[:, h, :],
                    exb[:, t, :],
                    xbf_all[:, t, h, :],
                    start=(t == 0),
                    stop=(t == NLOC_T - 1),
                )

        # ---- batched per-head epilogue ----
        # Zf = psagg[:, :, 64] + exp(e_rl);  R = 1/Zf
        Zf = small.tile([128, HEADS], f32, tag="Zf")
        nc.vector.tensor_add(Zf[:], psagg[:, :, HID], EXP3[:, 0:4])
        nc.vector.reciprocal(asm[:, 12:16], Zf[:])
        nc.vector.tensor_mul(asm[:, 0:4], EXP3[:, 0:4], asm[:, 12:16])
        # h_loc = (agg + exp(e_rl)*x_rem) * R + bgat
        xt1 = persist.tile([128, OUT], f32, tag="xt1")
        xt1_v = xt1.rearrange("p (h c) -> p h c", h=HEADS)
        nc.vector.tensor_mul(
            xt1_v, x_rem_v, EXP3[:, 0:4].to_broadcast((128, HEADS, HID))
        )
        hpre = persist.tile([128, OUT], f32, tag="hpre")
        hpre_v = hpre.rearrange("p (h c) -> p h c", h=HEADS)
        nc.vector.tensor_add(hpre_v, psagg[:, :, 0:HID], xt1_v)
        h_loc = persist.tile([128, OUT], f32, tag="h_loc")
        nc.vector.tensor_mul(
            h_loc.rearrange("p (h c) -> p h c", h=HEADS),
            hpre_v, asm[:, 12:16].to_broadcast((128, HEADS, HID)),
        )
        nc.vector.tensor_add(h_loc[:], h_loc[:], sb_Bg[:])
        nc.sync.dma_start(asmall[:], asm[:])

        # ---- local half of decoders ----
        hn_loc = layer_norm(h_loc, OUT, "hn")
        ha0, hb0 = transpose_h(hn_loc, 0)
        run_decoders_nt(0, ha0, hb0)

        nc.sync.dma_start(rec_l.rearrange("(a p) n -> p a n", p=128), rec_sb["l"][:])
        nc.sync.dma_start(rec_r.rearrange("(a p) n -> p a n", p=128), rec_sb["r"][:])

    nc.compile()
    return nc


_NC_CACHE = {}


def _get_nc():
    if "nc" not in _NC_CACHE:
        _NC_CACHE["nc"] = _build_nc()
    return _NC_CACHE["nc"]


# ---------------------------------------------------------------------------
# Host side


def _expected_edge_index():
    local = np.arange(L)
    s = np.repeat(local, L)
    d = np.tile(local, L)
    m = s != d
    src = np.concatenate([s[m], local, L + local, np.arange(N)])
    dst = np.concatenate([d[m], L + local, local, np.arange(N)])
    return np.stack([src, dst])


def _np_reference_fallback(node_features, gat, norm, dec_local, dec_remote,
                           node_types, edge_index):
    """Pure-numpy replica of the reference; used only if the edge structure
    is not the expected deterministic pattern."""
    x = node_features @ gat["W"]
    xh = x.reshape(N, HEADS, HID)
    a_src = (xh * gat["att_src"]).sum(-1)
    a_dst = (xh * gat["att_dst"]).sum(-1)
    src, dst = edge_index[0], edge_index[1]
    e = a_src[src] + a_dst[dst]
    e = np.where(e >= 0, e, NEG_SLOPE * e).astype(np.float32)
    emax = np.full((N, HEADS), -np.inf, np.float32)
    np.maximum.at(emax, dst, e)
    ex = np.exp(e - emax[dst])
    zs = np.zeros((N, HEADS), np.float32)
    np.add.at(zs, dst, ex)
    alpha = ex / zs[dst]
    msg = xh[src] * alpha[:, :, None]
    out = np.zeros((N, HEADS, HID), np.float32)
    np.add.at(out, dst, msg)
    out = out.reshape(N, OUT) + gat["bias"]

    def ln(v, g, b):
        m = v.mean(-1, keepdims=True)
        var = ((v - m) ** 2).mean(-1, keepdims=True)
        return (v - m) / np.sqrt(var + EPS) * g + b

    h = ln(out, norm["g"], norm["b"])

    def dec(v, p):
        t = ln(np.maximum(v @ p["W1"] + p["b1"], 0), p["g1"], p["b1n"])
        t = ln(np.maximum(t @ p["W2"] + p["b2"], 0), p["g2"], p["b2n"])
        return t @ p["W3"] + p["b3"]

    rl = dec(h, dec_local)
    rr = dec(h, dec_remote)
    rec = np.where((node_types == 1)[:, None], rl, rr)
    return rec.astype(np.float32), edge_index, alpha.astype(np.float32)


def _to_np(v):
    return {k: _to_np(x) for k, x in v.items()} if isinstance(v, dict) else np.asarray(v)


def kernel(node_features, gat, norm, dec_local, dec_remote, node_types,
           edge_index, trace=False):
    _ensure_axon_hooks_stub()
    node_features = np.asarray(node_features, np.float32)
    gat, norm = _to_np(gat), _to_np(norm)
    dec_local, dec_remote = _to_np(dec_local), _to_np(dec_remote)
    node_types_np = np.asarray(node_types)
    edge_index_np = np.asarray(edge_index)

    if not np.array_equal(edge_index_np.astype(np.int64), _expected_edge_index()):
        return _np_reference_fallback(
            node_features, gat, norm, dec_local, dec_remote,
            node_types_np, edge_index_np,
        )

    from concourse.bass_utils import run_bass_kernel_spmd

    f32 = np.float32
    W = gat["W"].astype(f32)
    att_src = gat["att_src"].astype(f32)
    att_dst = gat["att_dst"].astype(f32)
    Wsrc = (W.reshape(MAPPED, HEADS, HID) * att_src[None]).sum(-1)
    Wdst = (W.reshape(MAPPED, HEADS, HID) * att_dst[None]).sum(-1)
    Wg_ext = np.ascontiguousarray(np.concatenate([W, Wsrc, Wdst], axis=1))
    nfT = np.ascontiguousarray(node_features.T)

    # fold LN affine params into the following linear layer; biases
    # pre-broadcast to [128, w]
    def fold(dec):
        g, b = norm["g"].astype(f32), norm["b"].astype(f32)
        W1 = g[:, None] * dec["W1"]
        b1 = dec["b1"] + b @ dec["W1"]
        W2 = dec["g1"][:, None] * dec["W2"]
        b2 = dec["b2"] + dec["b1n"] @ dec["W2"]
        W3 = dec["g2"][:, None] * dec["W3"]
        b3 = dec["b3"] + dec["b2n"] @ dec["W3"]

        def bc(v):
            return np.ascontiguousarray(
                np.broadcast_to(v.reshape(1, -1).astype(f32), (128, v.size))
            )

        return (
            np.ascontiguousarray(W1, f32), bc(b1),
            np.ascontiguousarray(W2, f32), bc(b2),
            np.ascontiguousarray(W3, f32), bc(b3),
        )

    W1l, b1l, W2l, b2l, W3l, b3l = fold(dec_local)
    W1r, b1r, W2r, b2r, W3r, b3r = fold(dec_remote)

    shared = {
        "nfT": nfT, "Wg": Wg_ext,
        "bgat": gat["bias"].reshape(1, -1).astype(f32),
        "W1l": W1l, "b1l": b1l, "W2l": W2l, "b2l": b2l, "W3l": W3l, "b3l": b3l,
        "W1r": W1r, "b1r": b1r, "W2r": W2r, "b2r": b2r, "W3r": W3r, "b3r": b3r,
    }
    in_maps = []
    for c in range(NCORES):
        m = dict(shared)
        m["nfT_loc"] = np.ascontiguousarray(nfT[:, c * SH : (c + 1) * SH])
        m["nfT_rem"] = np.ascontiguousarray(nfT[:, L + c * SH : L + (c + 1) * SH])
        in_maps.append(m)

    nc = _get_nc()
    res = run_bass_kernel_spmd(nc, in_maps, core_ids=list(range(NCORES)), trace=trace)
    _NC_CACHE["last_results"] = res
    outs = res.results

    # ---- host unshard / assembly ----
    # alphaT_full[s, d, h] = exp(e) * R
    alphaT_full = np.empty((L, L, HEADS), np.float32)
    a_rl = np.empty((L, HEADS), np.float32)
    a_lr = np.empty((L, HEADS), np.float32)
    a_rr = np.empty((L, HEADS), np.float32)
    for c in range(NCORES):
        sl = slice(c * SH, (c + 1) * SH)
        sm = outs[c]["asmall"]
        a_rl[sl], a_lr[sl], a_rr[sl] = sm[:, 0:4], sm[:, 4:8], sm[:, 8:12]
        R = sm[:, 12:16]  # [d, h]
        blk = outs[c]["alpha_t"].reshape(128, HEADS, NLOC_T, SH)  # [p, h, t, d]
        # -> [s=t*128+p, d, h]
        blk = np.transpose(blk, (2, 0, 3, 1)).reshape(L, SH, HEADS)
        alphaT_full[:, sl, :] = blk * R[None, :, :]

    per_edge = alphaT_full.reshape(L * L, HEADS)
    mask = ~np.eye(L, dtype=bool).reshape(-1)
    clique = per_edge[mask]
    idx = np.arange(L)
    self_local = alphaT_full[idx, idx, :]
    alpha = np.concatenate(
        [clique, a_lr, a_rl, self_local, a_rr], axis=0
    ).astype(np.float32)

    rec_local = np.empty((N, IN_DIM), np.float32)
    rec_remote = np.empty((N, IN_DIM), np.float32)
    for c in range(NCORES):
        sl = slice(c * SH, (c + 1) * SH)
        slr = slice(L + c * SH, L + (c + 1) * SH)
        rec_local[sl] = outs[c]["rec_l"][0:SH]
        rec_local[slr] = outs[c]["rec_l"][SH : 2 * SH]
        rec_remote[sl] = outs[c]["rec_r"][0:SH]
        rec_remote[slr] = outs[c]["rec_r"][SH : 2 * SH]
    reconstructed = np.where((node_types_np == 1)[:, None], rec_local, rec_remote)

    return reconstructed.astype(np.float32), edge_index_np, alpha


# revision 14
# speedup vs baseline: 1.0649x; 1.0649x over previous
"""Self-contained Trainium2 (Bass/Tile) kernel for the DeviceGAT problem.

Computes, on 8 NeuronCores, the GAT layer + LayerNorm + two decoder MLPs of
reference.py, exploiting the deterministic graph structure:
  - edges = dense local clique (1024 local nodes, no self edges)
            + local->remote pairs (i -> L+i)
            + remote->local pairs (L+i -> i)
            + self loops for all 2048 nodes
  - so each local dst d has in-edges from all 1024 local nodes (incl. itself
    via the self loop) plus remote node L+d; each remote dst L+i has in-edges
    {i, L+i}.

Sharding: destination rows are sharded 8 ways (128 local dst rows + the
matching 128 remote dst rows per core).  Node features / params replicated.

Scores are built directly in transposed [src, dst] layout (a_dst broadcast by
a tiny selector matmul, a_src as the per-partition activation bias), so the
attention needs NO on-chip transposes.  exp(scores) is written to HBM
unnormalized together with the per-(dst, head) softmax reciprocal R; the host
applies R while permuting the dense block into the reference per-edge order.
The aggregation matmul contracts src on the partition axis (exb^T @ [x | 1])
in bf16, producing both the weighted message sum and the softmax denominator
for all four heads in one PSUM tile; per-head epilogue work is batched into a
handful of broadcast-AP vector ops.
"""

import os
import sys
import types
import numpy as np

# ---------------------------------------------------------------------------
# Problem constants (from the reference problem definition; deterministic).
L = 1024
N = 2 * L
MAPPED = 32
HID = 64
HEADS = 4
OUT = HID * HEADS        # 256
IN_DIM = 128
NEG_SLOPE = 0.2
EPS = 1e-5
NCORES = 8
SH = L // NCORES         # 128 dst rows per core
NLOC_T = L // 128        # 8 local node tiles


def _ensure_axon_hooks_stub():
    """run_bass_kernel_spmd(trace=True) imports antenv.axon_hooks; provide a
    graceful stub when the image lacks it so tracing degrades instead of
    crashing.  (Harness runs trace=False and never hits this, but be safe.)"""
    try:
        import antenv.axon_hooks  # noqa: F401
        return
    except Exception:
        pass
    try:
        import antenv
    except Exception:
        antenv = types.ModuleType("antenv")
        sys.modules["antenv"] = antenv
    mod = types.ModuleType("antenv.axon_hooks")
    mod._HOOK = None

    def set_axon_ntff_profile_hook(hook):
        mod._HOOK = hook

    def get_axon_ntff_profile_hook():
        if mod._HOOK is not None:
            return mod._HOOK
        so = "/opt/axon/libaxon_pjrt.so"
        if os.path.exists(so):
            import contextlib
            import ctypes

            try:
                lib = ctypes.CDLL(so)
            except OSError:
                return None
            if not hasattr(lib, "axon_start_nrt_profile"):
                return None
            lib.axon_start_nrt_profile.argtypes = [
                ctypes.POINTER(ctypes.c_int64),
                ctypes.c_size_t,
            ]
            lib.axon_start_nrt_profile.restype = ctypes.c_int64
            lib.axon_stop_nrt_profile.argtypes = [ctypes.c_char_p]
            lib.axon_stop_nrt_profile.restype = ctypes.c_int64

            @contextlib.contextmanager
            def _hook(output_dir, device_ids):
                import jax

                jax.devices()
                if device_ids:
                    ids = (ctypes.c_int64 * len(device_ids))(*device_ids)
                    rc = lib.axon_start_nrt_profile(ids, len(device_ids))
                else:
                    rc = lib.axon_start_nrt_profile(None, 0)
                if rc != 0:
                    raise RuntimeError(f"axon_start_nrt_profile rc={rc}")
                try:
                    yield
                finally:
                    n = lib.axon_stop_nrt_profile(str(output_dir).encode())
                    print(f"profile: {n} file(s) in {output_dir}", file=sys.stderr)

            mod._HOOK = _hook
            return mod._HOOK
        return None

    mod.set_axon_ntff_profile_hook = set_axon_ntff_profile_hook
    mod.get_axon_ntff_profile_hook = get_axon_ntff_profile_hook
    sys.modules["antenv.axon_hooks"] = mod


# ---------------------------------------------------------------------------
# Bass kernel builder


def _build_nc():
    from contextlib import ExitStack

    import concourse.tile as tile
    from concourse import bacc, mybir
    from concourse.masks import make_identity

    f32 = mybir.dt.float32
    bf16 = mybir.dt.bfloat16
    AF = mybir.ActivationFunctionType
    MUL = mybir.AluOpType.mult
    ADD = mybir.AluOpType.add
    WEXT = OUT + 2 * HEADS  # 264: [Wg | Wsrc | Wdst]

    nc = bacc.Bacc(
        trn_type="TRN2", target_bir_lowering=False, debug=False, num_devices=NCORES
    )

    # ---- I/O ----
    nfT = nc.dram_tensor("nfT", [MAPPED, N], f32, kind="ExternalInput")
    nfT_loc = nc.dram_tensor("nfT_loc", [MAPPED, SH], f32, kind="ExternalInput")
    nfT_rem = nc.dram_tensor("nfT_rem", [MAPPED, SH], f32, kind="ExternalInput")
    Wg = nc.dram_tensor("Wg", [MAPPED, WEXT], f32, kind="ExternalInput")
    bgat = nc.dram_tensor("bgat", [1, OUT], f32, kind="ExternalInput")
    decs = {}
    for tag in ("l", "r"):
        decs[tag] = {
            "W1": nc.dram_tensor(f"W1{tag}", [OUT, 128], f32, kind="ExternalInput"),
            "b1": nc.dram_tensor(f"b1{tag}", [128, 128], f32, kind="ExternalInput"),
            "W2": nc.dram_tensor(f"W2{tag}", [128, 64], f32, kind="ExternalInput"),
            "b2": nc.dram_tensor(f"b2{tag}", [128, 64], f32, kind="ExternalInput"),
            "W3": nc.dram_tensor(f"W3{tag}", [64, 128], f32, kind="ExternalInput"),
            "b3": nc.dram_tensor(f"b3{tag}", [128, 128], f32, kind="ExternalInput"),
        }

    # dense unnormalized exp(scores), transposed layout, raw per-head blocks
    # [p, t, d]: src node s = t*128+p, dst column d
    alpha_t = nc.dram_tensor("alpha_t", [128, HEADS * L], f32, kind="ExternalOutput")
    # a_rl | a_lr | a_rr | R  (R = per-(d,h) softmax reciprocal)
    asmall = nc.dram_tensor("asmall", [SH, 16], f32, kind="ExternalOutput")
    rec_l = nc.dram_tensor("rec_l", [2 * SH, IN_DIM], f32, kind="ExternalOutput")
    rec_r = nc.dram_tensor("rec_r", [2 * SH, IN_DIM], f32, kind="ExternalOutput")

    with tile.TileContext(nc) as tc, ExitStack() as ctx:
        consts = ctx.enter_context(tc.tile_pool(name="consts", bufs=1))
        big = ctx.enter_context(tc.tile_pool(name="big", bufs=2))
        persist = ctx.enter_context(tc.tile_pool(name="persist", bufs=1))
        small = ctx.enter_context(tc.tile_pool(name="small", bufs=2))
        psB = ctx.enter_context(tc.tile_pool(name="psB", bufs=2, space="PSUM"))
        psG = ctx.enter_context(tc.tile_pool(name="psG", bufs=1, space="PSUM"))
        psT = ctx.enter_context(tc.tile_pool(name="psT", bufs=2, space="PSUM"))
        psX = ctx.enter_context(tc.tile_pool(name="psX", bufs=2, space="PSUM"))

        # ---- load constants ----
        sb_nfT = consts.tile([MAPPED, N], f32)
        nc.sync.dma_start(sb_nfT[:], nfT[:])
        sb_nfT_loc = consts.tile([MAPPED, SH], f32)
        nc.sync.dma_start(sb_nfT_loc[:], nfT_loc[:])
        sb_nfT_rem = consts.tile([MAPPED, SH], f32)
        nc.sync.dma_start(sb_nfT_rem[:], nfT_rem[:])
        sb_Wg = consts.tile([MAPPED, WEXT], f32)
        nc.sync.dma_start(sb_Wg[:], Wg[:])
        sbd = {}
        for tag in ("l", "r"):
            d = decs[tag]
            sbd[tag] = {
                "W1": consts.tile([128, 2, 128], f32, tag=f"W1{tag}", name=f"sbW1{tag}"),
                "b1": consts.tile([128, 128], f32, tag=f"b1{tag}", name=f"sbb1{tag}"),
                "W2": consts.tile([128, 64], f32, tag=f"W2{tag}", name=f"sbW2{tag}"),
                "b2": consts.tile([128, 64], f32, tag=f"b2{tag}", name=f"sbb2{tag}"),
                "W3": consts.tile([64, 128], f32, tag=f"W3{tag}", name=f"sbW3{tag}"),
                "b3": consts.tile([128, 128], f32, tag=f"b3{tag}", name=f"sbb3{tag}"),
            }
            nc.sync.dma_start(
                sbd[tag]["W1"][:], d["W1"].rearrange("(a p) n -> p a n", p=128)
            )
            for k in ("b1", "W2", "b2", "W3", "b3"):
                nc.sync.dma_start(sbd[tag][k][:], d[k][:])

        eps_col = consts.tile([128, 1], f32)
        nc.vector.memset(eps_col[:], EPS)
        ident = consts.tile([128, 128], f32)
        make_identity(nc, ident[:])
        # head-selector: sel4[k, h*128+m] = (k == h); used as k=4 lhsT to
        # broadcast row h of a [4, x] tile across 128 partitions
        sel4 = consts.tile([HEADS, HEADS * 128], f32)
        nc.gpsimd.memset(sel4[:], 0.0)
        sel4v = sel4.rearrange("p (a b) -> p a b", a=HEADS)
        nc.gpsimd.affine_select(
            out=sel4v,
            in_=sel4v,
            compare_op=mybir.AluOpType.not_equal,
            fill=1.0,
            base=0,
            pattern=[[-1, HEADS], [0, 128]],
            channel_multiplier=1,
        )
        # Bgat broadcast tile (for h_rem / h_loc bias)
        sb_Bg = persist.tile([128, OUT], f32, tag="bg")
        nc.gpsimd.dma_start(sb_Bg[:], bgat[:].to_broadcast((128, OUT)))

        # ---- projections on PE: x|a_src|a_dst per node tile ----
        # xbf_all[p, t, h, 0:64] = x, [..., 64] = 1.0  (bf16, agg rhs)
        xbf_all = persist.tile([128, NLOC_T, HEADS, HID + 1], bf16, tag="xbf")
        nc.vector.memset(xbf_all[:], 1.0)
        A_t = []    # f32 [128, 8]: a_src cols 0:4, a_dst cols 4:8
        for t in range(NLOC_T):
            ps = psX.tile([128, WEXT], f32, tag="psx")
            nc.tensor.matmul(ps[:], sb_nfT[:, t * 128 : (t + 1) * 128], sb_Wg[:])
            nc.vector.tensor_copy(
                xbf_all[:, t, :, 0:HID],
                ps[:, 0:OUT].rearrange("p (h c) -> p h c", h=HEADS),
            )
            at = persist.tile([128, 2 * HEADS], f32, tag=f"at{t}", name=f"at{t}")
            nc.scalar.copy(at[:], ps[:, OUT:WEXT])
            A_t.append(at)

        ps = psX.tile([128, WEXT], f32, tag="psx")
        nc.tensor.matmul(ps[:], sb_nfT_rem[:], sb_Wg[:])
        x_rem = persist.tile([128, OUT], f32, tag="xrem")
        nc.scalar.copy(x_rem[:], ps[:, 0:OUT])
        A_rem = persist.tile([128, 2 * HEADS], f32, tag="arem")
        nc.vector.tensor_copy(A_rem[:], ps[:, OUT:WEXT])

        ps = psX.tile([128, WEXT], f32, tag="psx")
        nc.tensor.matmul(ps[:], sb_nfT_loc[:], sb_Wg[:])
        x_shard = persist.tile([128, OUT], f32, tag="xshard")
        nc.vector.tensor_copy(x_shard[:], ps[:, 0:OUT])
        A_loc = persist.tile([128, 2 * HEADS], f32, tag="aloc")
        nc.scalar.copy(A_loc[:], ps[:, OUT:WEXT])

        # a_dst of the shard as rows: [4, 128]
        psd = psX.tile([128, WEXT], f32, tag="psx")
        nc.tensor.matmul(
            psd[0:HEADS, 0:SH], sb_Wg[:, OUT + HEADS : WEXT], sb_nfT_loc[:]
        )
        a_dstT = consts.tile([HEADS, SH], f32)
        nc.scalar.copy(a_dstT[:], psd[0:HEADS, 0:SH])

        # ---- special-edge scores: e_rl | e_lr | e_rr  [128, 12] ----
        E3 = persist.tile([128, 12], f32, tag="E3")
        nc.vector.tensor_add(E3[:, 0:4], A_rem[:, 0:4], A_loc[:, 4:8])
        nc.vector.tensor_add(E3[:, 4:8], A_loc[:, 0:4], A_rem[:, 4:8])
        nc.vector.tensor_add(E3[:, 8:12], A_rem[:, 0:4], A_rem[:, 4:8])
        LR3 = persist.tile([128, 12], f32, tag="LR3")
        nc.scalar.activation(LR3[:], E3[:], AF.Prelu, alpha=NEG_SLOPE)
        EXP3 = persist.tile([128, 12], f32, tag="EXP3")
        nc.scalar.activation(EXP3[:], LR3[:], AF.Exp)

        # remote-dst softmax (2 edges) and h_rem (batched broadcast ops)
        asm = persist.tile([128, 16], f32, tag="asm")  # a_rl | a_lr | a_rr | R
        Zr = persist.tile([128, 4], f32, tag="Zr")
        nc.vector.tensor_add(Zr[:], EXP3[:, 4:8], EXP3[:, 8:12])
        Rr = persist.tile([128, 4], f32, tag="Rr")
        nc.vector.reciprocal(Rr[:], Zr[:])
        nc.vector.tensor_mul(asm[:, 4:8], EXP3[:, 4:8], Rr[:])
        nc.vector.tensor_mul(asm[:, 8:12], EXP3[:, 8:12], Rr[:])

        x_shard_v = x_shard.rearrange("p (h c) -> p h c", h=HEADS)
        x_rem_v = x_rem.rearrange("p (h c) -> p h c", h=HEADS)
        hrem_t1 = persist.tile([128, OUT], f32, tag="hrem_t1")
        nc.vector.tensor_mul(
            hrem_t1.rearrange("p (h c) -> p h c", h=HEADS),
            x_shard_v, asm[:, 4:8].to_broadcast((128, HEADS, HID)),
        )
        hrem_t2 = persist.tile([128, OUT], f32, tag="hrem_t2")
        nc.vector.tensor_mul(
            hrem_t2.rearrange("p (h c) -> p h c", h=HEADS),
            x_rem_v, asm[:, 8:12].to_broadcast((128, HEADS, HID)),
        )
        h_rem = persist.tile([128, OUT], f32, tag="h_rem")
        nc.vector.tensor_add(h_rem[:], hrem_t1[:], hrem_t2[:])
        nc.vector.tensor_add(h_rem[:], h_rem[:], sb_Bg[:])

        # ---- layernorm helper (stats DVE, sqrt ACT, apply DVE) ----
        BNS = nc.vector.BN_STATS_DIM
        BNA = nc.vector.BN_AGGR_DIM

        def layer_norm(x_t, width, tagp):
            st = small.tile([128, BNS], f32, tag="bnst")
            nc.vector.bn_stats(st[:], x_t[:, 0:width])
            mv = small.tile([128, BNA], f32, tag="bnmv")
            nc.vector.bn_aggr(mv[:], st[:])
            sdev = small.tile([128, 1], f32, tag="sdev")
            nc.scalar.activation(sdev[:], mv[:, 1:2], AF.Sqrt, bias=eps_col[:])
            rstd = small.tile([128, 1], f32, tag="rstd")
            nc.vector.reciprocal(rstd[:], sdev[:])
            negmr = small.tile([128, 1], f32, tag="negmr")
            nc.vector.tensor_scalar(
                negmr[:], mv[:, 0:1], rstd[:], -1.0, op0=MUL, op1=MUL
            )
            out_t = big.tile([128, width], f32, tag=tagp)
            nc.vector.tensor_scalar(
                out_t[:], x_t[:, 0:width], rstd[:], negmr[:], op0=MUL, op1=ADD
            )
            return out_t

        # ---- decoder machinery (f32) ----
        rec_sb = {
            "l": persist.tile([128, 2, IN_DIM], f32, tag="recl", name="recl"),
            "r": persist.tile([128, 2, IN_DIM], f32, tag="recr", name="recr"),
        }

        def transpose_h(hn_t, nt):
            psTt = psT.tile([128, 4, 128], f32, tag="psT")
            nc.tensor.transpose(psTt[:, 0, :], hn_t[:, 0:128], ident[:])
            nc.tensor.transpose(psTt[:, 1, :], hn_t[:, 128:256], ident[:])
            ha = big.tile([128, 128], f32, tag=f"hTa{nt}", name=f"hTa{nt}")
            hb = big.tile([128, 128], f32, tag=f"hTb{nt}", name=f"hTb{nt}")
            if nt == 0:
                nc.scalar.copy(ha[:], psTt[:, 0, :])
                nc.vector.tensor_copy(hb[:], psTt[:, 1, :])
            else:
                nc.vector.tensor_copy(ha[:], psTt[:, 0, :])
                nc.scalar.copy(hb[:], psTt[:, 1, :])
            return ha, hb

        def run_decoders_nt(nt, ha, hb):
            for di, tag in enumerate(("l", "r")):
                p = sbd[tag]
                rec_t = rec_sb[tag]
                ps1 = psX.tile([128, WEXT], f32, tag="psx")
                nc.tensor.matmul(ps1[:, 0:128], ha[:], p["W1"][:, 0, :],
                                 start=True, stop=False)
                nc.tensor.matmul(ps1[:, 0:128], hb[:], p["W1"][:, 1, :],
                                 start=False, stop=True)
                s1 = big.tile([128, 128], f32, tag="s1")
                nc.vector.tensor_add(s1[:], ps1[:, 0:128], p["b1"][:])
                r1 = big.tile([128, 128], f32, tag="r1")
                nc.scalar.activation(r1[:], s1[:], AF.Relu)
                n1 = layer_norm(r1, 128, "n1")
                psn = psT.tile([128, 4, 128], f32, tag="psT")
                nc.tensor.transpose(psn[:, 0, :], n1[:], ident[:])
                n1T = big.tile([128, 128], f32, tag="n1T")
                if (nt + di) % 2 == 0:
                    nc.scalar.copy(n1T[:], psn[:, 0, :])
                else:
                    nc.vector.tensor_copy(n1T[:], psn[:, 0, :])

                ps2 = psX.tile([128, WEXT], f32, tag="psx")
                nc.tensor.matmul(ps2[:, 0:64], n1T[:], p["W2"][:],
                                 start=True, stop=True)
                s2 = big.tile([128, 64], f32, tag="s2")
                nc.vector.tensor_add(s2[:], ps2[:, 0:64], p["b2"][:])
                r2 = big.tile([128, 64], f32, tag="r2")
                nc.scalar.activation(r2[:], s2[:], AF.Relu)
                n2 = layer_norm(r2, 64, "n2")
                psn2 = psT.tile([128, 4, 128], f32, tag="psT")
                nc.tensor.transpose(psn2[0:64, 0, :], n2[:, 0:64], ident[:])
                n2T = big.tile([64, 128], f32, tag="n2T")
                if (nt + di) % 2 == 0:
                    nc.vector.tensor_copy(n2T[:], psn2[0:64, 0, :])
                else:
                    nc.scalar.copy(n2T[:], psn2[0:64, 0, :])

                ps3 = psX.tile([128, WEXT], f32, tag="psx")
                nc.tensor.matmul(ps3[:, 0:IN_DIM], n2T[:], p["W3"][:],
                                 start=True, stop=True)
                nc.vector.tensor_add(rec_t[:, nt, :], ps3[:, 0:IN_DIM], p["b3"][:])

        # remote half of the decoders runs during the attention head loop
        hn_rem = layer_norm(h_rem, OUT, "hn")
        ha1, hb1 = transpose_h(hn_rem, 1)
        run_decoders_nt(1, ha1, hb1)

        # ---- dense attention per head (transposed [s, d] layout) ----
        # all 4 heads accumulate into one PSUM tile: [:, h, 0:64] = msg sum,
        # [:, h, 64] = sum(exp)
        psagg = psG.tile([128, HEADS, HID + 1], f32, tag="agg")
        for h in range(HEADS):
            # Bh[s_p, d] = a_dst[d]  (selector matmul broadcast)
            psBh = psB.tile([128, SH], f32, tag="Bh")
            nc.tensor.matmul(psBh[:], sel4[:, h * 128 : (h + 1) * 128], a_dstT[:])
            # e_T[p, t, d] = prelu(a_src[t*128+p] + a_dst[d])
            e_T = big.tile([128, NLOC_T, SH], f32, tag="eT")
            for t in range(NLOC_T):
                nc.scalar.activation(
                    e_T[:, t, :], psBh[:], AF.Prelu,
                    bias=A_t[t][:, h : h + 1], alpha=NEG_SLOPE,
                )
            ex_T = big.tile([128, NLOC_T, SH], f32, tag="exT")
            nc.scalar.activation(
                ex_T.rearrange("p a b -> p (a b)"),
                e_T.rearrange("p a b -> p (a b)"),
                AF.Exp,
            )
            # unnormalized exp block straight to HBM (host normalizes)
            nc.sync.dma_start(
                alpha_t[:, h * L : (h + 1) * L],
                ex_T.rearrange("p a b -> p (a b)"),
            )
            exb = big.tile([128, NLOC_T, SH], bf16, tag="exb")
            nc.vector.tensor_copy(exb[:], ex_T[:])

            # aggregation + Z in one accumulation: exb[s,:]^T @ [x_h | 1]
            for t in range(NLOC_T):
                nc.tensor.matmul(
                    psagg[:, h, :],
                    exb[:, t, :],
                    xbf_all[:, t, h, :],
                    start=(t == 0),
                    stop=(t == NLOC_T - 1),
                )

        # ---- batched per-head epilogue ----
        # Zf = psagg[:, :, 64] + exp(e_rl);  R = 1/Zf
        Zf = small.tile([128, HEADS], f32, tag="Zf")
        nc.vector.tensor_add(Zf[:], psagg[:, :, HID], EXP3[:, 0:4])
        nc.vector.reciprocal(asm[:, 12:16], Zf[:])
        nc.vector.tensor_mul(asm[:, 0:4], EXP3[:, 0:4], asm[:, 12:16])
        # h_loc = (agg + exp(e_rl)*x_rem) * R + bgat
        xt1 = persist.tile([128, OUT], f32, tag="xt1")
        xt1_v = xt1.rearrange("p (h c) -> p h c", h=HEADS)
        nc.vector.tensor_mul(
            xt1_v, x_rem_v, EXP3[:, 0:4].to_broadcast((128, HEADS, HID))
        )
        hpre = persist.tile([128, OUT], f32, tag="hpre")
        hpre_v = hpre.rearrange("p (h c) -> p h c", h=HEADS)
        nc.vector.tensor_add(hpre_v, psagg[:, :, 0:HID], xt1_v)
        h_loc = persist.tile([128, OUT], f32, tag="h_loc")
        nc.vector.tensor_mul(
            h_loc.rearrange("p (h c) -> p h c", h=HEADS),
            hpre_v, asm[:, 12:16].to_broadcast((128, HEADS, HID)),
        )
        nc.vector.tensor_add(h_loc[:], h_loc[:], sb_Bg[:])
        nc.sync.dma_start(asmall[:], asm[:])

        # ---- local half of decoders ----
        hn_loc = layer_norm(h_loc, OUT, "hn")
        ha0, hb0 = transpose_h(hn_loc, 0)
        run_decoders_nt(0, ha0, hb0)

        nc.sync.dma_start(rec_l.rearrange("(a p) n -> p a n", p=128), rec_sb["l"][:])
        nc.sync.dma_start(rec_r.rearrange("(a p) n -> p a n", p=128), rec_sb["r"][:])

    nc.compile()
    return nc


_NC_CACHE = {}


def _get_nc():
    if "nc" not in _NC_CACHE:
        _NC_CACHE["nc"] = _build_nc()
    return _NC_CACHE["nc"]


# ---------------------------------------------------------------------------
# Host side


def _expected_edge_index():
    local = np.arange(L)
    s = np.repeat(local, L)
    d = np.tile(local, L)
    m = s != d
    src = np.concatenate([s[m], local, L + local, np.arange(N)])
    dst = np.concatenate([d[m], L + local, local, np.arange(N)])
    return np.stack([src, dst])


def _np_reference_fallback(node_features, gat, norm, dec_local, dec_remote,
                           node_types, edge_index):
    """Pure-numpy replica of the reference; used only if the edge structure
    is not the expected deterministic pattern."""
    x = node_features @ gat["W"]
    xh = x.reshape(N, HEADS, HID)
    a_src = (xh * gat["att_src"]).sum(-1)
    a_dst = (xh * gat["att_dst"]).sum(-1)
    src, dst = edge_index[0], edge_index[1]
    e = a_src[src] + a_dst[dst]
    e = np.where(e >= 0, e, NEG_SLOPE * e).astype(np.float32)
    emax = np.full((N, HEADS), -np.inf, np.float32)
    np.maximum.at(emax, dst, e)
    ex = np.exp(e - emax[dst])
    zs = np.zeros((N, HEADS), np.float32)
    np.add.at(zs, dst, ex)
    alpha = ex / zs[dst]
    msg = xh[src] * alpha[:, :, None]
    out = np.zeros((N, HEADS, HID), np.float32)
    np.add.at(out, dst, msg)
    out = out.reshape(N, OUT) + gat["bias"]

    def ln(v, g, b):
        m = v.mean(-1, keepdims=True)
        var = ((v - m) ** 2).mean(-1, keepdims=True)
        return (v - m) / np.sqrt(var + EPS) * g + b

    h = ln(out, norm["g"], norm["b"])

    def dec(v, p):
        t = ln(np.maximum(v @ p["W1"] + p["b1"], 0), p["g1"], p["b1n"])
        t = ln(np.maximum(t @ p["W2"] + p["b2"], 0), p["g2"], p["b2n"])
        return t @ p["W3"] + p["b3"]

    rl = dec(h, dec_local)
    rr = dec(h, dec_remote)
    rec = np.where((node_types == 1)[:, None], rl, rr)
    return rec.astype(np.float32), edge_index, alpha.astype(np.float32)


def _to_np(v):
    return {k: _to_np(x) for k, x in v.items()} if isinstance(v, dict) else np.asarray(v)


def kernel(node_features, gat, norm, dec_local, dec_remote, node_types,
           edge_index, trace=False):
    _ensure_axon_hooks_stub()
    node_features = np.asarray(node_features, np.float32)
    gat, norm = _to_np(gat), _to_np(norm)
    dec_local, dec_remote = _to_np(dec_local), _to_np(dec_remote)
    node_types_np = np.asarray(node_types)
    edge_index_np = np.asarray(edge_index)

    if not np.array_equal(edge_index_np.astype(np.int64), _expected_edge_index()):
        return _np_reference_fallback(
            node_features, gat, norm, dec_local, dec_remote,
            node_types_np, edge_index_np,
        )

    from concourse.bass_utils import run_bass_kernel_spmd

    f32 = np.float32
    W = gat["W"].astype(f32)
    att_src = gat["att_src"].astype(f32)
    att_dst = gat["att_dst"].astype(f32)
    Wsrc = (W.reshape(MAPPED, HEADS, HID) * att_src[None]).sum(-1)
    Wdst = (W.reshape(MAPPED, HEADS, HID) * att_dst[None]).sum(-1)
    Wg_ext = np.ascontiguousarray(np.concatenate([W, Wsrc, Wdst], axis=1))
    nfT = np.ascontiguousarray(node_features.T)

    # fold LN affine params into the following linear layer; biases
    # pre-broadcast to [128, w]
    def fold(dec):
        g, b = norm["g"].astype(f32), norm["b"].astype(f32)
        W1 = g[:, None] * dec["W1"]
        b1 = dec["b1"] + b @ dec["W1"]
        W2 = dec["g1"][:, None] * dec["W2"]
        b2 = dec["b2"] + dec["b1n"] @ dec["W2"]
        W3 = dec["g2"][:, None] * dec["W3"]
        b3 = dec["b3"] + dec["b2n"] @ dec["W3"]

        def bc(v):
            return np.ascontiguousarray(
                np.broadcast_to(v.reshape(1, -1).astype(f32), (128, v.size))
            )

        return (
            np.ascontiguousarray(W1, f32), bc(b1),
            np.ascontiguousarray(W2, f32), bc(b2),
            np.ascontiguousarray(W3, f32), bc(b3),
        )

    W1l, b1l, W2l, b2l, W3l, b3l = fold(dec_local)
    W1r, b1r, W2r, b2r, W3r, b3r = fold(dec_remote)

    shared = {
        "nfT": nfT, "Wg": Wg_ext,
        "bgat": gat["bias"].reshape(1, -1).astype(f32),
        "W1l": W1l, "b1l": b1l, "W2l": W2l, "b2l": b2l, "W3l": W3l, "b3l": b3l,
        "W1r": W1r, "b1r": b1r, "W2r": W2r, "b2r": b2r, "W3r": W3r, "b3r": b3r,
    }
    in_maps = []
    for c in range(NCORES):
        m = dict(shared)
        m["nfT_loc"] = np.ascontiguousarray(nfT[:, c * SH : (c + 1) * SH])
        m["nfT_rem"] = np.ascontiguousarray(nfT[:, L + c * SH : L + (c + 1) * SH])
        in_maps.append(m)

    nc = _get_nc()
    res = run_bass_kernel_spmd(nc, in_maps, core_ids=list(range(NCORES)), trace=trace)
    _NC_CACHE["last_results"] = res
    outs = res.results

    # ---- host unshard / assembly ----
    # alphaT_full[s, d, h] = exp(e) * R
    alphaT_full = np.empty((L, L, HEADS), np.float32)
    a_rl = np.empty((L, HEADS), np.float32)
    a_lr = np.empty((L, HEADS), np.float32)
    a_rr = np.empty((L, HEADS), np.float32)
    for c in range(NCORES):
        sl = slice(c * SH, (c + 1) * SH)
        sm = outs[c]["asmall"]
        a_rl[sl], a_lr[sl], a_rr[sl] = sm[:, 0:4], sm[:, 4:8], sm[:, 8:12]
        R = sm[:, 12:16]  # [d, h]
        blk = outs[c]["alpha_t"].reshape(128, HEADS, NLOC_T, SH)  # [p, h, t, d]
        # -> [s=t*128+p, d, h]
        blk = np.transpose(blk, (2, 0, 3, 1)).reshape(L, SH, HEADS)
        alphaT_full[:, sl, :] = blk * R[None, :, :]

    per_edge = alphaT_full.reshape(L * L, HEADS)
    mask = ~np.eye(L, dtype=bool).reshape(-1)
    clique = per_edge[mask]
    idx = np.arange(L)
    self_local = alphaT_full[idx, idx, :]
    alpha = np.concatenate(
        [clique, a_lr, a_rl, self_local, a_rr], axis=0
    ).astype(np.float32)

    rec_local = np.empty((N, IN_DIM), np.float32)
    rec_remote = np.empty((N, IN_DIM), np.float32)
    for c in range(NCORES):
        sl = slice(c * SH, (c + 1) * SH)
        slr = slice(L + c * SH, L + (c + 1) * SH)
        rec_local[sl] = outs[c]["rec_l"][0:SH]
        rec_local[slr] = outs[c]["rec_l"][SH : 2 * SH]
        rec_remote[sl] = outs[c]["rec_r"][0:SH]
        rec_remote[slr] = outs[c]["rec_r"][SH : 2 * SH]
    reconstructed = np.where((node_types_np == 1)[:, None], rec_local, rec_remote)

    return reconstructed.astype(np.float32), edge_index_np, alpha


# revision 17
# speedup vs baseline: 1.0883x; 1.0220x over previous
"""Self-contained Trainium2 (Bass/Tile) kernel for the DeviceGAT problem.

Computes, on 8 NeuronCores, the GAT layer + LayerNorm + two decoder MLPs of
reference.py, exploiting the deterministic graph structure:
  - edges = dense local clique (1024 local nodes, no self edges)
            + local->remote pairs (i -> L+i)
            + remote->local pairs (L+i -> i)
            + self loops for all 2048 nodes
  - so each local dst d has in-edges from all 1024 local nodes (incl. itself
    via the self loop) plus remote node L+d; each remote dst L+i has in-edges
    {i, L+i}.

Sharding: destination rows are sharded 8 ways (128 local dst rows + the
matching 128 remote dst rows per core).  Node features / params replicated.

Scores are built directly in transposed [src, dst] layout (a_dst broadcast by
a tiny selector matmul, a_src as the per-partition activation bias), so the
attention needs NO on-chip transposes.  exp(scores) is written to HBM
unnormalized together with the per-(dst, head) softmax reciprocal R; the host
applies R while permuting the dense block into the reference per-edge order.
The aggregation matmul contracts src on the partition axis (exb^T @ [x | 1])
in bf16, producing both the weighted message sum and the softmax denominator
for all four heads in one PSUM tile; per-head epilogue work is batched into a
handful of broadcast-AP vector ops.
"""

import os
import sys
import types
import numpy as np

# ---------------------------------------------------------------------------
# Problem constants (from the reference problem definition; deterministic).
L = 1024
N = 2 * L
MAPPED = 32
HID = 64
HEADS = 4
OUT = HID * HEADS        # 256
IN_DIM = 128
NEG_SLOPE = 0.2
EPS = 1e-5
NCORES = 8
SH = L // NCORES         # 128 dst rows per core
NLOC_T = L // 128        # 8 local node tiles


def _ensure_axon_hooks_stub():
    """run_bass_kernel_spmd(trace=True) imports antenv.axon_hooks; provide a
    graceful stub when the image lacks it so tracing degrades instead of
    crashing.  (Harness runs trace=False and never hits this, but be safe.)"""
    try:
        import antenv.axon_hooks  # noqa: F401
        return
    except Exception:
        pass
    try:
        import antenv
    except Exception:
        antenv = types.ModuleType("antenv")
        sys.modules["antenv"] = antenv
    mod = types.ModuleType("antenv.axon_hooks")
    mod._HOOK = None

    def set_axon_ntff_profile_hook(hook):
        mod._HOOK = hook

    def get_axon_ntff_profile_hook():
        if mod._HOOK is not None:
            return mod._HOOK
        so = "/opt/axon/libaxon_pjrt.so"
        if os.path.exists(so):
            import contextlib
            import ctypes

            try:
                lib = ctypes.CDLL(so)
            except OSError:
                return None
            if not hasattr(lib, "axon_start_nrt_profile"):
                return None
            lib.axon_start_nrt_profile.argtypes = [
                ctypes.POINTER(ctypes.c_int64),
                ctypes.c_size_t,
            ]
            lib.axon_start_nrt_profile.restype = ctypes.c_int64
            lib.axon_stop_nrt_profile.argtypes = [ctypes.c_char_p]
            lib.axon_stop_nrt_profile.restype = ctypes.c_int64

            @contextlib.contextmanager
            def _hook(output_dir, device_ids):
                import jax

                jax.devices()
                if device_ids:
                    ids = (ctypes.c_int64 * len(device_ids))(*device_ids)
                    rc = lib.axon_start_nrt_profile(ids, len(device_ids))
                else:
                    rc = lib.axon_start_nrt_profile(None, 0)
                if rc != 0:
                    raise RuntimeError(f"axon_start_nrt_profile rc={rc}")
                try:
                    yield
                finally:
                    n = lib.axon_stop_nrt_profile(str(output_dir).encode())
                    print(f"profile: {n} file(s) in {output_dir}", file=sys.stderr)

            mod._HOOK = _hook
            return mod._HOOK
        return None

    mod.set_axon_ntff_profile_hook = set_axon_ntff_profile_hook
    mod.get_axon_ntff_profile_hook = get_axon_ntff_profile_hook
    sys.modules["antenv.axon_hooks"] = mod


# ---------------------------------------------------------------------------
# Bass kernel builder


def _build_nc():
    from contextlib import ExitStack

    import concourse.bass as bass
    import concourse.tile as tile
    from concourse import bacc, mybir
    from concourse.masks import make_identity

    f32 = mybir.dt.float32
    bf16 = mybir.dt.bfloat16
    AF = mybir.ActivationFunctionType
    MUL = mybir.AluOpType.mult
    ADD = mybir.AluOpType.add
    WEXT = OUT + 2 * HEADS  # 264: [Wg | Wsrc | Wdst]

    nc = bacc.Bacc(
        trn_type="TRN2", target_bir_lowering=False, debug=False, num_devices=NCORES
    )

    # ---- I/O ----
    # nf_all: [32, N + SH + SH + WEXT] = nfT | nfT_loc | nfT_rem | Wg_ext
    NFC = N + 2 * SH + WEXT
    nf_all = nc.dram_tensor("nf_all", [MAPPED, NFC], f32, kind="ExternalInput")
    # dec_all: [128, 1792] = for each dec: W1(256) b1(128) W2(64) b2(64)
    # W3pad(128) b3(128); then Bg(256)
    DCC = 2 * (256 + 128 + 64 + 64 + 128 + 128) + OUT
    dec_all = nc.dram_tensor("dec_all", [128, DCC], f32, kind="ExternalInput")

    # dense unnormalized exp(scores), transposed layout, raw per-head blocks
    # [p, t, d]: src node s = t*128+p, dst column d
    alpha_t = nc.dram_tensor("alpha_t", [128, HEADS * L], f32, kind="ExternalOutput")
    # a_rl | a_lr | a_rr | R  (R = per-(d,h) softmax reciprocal)
    asmall = nc.dram_tensor("asmall", [SH, 16], f32, kind="ExternalOutput")
    rec_l = nc.dram_tensor("rec_l", [2 * SH, IN_DIM], f32, kind="ExternalOutput")
    rec_r = nc.dram_tensor("rec_r", [2 * SH, IN_DIM], f32, kind="ExternalOutput")

    with tile.TileContext(nc) as tc, ExitStack() as ctx:
        consts = ctx.enter_context(tc.tile_pool(name="consts", bufs=1))
        big = ctx.enter_context(tc.tile_pool(name="big", bufs=2))
        persist = ctx.enter_context(tc.tile_pool(name="persist", bufs=1))
        small = ctx.enter_context(tc.tile_pool(name="small", bufs=2))
        psB = ctx.enter_context(tc.tile_pool(name="psB", bufs=2, space="PSUM"))
        psG = ctx.enter_context(tc.tile_pool(name="psG", bufs=1, space="PSUM"))
        psT = ctx.enter_context(tc.tile_pool(name="psT", bufs=2, space="PSUM"))
        psX = ctx.enter_context(tc.tile_pool(name="psX", bufs=2, space="PSUM"))

        # ---- load constants (2 big DMAs) ----
        sb_nf = consts.tile([MAPPED, NFC], f32)
        nc.sync.dma_start(sb_nf[:], nf_all[:])
        sb_nfT = sb_nf[:, 0:N]
        sb_nfT_loc = sb_nf[:, N : N + SH]
        sb_nfT_rem = sb_nf[:, N + SH : N + 2 * SH]
        sb_Wg = sb_nf[:, N + 2 * SH : NFC]
        sb_dec = consts.tile([128, DCC], f32)
        nc.sync.dma_start(sb_dec[:], dec_all[:])
        sbd = {}
        off = 0
        for tag in ("l", "r"):
            sbd[tag] = {
                "W1": sb_dec[:, off : off + 256].rearrange("p (a n) -> p a n", a=2),
                "b1": sb_dec[:, off + 256 : off + 384],
                "W2": sb_dec[:, off + 384 : off + 448],
                "b2": sb_dec[:, off + 448 : off + 512],
                "W3": sb_dec[0:64, off + 512 : off + 640],
                "b3": sb_dec[:, off + 640 : off + 768],
            }
            off += 768

        eps_col = consts.tile([128, 1], f32)
        nc.vector.memset(eps_col[:], EPS)
        ident = consts.tile([128, 128], f32)
        make_identity(nc, ident[:])
        # head-selector: sel4[k, h*128+m] = (k == h); used as k=4 lhsT to
        # broadcast row h of a [4, x] tile across 128 partitions
        sel4 = consts.tile([HEADS, HEADS * 128], f32)
        nc.gpsimd.memset(sel4[:], 0.0)
        sel4v = sel4.rearrange("p (a b) -> p a b", a=HEADS)
        nc.gpsimd.affine_select(
            out=sel4v,
            in_=sel4v,
            compare_op=mybir.AluOpType.not_equal,
            fill=1.0,
            base=0,
            pattern=[[-1, HEADS], [0, 128]],
            channel_multiplier=1,
        )
        sb_Bg = sb_dec[:, 2 * 768 : 2 * 768 + OUT]

        # ---- projections on PE: x|a_src|a_dst per node tile ----
        # xbf_all[p, t, h, 0:64] = x, [..., 64] = 1.0  (bf16, agg rhs)
        xbf_all = persist.tile([128, NLOC_T, HEADS, HID + 1], bf16, tag="xbf")
        nc.vector.memset(xbf_all[:], 1.0)
        # a_src for all local tiles: [128, t, h]
        A_src = persist.tile([128, NLOC_T, HEADS], f32, tag="asrc")
        for t in range(NLOC_T):
            ps = psX.tile([128, WEXT], f32, tag="psx")
            nc.tensor.matmul(ps[:], sb_nfT[:, t * 128 : (t + 1) * 128], sb_Wg[:])
            nc.vector.tensor_copy(
                xbf_all[:, t, :, 0:HID],
                ps[:, 0:OUT].rearrange("p (h c) -> p h c", h=HEADS),
            )
            nc.scalar.copy(A_src[:, t, :], ps[:, OUT : OUT + HEADS])

        ps = psX.tile([128, WEXT], f32, tag="psx")
        nc.tensor.matmul(ps[:], sb_nfT_rem[:], sb_Wg[:])
        x_rem = persist.tile([128, OUT], f32, tag="xrem")
        nc.scalar.copy(x_rem[:], ps[:, 0:OUT])
        A_rem = persist.tile([128, 2 * HEADS], f32, tag="arem")
        nc.vector.tensor_copy(A_rem[:], ps[:, OUT:WEXT])

        ps = psX.tile([128, WEXT], f32, tag="psx")
        nc.tensor.matmul(ps[:], sb_nfT_loc[:], sb_Wg[:])
        x_shard = persist.tile([128, OUT], f32, tag="xshard")
        nc.vector.tensor_copy(x_shard[:], ps[:, 0:OUT])
        A_loc = persist.tile([128, 2 * HEADS], f32, tag="aloc")
        nc.scalar.copy(A_loc[:], ps[:, OUT:WEXT])

        # a_dst of the shard as rows: [4, 128]
        psd = psX.tile([128, WEXT], f32, tag="psx")
        nc.tensor.matmul(
            psd[0:HEADS, 0:SH], sb_Wg[:, OUT + HEADS : WEXT], sb_nfT_loc[:]
        )
        a_dstT = consts.tile([HEADS, SH], f32)
        nc.scalar.copy(a_dstT[:], psd[0:HEADS, 0:SH])

        # ---- special-edge scores: e_rl | e_lr | e_rr  [128, 12] ----
        E3 = persist.tile([128, 12], f32, tag="E3")
        nc.vector.tensor_add(E3[:, 0:4], A_rem[:, 0:4], A_loc[:, 4:8])
        nc.vector.tensor_add(E3[:, 4:8], A_loc[:, 0:4], A_rem[:, 4:8])
        nc.vector.tensor_add(E3[:, 8:12], A_rem[:, 0:4], A_rem[:, 4:8])
        LR3 = persist.tile([128, 12], f32, tag="LR3")
        nc.scalar.activation(LR3[:], E3[:], AF.Prelu, alpha=NEG_SLOPE)
        EXP3 = persist.tile([128, 12], f32, tag="EXP3")
        nc.scalar.activation(EXP3[:], LR3[:], AF.Exp)

        # remote-dst softmax (2 edges) and h_rem (batched broadcast ops)
        asm = persist.tile([128, 16], f32, tag="asm")  # a_rl | a_lr | a_rr | R
        Zr = persist.tile([128, 4], f32, tag="Zr")
        nc.vector.tensor_add(Zr[:], EXP3[:, 4:8], EXP3[:, 8:12])
        Rr = persist.tile([128, 4], f32, tag="Rr")
        nc.vector.reciprocal(Rr[:], Zr[:])
        nc.vector.tensor_mul(asm[:, 4:8], EXP3[:, 4:8], Rr[:])
        nc.vector.tensor_mul(asm[:, 8:12], EXP3[:, 8:12], Rr[:])

        x_shard_v = x_shard.rearrange("p (h c) -> p h c", h=HEADS)
        x_rem_v = x_rem.rearrange("p (h c) -> p h c", h=HEADS)
        hrem_t1 = persist.tile([128, OUT], f32, tag="hrem_t1")
        nc.vector.tensor_mul(
            hrem_t1.rearrange("p (h c) -> p h c", h=HEADS),
            x_shard_v, asm[:, 4:8].to_broadcast((128, HEADS, HID)),
        )
        hrem_t2 = persist.tile([128, OUT], f32, tag="hrem_t2")
        nc.vector.tensor_mul(
            hrem_t2.rearrange("p (h c) -> p h c", h=HEADS),
            x_rem_v, asm[:, 8:12].to_broadcast((128, HEADS, HID)),
        )
        h_rem = persist.tile([128, OUT], f32, tag="h_rem")
        nc.vector.tensor_add(h_rem[:], hrem_t1[:], hrem_t2[:])
        nc.vector.tensor_add(h_rem[:], h_rem[:], sb_Bg[:])

        # ---- layernorm helper (stats DVE, sqrt ACT, apply DVE) ----
        BNS = nc.vector.BN_STATS_DIM
        BNA = nc.vector.BN_AGGR_DIM

        def layer_norm(x_t, width, tagp):
            st = small.tile([128, BNS], f32, tag="bnst")
            nc.vector.bn_stats(st[:], x_t[:, 0:width])
            mv = small.tile([128, BNA], f32, tag="bnmv")
            nc.vector.bn_aggr(mv[:], st[:])
            sdev = small.tile([128, 1], f32, tag="sdev")
            nc.scalar.activation(sdev[:], mv[:, 1:2], AF.Sqrt, bias=eps_col[:])
            rstd = small.tile([128, 1], f32, tag="rstd")
            nc.vector.reciprocal(rstd[:], sdev[:])
            negm = small.tile([128, 1], f32, tag="negm")
            nc.vector.tensor_scalar(negm[:], mv[:, 0:1], -1.0, None, op0=MUL)
            out_t = big.tile([128, width], f32, tag=tagp)
            nc.vector.tensor_scalar(
                out_t[:], x_t[:, 0:width], negm[:], rstd[:], op0=ADD, op1=MUL
            )
            return out_t

        # ---- decoder machinery (f32) ----
        rec_sb = {
            "l": persist.tile([128, 2, IN_DIM], f32, tag="recl", name="recl"),
            "r": persist.tile([128, 2, IN_DIM], f32, tag="recr", name="recr"),
        }

        def transpose_h(hn_t, nt):
            psTt = psT.tile([128, 4, 128], f32, tag="psT")
            nc.tensor.transpose(psTt[:, 0, :], hn_t[:, 0:128], ident[:])
            nc.tensor.transpose(psTt[:, 1, :], hn_t[:, 128:256], ident[:])
            ha = big.tile([128, 128], f32, tag=f"hTa{nt}", name=f"hTa{nt}")
            hb = big.tile([128, 128], f32, tag=f"hTb{nt}", name=f"hTb{nt}")
            if nt == 0:
                nc.scalar.copy(ha[:], psTt[:, 0, :])
                nc.vector.tensor_copy(hb[:], psTt[:, 1, :])
            else:
                nc.vector.tensor_copy(ha[:], psTt[:, 0, :])
                nc.scalar.copy(hb[:], psTt[:, 1, :])
            return ha, hb

        def run_decoders_nt(nt, ha, hb):
            for di, tag in enumerate(("l", "r")):
                p = sbd[tag]
                rec_t = rec_sb[tag]
                ps1 = psX.tile([128, WEXT], f32, tag="psx")
                nc.tensor.matmul(ps1[:, 0:128], ha[:], p["W1"][:, 0, :],
                                 start=True, stop=False)
                nc.tensor.matmul(ps1[:, 0:128], hb[:], p["W1"][:, 1, :],
                                 start=False, stop=True)
                s1 = big.tile([128, 128], f32, tag="s1")
                nc.vector.tensor_add(s1[:], ps1[:, 0:128], p["b1"][:])
                r1 = big.tile([128, 128], f32, tag="r1")
                nc.scalar.activation(r1[:], s1[:], AF.Relu)
                n1 = layer_norm(r1, 128, "n1")
                psn = psT.tile([128, 4, 128], f32, tag="psT")
                nc.tensor.transpose(psn[:, 0, :], n1[:], ident[:])
                n1T = big.tile([128, 128], f32, tag="n1T")
                if (nt + di) % 2 == 0:
                    nc.scalar.copy(n1T[:], psn[:, 0, :])
                else:
                    nc.vector.tensor_copy(n1T[:], psn[:, 0, :])

                ps2 = psX.tile([128, WEXT], f32, tag="psx")
                nc.tensor.matmul(ps2[:, 0:64], n1T[:], p["W2"][:],
                                 start=True, stop=True)
                s2 = big.tile([128, 64], f32, tag="s2")
                nc.vector.tensor_add(s2[:], ps2[:, 0:64], p["b2"][:])
                r2 = big.tile([128, 64], f32, tag="r2")
                nc.scalar.activation(r2[:], s2[:], AF.Relu)
                n2 = layer_norm(r2, 64, "n2")
                psn2 = psT.tile([128, 4, 128], f32, tag="psT")
                nc.tensor.transpose(psn2[0:64, 0, :], n2[:, 0:64], ident[:])
                n2T = big.tile([64, 128], f32, tag="n2T")
                if (nt + di) % 2 == 0:
                    nc.vector.tensor_copy(n2T[:], psn2[0:64, 0, :])
                else:
                    nc.scalar.copy(n2T[:], psn2[0:64, 0, :])

                ps3 = psX.tile([128, WEXT], f32, tag="psx")
                nc.tensor.matmul(ps3[:, 0:IN_DIM], n2T[:], p["W3"][:],
                                 start=True, stop=True)
                nc.vector.tensor_add(rec_t[:, nt, :], ps3[:, 0:IN_DIM], p["b3"][:])

        # remote half of the decoders runs during the attention head loop
        hn_rem = layer_norm(h_rem, OUT, "hn")
        ha1, hb1 = transpose_h(hn_rem, 1)
        run_decoders_nt(1, ha1, hb1)

        # ---- dense attention per head (transposed [s, d] layout) ----
        # all 4 a_dst broadcasts in one PSUM bank
        psBh = psB.tile([128, HEADS, SH], f32, tag="Bh")
        for h in range(HEADS):
            nc.tensor.matmul(
                psBh[:, h, :], sel4[:, h * 128 : (h + 1) * 128], a_dstT[:]
            )
        # all 4 heads accumulate into one PSUM tile: [:, h, 0:64] = msg sum,
        # [:, h, 64] = sum(exp)
        psagg = psG.tile([128, HEADS, HID + 1], f32, tag="agg")
        for h in range(HEADS):
            # S[p, t, d] = a_src[t*128+p, h] + a_dst[d]  (one broadcast add)
            S_h = big.tile([128, NLOC_T, SH], f32, tag="eT")
            bh_slice = psBh[:, h, :]
            bh_bcast = bass.AP(
                tensor=bh_slice.tensor,
                offset=bh_slice.offset,
                ap=[bh_slice.ap[0], [0, NLOC_T], bh_slice.ap[1]],
            )
            nc.vector.tensor_add(
                S_h[:], bh_bcast, A_src[:, :, h].to_broadcast((128, NLOC_T, SH))
            )
            ex_T = big.tile([128, NLOC_T, SH], f32, tag="exT")
            eflat = S_h.rearrange("p a b -> p (a b)")
            nc.scalar.activation(eflat, eflat, AF.Prelu, alpha=NEG_SLOPE)
            nc.scalar.activation(
                ex_T.rearrange("p a b -> p (a b)"), eflat, AF.Exp
            )
            # unnormalized exp block straight to HBM (host normalizes)
            nc.sync.dma_start(
                alpha_t[:, h * L : (h + 1) * L],
                ex_T.rearrange("p a b -> p (a b)"),
            )
            exb = big.tile([128, NLOC_T, SH], bf16, tag="exb")
            nc.vector.tensor_copy(exb[:], ex_T[:])

            # aggregation + Z in one accumulation: exb[s,:]^T @ [x_h | 1]
            for t in range(NLOC_T):
                nc.tensor.matmul(
                    psagg[:, h, :],
                    exb[:, t, :],
                    xbf_all[:, t, h, :],
                    start=(t == 0),
                    stop=(t == NLOC_T - 1),
                )

        # ---- batched per-head epilogue ----
        # Zf = psagg[:, :, 64] + exp(e_rl);  R = 1/Zf
        Zf = small.tile([128, HEADS], f32, tag="Zf")
        nc.vector.tensor_add(Zf[:], psagg[:, :, HID], EXP3[:, 0:4])
        nc.vector.reciprocal(asm[:, 12:16], Zf[:])
        nc.vector.tensor_mul(asm[:, 0:4], EXP3[:, 0:4], asm[:, 12:16])
        # h_loc = (agg + exp(e_rl)*x_rem) * R + bgat
        xt1 = persist.tile([128, OUT], f32, tag="xt1")
        xt1_v = xt1.rearrange("p (h c) -> p h c", h=HEADS)
        nc.vector.tensor_mul(
            xt1_v, x_rem_v, EXP3[:, 0:4].to_broadcast((128, HEADS, HID))
        )
        hpre = persist.tile([128, OUT], f32, tag="hpre")
        hpre_v = hpre.rearrange("p (h c) -> p h c", h=HEADS)
        nc.vector.tensor_add(hpre_v, psagg[:, :, 0:HID], xt1_v)
        h_loc = persist.tile([128, OUT], f32, tag="h_loc")
        nc.vector.tensor_mul(
            h_loc.rearrange("p (h c) -> p h c", h=HEADS),
            hpre_v, asm[:, 12:16].to_broadcast((128, HEADS, HID)),
        )
        nc.vector.tensor_add(h_loc[:], h_loc[:], sb_Bg[:])
        nc.sync.dma_start(asmall[:], asm[:])

        # ---- local half of decoders ----
        hn_loc = layer_norm(h_loc, OUT, "hn")
        ha0, hb0 = transpose_h(hn_loc, 0)
        run_decoders_nt(0, ha0, hb0)

        nc.sync.dma_start(rec_l.rearrange("(a p) n -> p a n", p=128), rec_sb["l"][:])
        nc.sync.dma_start(rec_r.rearrange("(a p) n -> p a n", p=128), rec_sb["r"][:])

    nc.compile()
    return nc


_NC_CACHE = {}


def _get_nc():
    if "nc" not in _NC_CACHE:
        _NC_CACHE["nc"] = _build_nc()
    return _NC_CACHE["nc"]


# ---------------------------------------------------------------------------
# Host side


def _expected_edge_index():
    local = np.arange(L)
    s = np.repeat(local, L)
    d = np.tile(local, L)
    m = s != d
    src = np.concatenate([s[m], local, L + local, np.arange(N)])
    dst = np.concatenate([d[m], L + local, local, np.arange(N)])
    return np.stack([src, dst])


def _np_reference_fallback(node_features, gat, norm, dec_local, dec_remote,
                           node_types, edge_index):
    """Pure-numpy replica of the reference; used only if the edge structure
    is not the expected deterministic pattern."""
    x = node_features @ gat["W"]
    xh = x.reshape(N, HEADS, HID)
    a_src = (xh * gat["att_src"]).sum(-1)
    a_dst = (xh * gat["att_dst"]).sum(-1)
    src, dst = edge_index[0], edge_index[1]
    e = a_src[src] + a_dst[dst]
    e = np.where(e >= 0, e, NEG_SLOPE * e).astype(np.float32)
    emax = np.full((N, HEADS), -np.inf, np.float32)
    np.maximum.at(emax, dst, e)
    ex = np.exp(e - emax[dst])
    zs = np.zeros((N, HEADS), np.float32)
    np.add.at(zs, dst, ex)
    alpha = ex / zs[dst]
    msg = xh[src] * alpha[:, :, None]
    out = np.zeros((N, HEADS, HID), np.float32)
    np.add.at(out, dst, msg)
    out = out.reshape(N, OUT) + gat["bias"]

    def ln(v, g, b):
        m = v.mean(-1, keepdims=True)
        var = ((v - m) ** 2).mean(-1, keepdims=True)
        return (v - m) / np.sqrt(var + EPS) * g + b

    h = ln(out, norm["g"], norm["b"])

    def dec(v, p):
        t = ln(np.maximum(v @ p["W1"] + p["b1"], 0), p["g1"], p["b1n"])
        t = ln(np.maximum(t @ p["W2"] + p["b2"], 0), p["g2"], p["b2n"])
        return t @ p["W3"] + p["b3"]

    rl = dec(h, dec_local)
    rr = dec(h, dec_remote)
    rec = np.where((node_types == 1)[:, None], rl, rr)
    return rec.astype(np.float32), edge_index, alpha.astype(np.float32)


def _to_np(v):
    return {k: _to_np(x) for k, x in v.items()} if isinstance(v, dict) else np.asarray(v)


def kernel(node_features, gat, norm, dec_local, dec_remote, node_types,
           edge_index, trace=False):
    _ensure_axon_hooks_stub()
    node_features = np.asarray(node_features, np.float32)
    gat, norm = _to_np(gat), _to_np(norm)
    dec_local, dec_remote = _to_np(dec_local), _to_np(dec_remote)
    node_types_np = np.asarray(node_types)
    edge_index_np = np.asarray(edge_index)

    if not np.array_equal(edge_index_np.astype(np.int64), _expected_edge_index()):
        return _np_reference_fallback(
            node_features, gat, norm, dec_local, dec_remote,
            node_types_np, edge_index_np,
        )

    from concourse.bass_utils import run_bass_kernel_spmd

    f32 = np.float32
    W = gat["W"].astype(f32)
    att_src = gat["att_src"].astype(f32)
    att_dst = gat["att_dst"].astype(f32)
    Wsrc = (W.reshape(MAPPED, HEADS, HID) * att_src[None]).sum(-1)
    Wdst = (W.reshape(MAPPED, HEADS, HID) * att_dst[None]).sum(-1)
    Wg_ext = np.ascontiguousarray(np.concatenate([W, Wsrc, Wdst], axis=1))
    nfT = np.ascontiguousarray(node_features.T)

    # fold LN affine params into the following linear layer; biases
    # pre-broadcast to [128, w]
    def fold(dec):
        g, b = norm["g"].astype(f32), norm["b"].astype(f32)
        W1 = g[:, None] * dec["W1"]
        b1 = dec["b1"] + b @ dec["W1"]
        W2 = dec["g1"][:, None] * dec["W2"]
        b2 = dec["b2"] + dec["b1n"] @ dec["W2"]
        W3 = dec["g2"][:, None] * dec["W3"]
        b3 = dec["b3"] + dec["b2n"] @ dec["W3"]

        def bc(v):
            return np.ascontiguousarray(
                np.broadcast_to(v.reshape(1, -1).astype(f32), (128, v.size))
            )

        return (
            np.ascontiguousarray(W1, f32), bc(b1),
            np.ascontiguousarray(W2, f32), bc(b2),
            np.ascontiguousarray(W3, f32), bc(b3),
        )

    # dec_all: per dec: W1(rearranged, 256) b1(128) W2(64) b2(64) W3pad(128)
    # b3(128); then Bg(256)
    def pack_dec(dec):
        W1, b1, W2, b2, W3, b3 = fold(dec)
        W1r_ = W1.reshape(2, 128, 128).transpose(1, 0, 2).reshape(128, 256)
        W3p = np.zeros((128, 128), f32)
        W3p[0:64] = W3
        return np.concatenate([W1r_, b1, W2, b2, W3p, b3], axis=1)

    Bg = np.broadcast_to(gat["bias"].reshape(1, -1).astype(f32), (128, OUT))
    dec_all = np.ascontiguousarray(
        np.concatenate([pack_dec(dec_local), pack_dec(dec_remote), Bg], axis=1),
        f32,
    )

    in_maps = []
    for c in range(NCORES):
        nf_all = np.concatenate(
            [
                nfT,
                nfT[:, c * SH : (c + 1) * SH],
                nfT[:, L + c * SH : L + (c + 1) * SH],
                Wg_ext,
            ],
            axis=1,
        )
        in_maps.append({"nf_all": np.ascontiguousarray(nf_all), "dec_all": dec_all})

    nc = _get_nc()
    res = run_bass_kernel_spmd(nc, in_maps, core_ids=list(range(NCORES)), trace=trace)
    _NC_CACHE["last_results"] = res
    outs = res.results

    # ---- host unshard / assembly ----
    # alphaT_full[s, d, h] = exp(e) * R
    alphaT_full = np.empty((L, L, HEADS), np.float32)
    a_rl = np.empty((L, HEADS), np.float32)
    a_lr = np.empty((L, HEADS), np.float32)
    a_rr = np.empty((L, HEADS), np.float32)
    for c in range(NCORES):
        sl = slice(c * SH, (c + 1) * SH)
        sm = outs[c]["asmall"]
        a_rl[sl], a_lr[sl], a_rr[sl] = sm[:, 0:4], sm[:, 4:8], sm[:, 8:12]
        R = sm[:, 12:16]  # [d, h]
        blk = outs[c]["alpha_t"].reshape(128, HEADS, NLOC_T, SH)  # [p, h, t, d]
        # -> [s=t*128+p, d, h]
        blk = np.transpose(blk, (2, 0, 3, 1)).reshape(L, SH, HEADS)
        alphaT_full[:, sl, :] = blk * R[None, :, :]

    per_edge = alphaT_full.reshape(L * L, HEADS)
    mask = ~np.eye(L, dtype=bool).reshape(-1)
    clique = per_edge[mask]
    idx = np.arange(L)
    self_local = alphaT_full[idx, idx, :]
    alpha = np.concatenate(
        [clique, a_lr, a_rl, self_local, a_rr], axis=0
    ).astype(np.float32)

    rec_local = np.empty((N, IN_DIM), np.float32)
    rec_remote = np.empty((N, IN_DIM), np.float32)
    for c in range(NCORES):
        sl = slice(c * SH, (c + 1) * SH)
        slr = slice(L + c * SH, L + (c + 1) * SH)
        rec_local[sl] = outs[c]["rec_l"][0:SH]
        rec_local[slr] = outs[c]["rec_l"][SH : 2 * SH]
        rec_remote[sl] = outs[c]["rec_r"][0:SH]
        rec_remote[slr] = outs[c]["rec_r"][SH : 2 * SH]
    reconstructed = np.where((node_types_np == 1)[:, None], rec_local, rec_remote)

    return reconstructed.astype(np.float32), edge_index_np, alpha


# revision 19
# speedup vs baseline: 1.2283x; 1.1286x over previous
"""Self-contained Trainium2 (Bass/Tile) kernel for the DeviceGAT problem.

Computes, on 8 NeuronCores, the GAT layer + LayerNorm + two decoder MLPs of
reference.py, exploiting the deterministic graph structure:
  - edges = dense local clique (1024 local nodes, no self edges)
            + local->remote pairs (i -> L+i)
            + remote->local pairs (L+i -> i)
            + self loops for all 2048 nodes
  - so each local dst d has in-edges from all 1024 local nodes (incl. itself
    via the self loop) plus remote node L+d; each remote dst L+i has in-edges
    {i, L+i}.

Sharding: destination rows are sharded 8 ways (128 local dst rows + the
matching 128 remote dst rows per core).  Node features / params replicated.

Scores are built directly in transposed [src, dst] layout (a_dst broadcast by
a tiny selector matmul, a_src as the per-partition activation bias), so the
attention needs NO on-chip transposes.  exp(scores) is written to HBM
unnormalized together with the per-(dst, head) softmax reciprocal R; the host
applies R while permuting the dense block into the reference per-edge order.
The aggregation matmul contracts src on the partition axis (exb^T @ [x | 1])
in bf16, producing both the weighted message sum and the softmax denominator
for all four heads in one PSUM tile; per-head epilogue work is batched into a
handful of broadcast-AP vector ops.
"""

import os
import sys
import types
import numpy as np

# ---------------------------------------------------------------------------
# Problem constants (from the reference problem definition; deterministic).
L = 1024
N = 2 * L
MAPPED = 32
HID = 64
HEADS = 4
OUT = HID * HEADS        # 256
IN_DIM = 128
NEG_SLOPE = 0.2
EPS = 1e-5
NCORES = 8
SH = L // NCORES         # 128 dst rows per core
NLOC_T = L // 128        # 8 local node tiles


def _ensure_axon_hooks_stub():
    """run_bass_kernel_spmd(trace=True) imports antenv.axon_hooks; provide a
    graceful stub when the image lacks it so tracing degrades instead of
    crashing.  (Harness runs trace=False and never hits this, but be safe.)"""
    try:
        import antenv.axon_hooks  # noqa: F401
        return
    except Exception:
        pass
    try:
        import antenv
    except Exception:
        antenv = types.ModuleType("antenv")
        sys.modules["antenv"] = antenv
    mod = types.ModuleType("antenv.axon_hooks")
    mod._HOOK = None

    def set_axon_ntff_profile_hook(hook):
        mod._HOOK = hook

    def get_axon_ntff_profile_hook():
        if mod._HOOK is not None:
            return mod._HOOK
        so = "/opt/axon/libaxon_pjrt.so"
        if os.path.exists(so):
            import contextlib
            import ctypes

            try:
                lib = ctypes.CDLL(so)
            except OSError:
                return None
            if not hasattr(lib, "axon_start_nrt_profile"):
                return None
            lib.axon_start_nrt_profile.argtypes = [
                ctypes.POINTER(ctypes.c_int64),
                ctypes.c_size_t,
            ]
            lib.axon_start_nrt_profile.restype = ctypes.c_int64
            lib.axon_stop_nrt_profile.argtypes = [ctypes.c_char_p]
            lib.axon_stop_nrt_profile.restype = ctypes.c_int64

            @contextlib.contextmanager
            def _hook(output_dir, device_ids):
                import jax

                jax.devices()
                if device_ids:
                    ids = (ctypes.c_int64 * len(device_ids))(*device_ids)
                    rc = lib.axon_start_nrt_profile(ids, len(device_ids))
                else:
                    rc = lib.axon_start_nrt_profile(None, 0)
                if rc != 0:
                    raise RuntimeError(f"axon_start_nrt_profile rc={rc}")
                try:
                    yield
                finally:
                    n = lib.axon_stop_nrt_profile(str(output_dir).encode())
                    print(f"profile: {n} file(s) in {output_dir}", file=sys.stderr)

            mod._HOOK = _hook
            return mod._HOOK
        return None

    mod.set_axon_ntff_profile_hook = set_axon_ntff_profile_hook
    mod.get_axon_ntff_profile_hook = get_axon_ntff_profile_hook
    sys.modules["antenv.axon_hooks"] = mod


# ---------------------------------------------------------------------------
# Bass kernel builder


def _build_nc():
    from contextlib import ExitStack

    import concourse.bass as bass
    import concourse.tile as tile
    from concourse import bacc, mybir
    from concourse.masks import make_identity

    f32 = mybir.dt.float32
    bf16 = mybir.dt.bfloat16
    AF = mybir.ActivationFunctionType
    MUL = mybir.AluOpType.mult
    ADD = mybir.AluOpType.add
    WEXT = OUT + 2 * HEADS  # 264: [Wg | Wsrc | Wdst]

    nc = bacc.Bacc(
        trn_type="TRN2", target_bir_lowering=False, debug=False, num_devices=NCORES
    )

    # ---- I/O ----
    # nf_all: [32, N + SH + SH + WEXT] = nfT | nfT_loc | nfT_rem | Wg_ext
    NFC = N + 2 * SH + WEXT
    nf_all = nc.dram_tensor("nf_all", [MAPPED, NFC], f32, kind="ExternalInput")
    # dec_all: [128, 1792] = for each dec: W1(256) b1(128) W2(64) b2(64)
    # W3pad(128) b3(128); then Bg(256)
    DCC = 2 * (256 + 128 + 64 + 64 + 128 + 128) + OUT
    dec_all = nc.dram_tensor("dec_all", [128, DCC], f32, kind="ExternalInput")

    # dense unnormalized exp(scores), transposed layout, raw per-head blocks
    # [p, t, d]: src node s = t*128+p, dst column d
    alpha_t = nc.dram_tensor("alpha_t", [128, HEADS * L], f32, kind="ExternalOutput")
    # a_rl | a_lr | a_rr | R  (R = per-(d,h) softmax reciprocal)
    asmall = nc.dram_tensor("asmall", [SH, 16], f32, kind="ExternalOutput")
    rec_l = nc.dram_tensor("rec_l", [2 * SH, IN_DIM], f32, kind="ExternalOutput")
    rec_r = nc.dram_tensor("rec_r", [2 * SH, IN_DIM], f32, kind="ExternalOutput")

    with tile.TileContext(nc) as tc, ExitStack() as ctx:
        consts = ctx.enter_context(tc.tile_pool(name="consts", bufs=1))
        big = ctx.enter_context(tc.tile_pool(name="big", bufs=3))
        persist = ctx.enter_context(tc.tile_pool(name="persist", bufs=1))
        small = ctx.enter_context(tc.tile_pool(name="small", bufs=2))
        psB = ctx.enter_context(tc.tile_pool(name="psB", bufs=2, space="PSUM"))
        psG = ctx.enter_context(tc.tile_pool(name="psG", bufs=1, space="PSUM"))
        psT = ctx.enter_context(tc.tile_pool(name="psT", bufs=2, space="PSUM"))
        psX = ctx.enter_context(tc.tile_pool(name="psX", bufs=2, space="PSUM"))

        # ---- load constants (2 big DMAs) ----
        sb_nf = consts.tile([MAPPED, NFC], f32)
        nc.sync.dma_start(sb_nf[:], nf_all[:])
        sb_nfT = sb_nf[:, 0:N]
        sb_nfT_loc = sb_nf[:, N : N + SH]
        sb_nfT_rem = sb_nf[:, N + SH : N + 2 * SH]
        sb_Wg = sb_nf[:, N + 2 * SH : NFC]
        sb_dec = consts.tile([128, DCC], f32)
        nc.sync.dma_start(sb_dec[:], dec_all[:])
        # combined l|r decoder params
        dW1 = sb_dec[:, 0:512].rearrange("p (a n) -> p a n", a=2)  # [128,2,256]
        db1 = sb_dec[:, 512:768]
        dW2 = sb_dec[:, 768:896]      # cols 0:64 = W2l, 64:128 = W2r
        db2 = sb_dec[:, 896:1024]
        dW3 = sb_dec[0:64, 1024:1280]  # cols 0:128 = W3l, 128:256 = W3r
        db3 = sb_dec[:, 1280:1536]

        eps_col = consts.tile([128, 1], f32)
        nc.vector.memset(eps_col[:], EPS)
        ident = consts.tile([128, 128], f32)
        make_identity(nc, ident[:])
        # head-selector: sel4[k, h*128+m] = (k == h); used as k=4 lhsT to
        # broadcast row h of a [4, x] tile across 128 partitions
        sel4 = consts.tile([HEADS, HEADS * 128], f32)
        nc.gpsimd.memset(sel4[:], 0.0)
        sel4v = sel4.rearrange("p (a b) -> p a b", a=HEADS)
        nc.gpsimd.affine_select(
            out=sel4v,
            in_=sel4v,
            compare_op=mybir.AluOpType.not_equal,
            fill=1.0,
            base=0,
            pattern=[[-1, HEADS], [0, 128]],
            channel_multiplier=1,
        )
        sb_Bg = sb_dec[:, 1536 : 1536 + OUT]

        # ---- projections on PE: x|a_src|a_dst per node tile ----
        # xbf_all[p, t, h, 0:64] = x, [..., 64] = 1.0  (bf16, agg rhs)
        xbf_all = persist.tile([128, NLOC_T, HEADS, HID + 1], bf16, tag="xbf")
        nc.vector.memset(xbf_all[:], 1.0)
        # a_src for all local tiles: [128, t, h]
        A_src = persist.tile([128, NLOC_T, HEADS], f32, tag="asrc")
        for t in range(NLOC_T):
            ps = psX.tile([128, WEXT], f32, tag="psx")
            nc.tensor.matmul(ps[:], sb_nfT[:, t * 128 : (t + 1) * 128], sb_Wg[:])
            nc.vector.tensor_copy(
                xbf_all[:, t, :, 0:HID],
                ps[:, 0:OUT].rearrange("p (h c) -> p h c", h=HEADS),
            )
            nc.scalar.copy(A_src[:, t, :], ps[:, OUT : OUT + HEADS])

        ps = psX.tile([128, WEXT], f32, tag="psx")
        nc.tensor.matmul(ps[:], sb_nfT_rem[:], sb_Wg[:])
        x_rem = persist.tile([128, OUT], f32, tag="xrem")
        nc.scalar.copy(x_rem[:], ps[:, 0:OUT])
        A_rem = persist.tile([128, 2 * HEADS], f32, tag="arem")
        nc.vector.tensor_copy(A_rem[:], ps[:, OUT:WEXT])

        ps = psX.tile([128, WEXT], f32, tag="psx")
        nc.tensor.matmul(ps[:], sb_nfT_loc[:], sb_Wg[:])
        x_shard = persist.tile([128, OUT], f32, tag="xshard")
        nc.vector.tensor_copy(x_shard[:], ps[:, 0:OUT])
        A_loc = persist.tile([128, 2 * HEADS], f32, tag="aloc")
        nc.scalar.copy(A_loc[:], ps[:, OUT:WEXT])

        # a_dst of the shard as rows: [4, 128]
        psd = psX.tile([128, WEXT], f32, tag="psx")
        nc.tensor.matmul(
            psd[0:HEADS, 0:SH], sb_Wg[:, OUT + HEADS : WEXT], sb_nfT_loc[:]
        )
        a_dstT = consts.tile([HEADS, SH], f32)
        nc.scalar.copy(a_dstT[:], psd[0:HEADS, 0:SH])

        # ---- special-edge scores: e_rl | e_lr | e_rr  [128, 12] ----
        E3 = persist.tile([128, 12], f32, tag="E3")
        nc.vector.tensor_add(E3[:, 0:4], A_rem[:, 0:4], A_loc[:, 4:8])
        nc.vector.tensor_add(E3[:, 4:8], A_loc[:, 0:4], A_rem[:, 4:8])
        nc.vector.tensor_add(E3[:, 8:12], A_rem[:, 0:4], A_rem[:, 4:8])
        LR3 = persist.tile([128, 12], f32, tag="LR3")
        nc.scalar.activation(LR3[:], E3[:], AF.Prelu, alpha=NEG_SLOPE)
        EXP3 = persist.tile([128, 12], f32, tag="EXP3")
        nc.scalar.activation(EXP3[:], LR3[:], AF.Exp)

        # remote-dst softmax (2 edges) and h_rem (batched broadcast ops)
        asm = persist.tile([128, 16], f32, tag="asm")  # a_rl | a_lr | a_rr | R
        Zr = persist.tile([128, 4], f32, tag="Zr")
        nc.vector.tensor_add(Zr[:], EXP3[:, 4:8], EXP3[:, 8:12])
        Rr = persist.tile([128, 4], f32, tag="Rr")
        nc.vector.reciprocal(Rr[:], Zr[:])
        nc.vector.tensor_mul(asm[:, 4:8], EXP3[:, 4:8], Rr[:])
        nc.vector.tensor_mul(asm[:, 8:12], EXP3[:, 8:12], Rr[:])

        x_shard_v = x_shard.rearrange("p (h c) -> p h c", h=HEADS)
        x_rem_v = x_rem.rearrange("p (h c) -> p h c", h=HEADS)
        hrem_t1 = persist.tile([128, OUT], f32, tag="hrem_t1")
        nc.vector.tensor_mul(
            hrem_t1.rearrange("p (h c) -> p h c", h=HEADS),
            x_shard_v, asm[:, 4:8].to_broadcast((128, HEADS, HID)),
        )
        hrem_t2 = persist.tile([128, OUT], f32, tag="hrem_t2")
        nc.vector.tensor_mul(
            hrem_t2.rearrange("p (h c) -> p h c", h=HEADS),
            x_rem_v, asm[:, 8:12].to_broadcast((128, HEADS, HID)),
        )
        h_rem = persist.tile([128, OUT], f32, tag="h_rem")
        nc.vector.tensor_add(h_rem[:], hrem_t1[:], hrem_t2[:])
        nc.vector.tensor_add(h_rem[:], h_rem[:], sb_Bg[:])

        # ---- layernorm helper (stats DVE, sqrt ACT, apply DVE) ----
        BNS = nc.vector.BN_STATS_DIM
        BNA = nc.vector.BN_AGGR_DIM

        def layer_norm(x_t, width, tagp, groups=1):
            g, w = groups, width // groups
            xv = x_t[:, 0:width].rearrange("p (g w) -> p g w", g=g)
            st = small.tile([128, g, BNS], f32, tag="bnst")
            mv = small.tile([128, g, BNA], f32, tag="bnmv")
            for i in range(g):
                nc.vector.bn_stats(st[:, i, :], xv[:, i, :])
                nc.vector.bn_aggr(mv[:, i, :], st[:, i, :])
            sdev = small.tile([128, g], f32, tag="sdev")
            nc.scalar.activation(sdev[:], mv[:, :, 1], AF.Sqrt, bias=eps_col[:])
            rstd = small.tile([128, g], f32, tag="rstd")
            nc.vector.reciprocal(rstd[:], sdev[:])
            negm = small.tile([128, g], f32, tag="negm")
            nc.vector.tensor_scalar(negm[:], mv[:, :, 0], -1.0, None, op0=MUL)
            out_t = big.tile([128, width], f32, tag=tagp)
            ov = out_t.rearrange("p (g w) -> p g w", g=g)
            nc.vector.tensor_add(ov, xv, negm[:].to_broadcast((128, g, w)))
            nc.vector.tensor_mul(ov, ov, rstd[:].to_broadcast((128, g, w)))
            return out_t

        # ---- decoder machinery (f32) ----
        rec_sb = {
            "l": persist.tile([128, 2, IN_DIM], f32, tag="recl", name="recl"),
            "r": persist.tile([128, 2, IN_DIM], f32, tag="recr", name="recr"),
        }

        def transpose_h(hn_t, nt):
            psTt = psT.tile([128, 4, 128], f32, tag="psT")
            nc.tensor.transpose(psTt[:, 0, :], hn_t[:, 0:128], ident[:])
            nc.tensor.transpose(psTt[:, 1, :], hn_t[:, 128:256], ident[:])
            ha = big.tile([128, 128], f32, tag=f"hTa{nt}", name=f"hTa{nt}")
            hb = big.tile([128, 128], f32, tag=f"hTb{nt}", name=f"hTb{nt}")
            if nt == 0:
                nc.scalar.copy(ha[:], psTt[:, 0, :])
                nc.vector.tensor_copy(hb[:], psTt[:, 1, :])
            else:
                nc.vector.tensor_copy(ha[:], psTt[:, 0, :])
                nc.scalar.copy(hb[:], psTt[:, 1, :])
            return ha, hb

        def run_decoders_nt(nt, ha, hb):
            # both decoders (l|r) processed jointly in the free dim
            ps1 = psX.tile([128, WEXT], f32, tag="psx")
            nc.tensor.matmul(ps1[:, 0:256], ha[:], dW1[:, 0, :],
                             start=True, stop=False)
            nc.tensor.matmul(ps1[:, 0:256], hb[:], dW1[:, 1, :],
                             start=False, stop=True)
            s1 = big.tile([128, 256], f32, tag="s1")
            nc.vector.tensor_add(s1[:], ps1[:, 0:256], db1[:])
            r1 = big.tile([128, 256], f32, tag="r1")
            nc.scalar.activation(r1[:], s1[:], AF.Relu)
            n1 = layer_norm(r1, 256, "n1", groups=2)
            psn = psT.tile([128, 4, 128], f32, tag="psT")
            nc.tensor.transpose(psn[:, 0, :], n1[:, 0:128], ident[:])
            nc.tensor.transpose(psn[:, 1, :], n1[:, 128:256], ident[:])
            n1lT = big.tile([128, 128], f32, tag="n1lT")
            n1rT = big.tile([128, 128], f32, tag="n1rT")
            if nt == 0:
                nc.scalar.copy(n1lT[:], psn[:, 0, :])
                nc.vector.tensor_copy(n1rT[:], psn[:, 1, :])
            else:
                nc.vector.tensor_copy(n1lT[:], psn[:, 0, :])
                nc.scalar.copy(n1rT[:], psn[:, 1, :])

            ps2 = psX.tile([128, WEXT], f32, tag="psx")
            nc.tensor.matmul(ps2[:, 0:64], n1lT[:], dW2[:, 0:64],
                             start=True, stop=True)
            nc.tensor.matmul(ps2[:, 64:128], n1rT[:], dW2[:, 64:128],
                             start=True, stop=True)
            s2 = big.tile([128, 128], f32, tag="s2")
            nc.vector.tensor_add(s2[:], ps2[:, 0:128], db2[:])
            r2 = big.tile([128, 128], f32, tag="r2")
            nc.scalar.activation(r2[:], s2[:], AF.Relu)
            n2 = layer_norm(r2, 128, "n2", groups=2)
            psn2 = psT.tile([128, 4, 128], f32, tag="psT")
            nc.tensor.transpose(psn2[0:64, 0, :], n2[:, 0:64], ident[:])
            nc.tensor.transpose(psn2[0:64, 1, :], n2[:, 64:128], ident[:])
            n2lT = big.tile([64, 128], f32, tag="n2lT")
            n2rT = big.tile([64, 128], f32, tag="n2rT")
            if nt == 0:
                nc.vector.tensor_copy(n2lT[:], psn2[0:64, 0, :])
                nc.scalar.copy(n2rT[:], psn2[0:64, 1, :])
            else:
                nc.scalar.copy(n2lT[:], psn2[0:64, 0, :])
                nc.vector.tensor_copy(n2rT[:], psn2[0:64, 1, :])

            ps3 = psX.tile([128, WEXT], f32, tag="psx")
            nc.tensor.matmul(ps3[:, 0:128], n2lT[:], dW3[:, 0:128],
                             start=True, stop=True)
            nc.tensor.matmul(ps3[:, 128:256], n2rT[:], dW3[:, 128:256],
                             start=True, stop=True)
            nc.vector.tensor_add(rec_sb["l"][:, nt, :], ps3[:, 0:128], db3[:, 0:128])
            nc.vector.tensor_add(rec_sb["r"][:, nt, :], ps3[:, 128:256], db3[:, 128:256])

        # ---- dense attention per head (transposed [s, d] layout) ----
        # all 4 a_dst broadcasts in one PSUM bank
        psBh = psB.tile([128, HEADS, SH], f32, tag="Bh")
        for h in range(HEADS):
            nc.tensor.matmul(
                psBh[:, h, :], sel4[:, h * 128 : (h + 1) * 128], a_dstT[:]
            )
        # all 4 heads accumulate into one PSUM tile: [:, h, 0:64] = msg sum,
        # [:, h, 64] = sum(exp)
        psagg = psG.tile([128, HEADS, HID + 1], f32, tag="agg")
        for h in range(HEADS):
            # S[p, t, d] = a_src[t*128+p, h] + a_dst[d]  (one broadcast add)
            S_h = big.tile([128, NLOC_T, SH], f32, tag="eT")
            bh_slice = psBh[:, h, :]
            bh_bcast = bass.AP(
                tensor=bh_slice.tensor,
                offset=bh_slice.offset,
                ap=[bh_slice.ap[0], [0, NLOC_T], bh_slice.ap[1]],
            )
            nc.vector.tensor_add(
                S_h[:], bh_bcast, A_src[:, :, h].to_broadcast((128, NLOC_T, SH))
            )
            ex_T = big.tile([128, NLOC_T, SH], f32, tag="exT")
            eflat = S_h.rearrange("p a b -> p (a b)")
            nc.scalar.activation(eflat, eflat, AF.Prelu, alpha=NEG_SLOPE)
            nc.scalar.activation(
                ex_T.rearrange("p a b -> p (a b)"), eflat, AF.Exp
            )
            # unnormalized exp block straight to HBM (host normalizes)
            nc.sync.dma_start(
                alpha_t[:, h * L : (h + 1) * L],
                ex_T.rearrange("p a b -> p (a b)"),
            )
            exb = big.tile([128, NLOC_T, SH], bf16, tag="exb")
            nc.vector.tensor_copy(exb[:], ex_T[:])

            # aggregation + Z in one accumulation: exb[s,:]^T @ [x_h | 1]
            for t in range(NLOC_T):
                nc.tensor.matmul(
                    psagg[:, h, :],
                    exb[:, t, :],
                    xbf_all[:, t, h, :],
                    start=(t == 0),
                    stop=(t == NLOC_T - 1),
                )

        # remote half of the decoders (fills engine gaps of the head loop)
        hn_rem = layer_norm(h_rem, OUT, "hn")
        ha1, hb1 = transpose_h(hn_rem, 1)
        run_decoders_nt(1, ha1, hb1)

        # ---- batched per-head epilogue ----
        # Zf = psagg[:, :, 64] + exp(e_rl);  R = 1/Zf
        Zf = small.tile([128, HEADS], f32, tag="Zf")
        nc.vector.tensor_add(Zf[:], psagg[:, :, HID], EXP3[:, 0:4])
        nc.vector.reciprocal(asm[:, 12:16], Zf[:])
        nc.vector.tensor_mul(asm[:, 0:4], EXP3[:, 0:4], asm[:, 12:16])
        # h_loc = (agg + exp(e_rl)*x_rem) * R + bgat
        xt1 = persist.tile([128, OUT], f32, tag="xt1")
        xt1_v = xt1.rearrange("p (h c) -> p h c", h=HEADS)
        nc.vector.tensor_mul(
            xt1_v, x_rem_v, EXP3[:, 0:4].to_broadcast((128, HEADS, HID))
        )
        hpre = persist.tile([128, OUT], f32, tag="hpre")
        hpre_v = hpre.rearrange("p (h c) -> p h c", h=HEADS)
        nc.vector.tensor_add(hpre_v, psagg[:, :, 0:HID], xt1_v)
        h_loc = persist.tile([128, OUT], f32, tag="h_loc")
        nc.vector.tensor_mul(
            h_loc.rearrange("p (h c) -> p h c", h=HEADS),
            hpre_v, asm[:, 12:16].to_broadcast((128, HEADS, HID)),
        )
        nc.vector.tensor_add(h_loc[:], h_loc[:], sb_Bg[:])
        nc.sync.dma_start(asmall[:], asm[:])

        # ---- local half of decoders ----
        hn_loc = layer_norm(h_loc, OUT, "hn")
        ha0, hb0 = transpose_h(hn_loc, 0)
        run_decoders_nt(0, ha0, hb0)

        nc.sync.dma_start(rec_l.rearrange("(a p) n -> p a n", p=128), rec_sb["l"][:])
        nc.sync.dma_start(rec_r.rearrange("(a p) n -> p a n", p=128), rec_sb["r"][:])

    nc.compile()
    return nc


_NC_CACHE = {}


def _get_nc():
    if "nc" not in _NC_CACHE:
        _NC_CACHE["nc"] = _build_nc()
    return _NC_CACHE["nc"]


# ---------------------------------------------------------------------------
# Host side


def _expected_edge_index():
    local = np.arange(L)
    s = np.repeat(local, L)
    d = np.tile(local, L)
    m = s != d
    src = np.concatenate([s[m], local, L + local, np.arange(N)])
    dst = np.concatenate([d[m], L + local, local, np.arange(N)])
    return np.stack([src, dst])


def _np_reference_fallback(node_features, gat, norm, dec_local, dec_remote,
                           node_types, edge_index):
    """Pure-numpy replica of the reference; used only if the edge structure
    is not the expected deterministic pattern."""
    x = node_features @ gat["W"]
    xh = x.reshape(N, HEADS, HID)
    a_src = (xh * gat["att_src"]).sum(-1)
    a_dst = (xh * gat["att_dst"]).sum(-1)
    src, dst = edge_index[0], edge_index[1]
    e = a_src[src] + a_dst[dst]
    e = np.where(e >= 0, e, NEG_SLOPE * e).astype(np.float32)
    emax = np.full((N, HEADS), -np.inf, np.float32)
    np.maximum.at(emax, dst, e)
    ex = np.exp(e - emax[dst])
    zs = np.zeros((N, HEADS), np.float32)
    np.add.at(zs, dst, ex)
    alpha = ex / zs[dst]
    msg = xh[src] * alpha[:, :, None]
    out = np.zeros((N, HEADS, HID), np.float32)
    np.add.at(out, dst, msg)
    out = out.reshape(N, OUT) + gat["bias"]

    def ln(v, g, b):
        m = v.mean(-1, keepdims=True)
        var = ((v - m) ** 2).mean(-1, keepdims=True)
        return (v - m) / np.sqrt(var + EPS) * g + b

    h = ln(out, norm["g"], norm["b"])

    def dec(v, p):
        t = ln(np.maximum(v @ p["W1"] + p["b1"], 0), p["g1"], p["b1n"])
        t = ln(np.maximum(t @ p["W2"] + p["b2"], 0), p["g2"], p["b2n"])
        return t @ p["W3"] + p["b3"]

    rl = dec(h, dec_local)
    rr = dec(h, dec_remote)
    rec = np.where((node_types == 1)[:, None], rl, rr)
    return rec.astype(np.float32), edge_index, alpha.astype(np.float32)


def _to_np(v):
    return {k: _to_np(x) for k, x in v.items()} if isinstance(v, dict) else np.asarray(v)


def kernel(node_features, gat, norm, dec_local, dec_remote, node_types,
           edge_index, trace=False):
    _ensure_axon_hooks_stub()
    node_features = np.asarray(node_features, np.float32)
    gat, norm = _to_np(gat), _to_np(norm)
    dec_local, dec_remote = _to_np(dec_local), _to_np(dec_remote)
    node_types_np = np.asarray(node_types)
    edge_index_np = np.asarray(edge_index)

    if not np.array_equal(edge_index_np.astype(np.int64), _expected_edge_index()):
        return _np_reference_fallback(
            node_features, gat, norm, dec_local, dec_remote,
            node_types_np, edge_index_np,
        )

    from concourse.bass_utils import run_bass_kernel_spmd

    f32 = np.float32
    W = gat["W"].astype(f32)
    att_src = gat["att_src"].astype(f32)
    att_dst = gat["att_dst"].astype(f32)
    Wsrc = (W.reshape(MAPPED, HEADS, HID) * att_src[None]).sum(-1)
    Wdst = (W.reshape(MAPPED, HEADS, HID) * att_dst[None]).sum(-1)
    Wg_ext = np.ascontiguousarray(np.concatenate([W, Wsrc, Wdst], axis=1))
    nfT = np.ascontiguousarray(node_features.T)

    # fold LN affine params into the following linear layer; biases
    # pre-broadcast to [128, w]
    def fold(dec):
        g, b = norm["g"].astype(f32), norm["b"].astype(f32)
        W1 = g[:, None] * dec["W1"]
        b1 = dec["b1"] + b @ dec["W1"]
        W2 = dec["g1"][:, None] * dec["W2"]
        b2 = dec["b2"] + dec["b1n"] @ dec["W2"]
        W3 = dec["g2"][:, None] * dec["W3"]
        b3 = dec["b3"] + dec["b2n"] @ dec["W3"]

        def bc(v):
            return np.ascontiguousarray(
                np.broadcast_to(v.reshape(1, -1).astype(f32), (128, v.size))
            )

        return (
            np.ascontiguousarray(W1, f32), bc(b1),
            np.ascontiguousarray(W2, f32), bc(b2),
            np.ascontiguousarray(W3, f32), bc(b3),
        )

    # dec_all (combined l|r): W1lr(512, rearranged) b1lr(256) W2lr(128)
    # b2lr(128) W3lr-pad(256) b3lr(256) Bg(256)
    Fl = fold(dec_local)
    Fr = fold(dec_remote)
    W1lr = np.concatenate([Fl[0], Fr[0]], axis=1)          # [256, 256]
    W1lr = W1lr.reshape(2, 128, 256).transpose(1, 0, 2).reshape(128, 512)
    b1lr = np.concatenate([Fl[1], Fr[1]], axis=1)          # [128, 256]
    W2lr = np.concatenate([Fl[2], Fr[2]], axis=1)          # [128, 128]
    b2lr = np.concatenate([Fl[3], Fr[3]], axis=1)          # [128, 128]
    W3p = np.zeros((128, 256), f32)
    W3p[0:64, 0:128] = Fl[4]
    W3p[0:64, 128:256] = Fr[4]
    b3lr = np.concatenate([Fl[5], Fr[5]], axis=1)          # [128, 256]
    Bg = np.broadcast_to(gat["bias"].reshape(1, -1).astype(f32), (128, OUT))
    dec_all = np.ascontiguousarray(
        np.concatenate([W1lr, b1lr, W2lr, b2lr, W3p, b3lr, Bg], axis=1), f32
    )

    in_maps = []
    for c in range(NCORES):
        nf_all = np.concatenate(
            [
                nfT,
                nfT[:, c * SH : (c + 1) * SH],
                nfT[:, L + c * SH : L + (c + 1) * SH],
                Wg_ext,
            ],
            axis=1,
        )
        in_maps.append({"nf_all": np.ascontiguousarray(nf_all), "dec_all": dec_all})

    nc = _get_nc()
    res = run_bass_kernel_spmd(nc, in_maps, core_ids=list(range(NCORES)), trace=trace)
    _NC_CACHE["last_results"] = res
    outs = res.results

    # ---- host unshard / assembly ----
    # alphaT_full[s, d, h] = exp(e) * R
    alphaT_full = np.empty((L, L, HEADS), np.float32)
    a_rl = np.empty((L, HEADS), np.float32)
    a_lr = np.empty((L, HEADS), np.float32)
    a_rr = np.empty((L, HEADS), np.float32)
    for c in range(NCORES):
        sl = slice(c * SH, (c + 1) * SH)
        sm = outs[c]["asmall"]
        a_rl[sl], a_lr[sl], a_rr[sl] = sm[:, 0:4], sm[:, 4:8], sm[:, 8:12]
        R = sm[:, 12:16]  # [d, h]
        blk = outs[c]["alpha_t"].reshape(128, HEADS, NLOC_T, SH)  # [p, h, t, d]
        # -> [s=t*128+p, d, h]
        blk = np.transpose(blk, (2, 0, 3, 1)).reshape(L, SH, HEADS)
        alphaT_full[:, sl, :] = blk * R[None, :, :]

    per_edge = alphaT_full.reshape(L * L, HEADS)
    mask = ~np.eye(L, dtype=bool).reshape(-1)
    clique = per_edge[mask]
    idx = np.arange(L)
    self_local = alphaT_full[idx, idx, :]
    alpha = np.concatenate(
        [clique, a_lr, a_rl, self_local, a_rr], axis=0
    ).astype(np.float32)

    rec_local = np.empty((N, IN_DIM), np.float32)
    rec_remote = np.empty((N, IN_DIM), np.float32)
    for c in range(NCORES):
        sl = slice(c * SH, (c + 1) * SH)
        slr = slice(L + c * SH, L + (c + 1) * SH)
        rec_local[sl] = outs[c]["rec_l"][0:SH]
        rec_local[slr] = outs[c]["rec_l"][SH : 2 * SH]
        rec_remote[sl] = outs[c]["rec_r"][0:SH]
        rec_remote[slr] = outs[c]["rec_r"][SH : 2 * SH]
    reconstructed = np.where((node_types_np == 1)[:, None], rec_local, rec_remote)

    return reconstructed.astype(np.float32), edge_index_np, alpha


# revision 20
# speedup vs baseline: 1.2521x; 1.0193x over previous
"""Self-contained Trainium2 (Bass/Tile) kernel for the DeviceGAT problem.

Computes, on 8 NeuronCores, the GAT layer + LayerNorm + two decoder MLPs of
reference.py, exploiting the deterministic graph structure:
  - edges = dense local clique (1024 local nodes, no self edges)
            + local->remote pairs (i -> L+i)
            + remote->local pairs (L+i -> i)
            + self loops for all 2048 nodes
  - so each local dst d has in-edges from all 1024 local nodes (incl. itself
    via the self loop) plus remote node L+d; each remote dst L+i has in-edges
    {i, L+i}.

Sharding: destination rows are sharded 8 ways (128 local dst rows + the
matching 128 remote dst rows per core).  Node features / params replicated.

Scores are built directly in transposed [src, dst] layout (a_dst broadcast by
a tiny selector matmul, a_src as the per-partition activation bias), so the
attention needs NO on-chip transposes.  exp(scores) is written to HBM
unnormalized together with the per-(dst, head) softmax reciprocal R; the host
applies R while permuting the dense block into the reference per-edge order.
The aggregation matmul contracts src on the partition axis (exb^T @ [x | 1])
in bf16, producing both the weighted message sum and the softmax denominator
for all four heads in one PSUM tile; per-head epilogue work is batched into a
handful of broadcast-AP vector ops.
"""

import os
import sys
import types
import numpy as np

# ---------------------------------------------------------------------------
# Problem constants (from the reference problem definition; deterministic).
L = 1024
N = 2 * L
MAPPED = 32
HID = 64
HEADS = 4
OUT = HID * HEADS        # 256
IN_DIM = 128
NEG_SLOPE = 0.2
EPS = 1e-5
NCORES = 8
SH = L // NCORES         # 128 dst rows per core
NLOC_T = L // 128        # 8 local node tiles


def _ensure_axon_hooks_stub():
    """run_bass_kernel_spmd(trace=True) imports antenv.axon_hooks; provide a
    graceful stub when the image lacks it so tracing degrades instead of
    crashing.  (Harness runs trace=False and never hits this, but be safe.)"""
    try:
        import antenv.axon_hooks  # noqa: F401
        return
    except Exception:
        pass
    try:
        import antenv
    except Exception:
        antenv = types.ModuleType("antenv")
        sys.modules["antenv"] = antenv
    mod = types.ModuleType("antenv.axon_hooks")
    mod._HOOK = None

    def set_axon_ntff_profile_hook(hook):
        mod._HOOK = hook

    def get_axon_ntff_profile_hook():
        if mod._HOOK is not None:
            return mod._HOOK
        so = "/opt/axon/libaxon_pjrt.so"
        if os.path.exists(so):
            import contextlib
            import ctypes

            try:
                lib = ctypes.CDLL(so)
            except OSError:
                return None
            if not hasattr(lib, "axon_start_nrt_profile"):
                return None
            lib.axon_start_nrt_profile.argtypes = [
                ctypes.POINTER(ctypes.c_int64),
                ctypes.c_size_t,
            ]
            lib.axon_start_nrt_profile.restype = ctypes.c_int64
            lib.axon_stop_nrt_profile.argtypes = [ctypes.c_char_p]
            lib.axon_stop_nrt_profile.restype = ctypes.c_int64

            @contextlib.contextmanager
            def _hook(output_dir, device_ids):
                import jax

                jax.devices()
                if device_ids:
                    ids = (ctypes.c_int64 * len(device_ids))(*device_ids)
                    rc = lib.axon_start_nrt_profile(ids, len(device_ids))
                else:
                    rc = lib.axon_start_nrt_profile(None, 0)
                if rc != 0:
                    raise RuntimeError(f"axon_start_nrt_profile rc={rc}")
                try:
                    yield
                finally:
                    n = lib.axon_stop_nrt_profile(str(output_dir).encode())
                    print(f"profile: {n} file(s) in {output_dir}", file=sys.stderr)

            mod._HOOK = _hook
            return mod._HOOK
        return None

    mod.set_axon_ntff_profile_hook = set_axon_ntff_profile_hook
    mod.get_axon_ntff_profile_hook = get_axon_ntff_profile_hook
    sys.modules["antenv.axon_hooks"] = mod


# ---------------------------------------------------------------------------
# Bass kernel builder


def _build_nc():
    from contextlib import ExitStack

    import concourse.bass as bass
    import concourse.tile as tile
    from concourse import bacc, mybir
    from concourse.masks import make_identity

    f32 = mybir.dt.float32
    bf16 = mybir.dt.bfloat16
    AF = mybir.ActivationFunctionType
    MUL = mybir.AluOpType.mult
    ADD = mybir.AluOpType.add
    WEXT = OUT + 2 * HEADS  # 264: [Wg | Wsrc | Wdst]

    nc = bacc.Bacc(
        trn_type="TRN2", target_bir_lowering=False, debug=False, num_devices=NCORES
    )

    # ---- I/O ----
    # nf_all: [32, N + SH + SH + WEXT] = nfT | nfT_loc | nfT_rem | Wg_ext
    NFC = N + 2 * SH + WEXT
    nf_all = nc.dram_tensor("nf_all", [MAPPED, NFC], f32, kind="ExternalInput")
    # dec_all: [128, 1792] = for each dec: W1(256) b1(128) W2(64) b2(64)
    # W3pad(128) b3(128); then Bg(256)
    DCC = 2 * (256 + 128 + 64 + 64 + 128 + 128) + OUT
    dec_all = nc.dram_tensor("dec_all", [128, DCC], f32, kind="ExternalInput")

    # dense unnormalized exp(scores), transposed layout, raw per-head blocks
    # [p, t, d]: src node s = t*128+p, dst column d
    alpha_t = nc.dram_tensor("alpha_t", [128, HEADS * L], f32, kind="ExternalOutput")
    # a_rl | a_lr | a_rr | R  (R = per-(d,h) softmax reciprocal)
    asmall = nc.dram_tensor("asmall", [SH, 16], f32, kind="ExternalOutput")
    rec_l = nc.dram_tensor("rec_l", [2 * SH, IN_DIM], f32, kind="ExternalOutput")
    rec_r = nc.dram_tensor("rec_r", [2 * SH, IN_DIM], f32, kind="ExternalOutput")

    with tile.TileContext(nc) as tc, ExitStack() as ctx:
        consts = ctx.enter_context(tc.tile_pool(name="consts", bufs=1))
        big = ctx.enter_context(tc.tile_pool(name="big", bufs=3))
        scp = ctx.enter_context(tc.tile_pool(name="scp", bufs=4))
        persist = ctx.enter_context(tc.tile_pool(name="persist", bufs=1))
        small = ctx.enter_context(tc.tile_pool(name="small", bufs=2))
        psB = ctx.enter_context(tc.tile_pool(name="psB", bufs=2, space="PSUM"))
        psG = ctx.enter_context(tc.tile_pool(name="psG", bufs=1, space="PSUM"))
        psT = ctx.enter_context(tc.tile_pool(name="psT", bufs=2, space="PSUM"))
        psX = ctx.enter_context(tc.tile_pool(name="psX", bufs=2, space="PSUM"))

        # ---- load constants (2 big DMAs) ----
        sb_nf = consts.tile([MAPPED, NFC], f32)
        nc.sync.dma_start(sb_nf[:], nf_all[:])
        sb_nfT = sb_nf[:, 0:N]
        sb_nfT_loc = sb_nf[:, N : N + SH]
        sb_nfT_rem = sb_nf[:, N + SH : N + 2 * SH]
        sb_Wg = sb_nf[:, N + 2 * SH : NFC]
        sb_dec = consts.tile([128, DCC], f32)
        nc.sync.dma_start(sb_dec[:], dec_all[:])
        # combined l|r decoder params
        dW1 = sb_dec[:, 0:512].rearrange("p (a n) -> p a n", a=2)  # [128,2,256]
        db1 = sb_dec[:, 512:768]
        dW2 = sb_dec[:, 768:896]      # cols 0:64 = W2l, 64:128 = W2r
        db2 = sb_dec[:, 896:1024]
        dW3 = sb_dec[0:64, 1024:1280]  # cols 0:128 = W3l, 128:256 = W3r
        db3 = sb_dec[:, 1280:1536]

        eps_col = consts.tile([128, 1], f32)
        nc.vector.memset(eps_col[:], EPS)
        ident = consts.tile([128, 128], f32)
        make_identity(nc, ident[:])
        # head-selector: sel4[k, h*128+m] = (k == h); used as k=4 lhsT to
        # broadcast row h of a [4, x] tile across 128 partitions
        sel4 = consts.tile([HEADS, HEADS * 128], f32)
        nc.gpsimd.memset(sel4[:], 0.0)
        sel4v = sel4.rearrange("p (a b) -> p a b", a=HEADS)
        nc.gpsimd.affine_select(
            out=sel4v,
            in_=sel4v,
            compare_op=mybir.AluOpType.not_equal,
            fill=1.0,
            base=0,
            pattern=[[-1, HEADS], [0, 128]],
            channel_multiplier=1,
        )
        sb_Bg = sb_dec[:, 1536 : 1536 + OUT]

        # ---- projections on PE: x|a_src|a_dst per node tile ----
        # xbf_all[p, t, h, 0:64] = x, [..., 64] = 1.0  (bf16, agg rhs)
        xbf_all = persist.tile([128, NLOC_T, HEADS, HID + 1], bf16, tag="xbf")
        nc.gpsimd.memset(xbf_all[:], 1.0)
        # a_src for all local tiles: [128, t, h]
        A_src = persist.tile([128, NLOC_T, HEADS], f32, tag="asrc")
        for t in range(NLOC_T):
            ps = psX.tile([128, WEXT], f32, tag="psx")
            nc.tensor.matmul(ps[:], sb_nfT[:, t * 128 : (t + 1) * 128], sb_Wg[:])
            nc.vector.tensor_copy(
                xbf_all[:, t, :, 0:HID],
                ps[:, 0:OUT].rearrange("p (h c) -> p h c", h=HEADS),
            )
            nc.scalar.copy(A_src[:, t, :], ps[:, OUT : OUT + HEADS])

        ps = psX.tile([128, WEXT], f32, tag="psx")
        nc.tensor.matmul(ps[:], sb_nfT_rem[:], sb_Wg[:])
        x_rem = persist.tile([128, OUT], f32, tag="xrem")
        nc.scalar.copy(x_rem[:], ps[:, 0:OUT])
        A_rem = persist.tile([128, 2 * HEADS], f32, tag="arem")
        nc.vector.tensor_copy(A_rem[:], ps[:, OUT:WEXT])

        ps = psX.tile([128, WEXT], f32, tag="psx")
        nc.tensor.matmul(ps[:], sb_nfT_loc[:], sb_Wg[:])
        x_shard = persist.tile([128, OUT], f32, tag="xshard")
        nc.vector.tensor_copy(x_shard[:], ps[:, 0:OUT])
        A_loc = persist.tile([128, 2 * HEADS], f32, tag="aloc")
        nc.scalar.copy(A_loc[:], ps[:, OUT:WEXT])

        # a_dst of the shard as rows: [4, 128]
        psd = psX.tile([128, WEXT], f32, tag="psx")
        nc.tensor.matmul(
            psd[0:HEADS, 0:SH], sb_Wg[:, OUT + HEADS : WEXT], sb_nfT_loc[:]
        )
        a_dstT = consts.tile([HEADS, SH], f32)
        nc.scalar.copy(a_dstT[:], psd[0:HEADS, 0:SH])

        # ---- layernorm helper (stats DVE, sqrt ACT, apply DVE) ----
        BNS = nc.vector.BN_STATS_DIM
        BNA = nc.vector.BN_AGGR_DIM

        def layer_norm(x_t, width, tagp, groups=1):
            g, w = groups, width // groups
            xv = x_t[:, 0:width].rearrange("p (g w) -> p g w", g=g)
            st = small.tile([128, g, BNS], f32, tag="bnst")
            mv = small.tile([128, g, BNA], f32, tag="bnmv")
            for i in range(g):
                nc.vector.bn_stats(st[:, i, :], xv[:, i, :])
                nc.vector.bn_aggr(mv[:, i, :], st[:, i, :])
            sdev = small.tile([128, g], f32, tag="sdev")
            nc.scalar.activation(sdev[:], mv[:, :, 1], AF.Sqrt, bias=eps_col[:])
            rstd = small.tile([128, g], f32, tag="rstd")
            nc.vector.reciprocal(rstd[:], sdev[:])
            negm = small.tile([128, g], f32, tag="negm")
            nc.vector.tensor_scalar(negm[:], mv[:, :, 0], -1.0, None, op0=MUL)
            out_t = big.tile([128, width], f32, tag=tagp)
            ov = out_t.rearrange("p (g w) -> p g w", g=g)
            nc.vector.tensor_add(ov, xv, negm[:].to_broadcast((128, g, w)))
            nc.vector.tensor_mul(ov, ov, rstd[:].to_broadcast((128, g, w)))
            return out_t

        # ---- decoder machinery (f32) ----
        rec_sb = {
            "l": persist.tile([128, 2, IN_DIM], f32, tag="recl", name="recl"),
            "r": persist.tile([128, 2, IN_DIM], f32, tag="recr", name="recr"),
        }

        def transpose_h(hn_t, nt):
            psTt = psT.tile([128, 4, 128], f32, tag="psT")
            nc.tensor.transpose(psTt[:, 0, :], hn_t[:, 0:128], ident[:])
            nc.tensor.transpose(psTt[:, 1, :], hn_t[:, 128:256], ident[:])
            ha = big.tile([128, 128], f32, tag=f"hTa{nt}", name=f"hTa{nt}")
            hb = big.tile([128, 128], f32, tag=f"hTb{nt}", name=f"hTb{nt}")
            if nt == 0:
                nc.scalar.copy(ha[:], psTt[:, 0, :])
                nc.vector.tensor_copy(hb[:], psTt[:, 1, :])
            else:
                nc.vector.tensor_copy(ha[:], psTt[:, 0, :])
                nc.scalar.copy(hb[:], psTt[:, 1, :])
            return ha, hb

        def run_decoders_nt(nt, ha, hb):
            # both decoders (l|r) processed jointly in the free dim
            ps1 = psX.tile([128, WEXT], f32, tag="psx")
            nc.tensor.matmul(ps1[:, 0:256], ha[:], dW1[:, 0, :],
                             start=True, stop=False)
            nc.tensor.matmul(ps1[:, 0:256], hb[:], dW1[:, 1, :],
                             start=False, stop=True)
            s1 = big.tile([128, 256], f32, tag="s1")
            nc.vector.tensor_add(s1[:], ps1[:, 0:256], db1[:])
            r1 = big.tile([128, 256], f32, tag="r1")
            nc.scalar.activation(r1[:], s1[:], AF.Relu)
            n1 = layer_norm(r1, 256, "n1", groups=2)
            psn = psT.tile([128, 4, 128], f32, tag="psT")
            nc.tensor.transpose(psn[:, 0, :], n1[:, 0:128], ident[:])
            nc.tensor.transpose(psn[:, 1, :], n1[:, 128:256], ident[:])
            n1lT = big.tile([128, 128], f32, tag="n1lT")
            n1rT = big.tile([128, 128], f32, tag="n1rT")
            if nt == 0:
                nc.scalar.copy(n1lT[:], psn[:, 0, :])
                nc.vector.tensor_copy(n1rT[:], psn[:, 1, :])
            else:
                nc.vector.tensor_copy(n1lT[:], psn[:, 0, :])
                nc.scalar.copy(n1rT[:], psn[:, 1, :])

            ps2 = psX.tile([128, WEXT], f32, tag="psx")
            nc.tensor.matmul(ps2[:, 0:64], n1lT[:], dW2[:, 0:64],
                             start=True, stop=True)
            nc.tensor.matmul(ps2[:, 64:128], n1rT[:], dW2[:, 64:128],
                             start=True, stop=True)
            s2 = big.tile([128, 128], f32, tag="s2")
            nc.vector.tensor_add(s2[:], ps2[:, 0:128], db2[:])
            r2 = big.tile([128, 128], f32, tag="r2")
            nc.scalar.activation(r2[:], s2[:], AF.Relu)
            n2 = layer_norm(r2, 128, "n2", groups=2)
            psn2 = psT.tile([128, 4, 128], f32, tag="psT")
            nc.tensor.transpose(psn2[0:64, 0, :], n2[:, 0:64], ident[:])
            nc.tensor.transpose(psn2[0:64, 1, :], n2[:, 64:128], ident[:])
            n2lT = big.tile([64, 128], f32, tag="n2lT")
            n2rT = big.tile([64, 128], f32, tag="n2rT")
            if nt == 0:
                nc.vector.tensor_copy(n2lT[:], psn2[0:64, 0, :])
                nc.scalar.copy(n2rT[:], psn2[0:64, 1, :])
            else:
                nc.scalar.copy(n2lT[:], psn2[0:64, 0, :])
                nc.vector.tensor_copy(n2rT[:], psn2[0:64, 1, :])

            ps3 = psX.tile([128, WEXT], f32, tag="psx")
            nc.tensor.matmul(ps3[:, 0:128], n2lT[:], dW3[:, 0:128],
                             start=True, stop=True)
            nc.tensor.matmul(ps3[:, 128:256], n2rT[:], dW3[:, 128:256],
                             start=True, stop=True)
            nc.vector.tensor_add(rec_sb["l"][:, nt, :], ps3[:, 0:128], db3[:, 0:128])
            nc.vector.tensor_add(rec_sb["r"][:, nt, :], ps3[:, 128:256], db3[:, 128:256])

        # ---- dense attention per head (transposed [s, d] layout) ----
        # all 4 a_dst broadcasts in one PSUM bank
        psBh = psB.tile([128, HEADS, SH], f32, tag="Bh")
        for h in range(HEADS):
            nc.tensor.matmul(
                psBh[:, h, :], sel4[:, h * 128 : (h + 1) * 128], a_dstT[:]
            )
        # S[p, t, d] = a_src[t*128+p, h] + a_dst[d] for all heads up front, so
        # the DVE stream never stalls behind ACT
        S_hs = []
        for h in range(HEADS):
            S_h = scp.tile([128, NLOC_T, SH], f32, tag="eT", name=f"S{h}")
            bh_slice = psBh[:, h, :]
            bh_bcast = bass.AP(
                tensor=bh_slice.tensor,
                offset=bh_slice.offset,
                ap=[bh_slice.ap[0], [0, NLOC_T], bh_slice.ap[1]],
            )
            nc.vector.tensor_add(
                S_h[:], bh_bcast, A_src[:, :, h].to_broadcast((128, NLOC_T, SH))
            )
            S_hs.append(S_h)

        # all 4 heads accumulate into one PSUM tile: [:, h, 0:64] = msg sum,
        # [:, h, 64] = sum(exp)
        psagg = psG.tile([128, HEADS, HID + 1], f32, tag="agg")
        for h in range(HEADS):
            S_h = S_hs[h]
            ex_T = big.tile([128, NLOC_T, SH], f32, tag="exT")
            eflat = S_h.rearrange("p a b -> p (a b)")
            nc.scalar.activation(eflat, eflat, AF.Prelu, alpha=NEG_SLOPE)
            nc.scalar.activation(
                ex_T.rearrange("p a b -> p (a b)"), eflat, AF.Exp
            )
            # unnormalized exp block straight to HBM (host normalizes)
            nc.sync.dma_start(
                alpha_t[:, h * L : (h + 1) * L],
                ex_T.rearrange("p a b -> p (a b)"),
            )
            exb = big.tile([128, NLOC_T, SH], bf16, tag="exb")
            nc.vector.tensor_copy(exb[:], ex_T[:])

            # aggregation + Z in one accumulation: exb[s,:]^T @ [x_h | 1]
            for t in range(NLOC_T):
                nc.tensor.matmul(
                    psagg[:, h, :],
                    exb[:, t, :],
                    xbf_all[:, t, h, :],
                    start=(t == 0),
                    stop=(t == NLOC_T - 1),
                )

        # ---- special-edge scores: e_rl | e_lr | e_rr  [128, 12] ----
        E3 = persist.tile([128, 12], f32, tag="E3")
        nc.vector.tensor_add(E3[:, 0:4], A_rem[:, 0:4], A_loc[:, 4:8])
        nc.vector.tensor_add(E3[:, 4:8], A_loc[:, 0:4], A_rem[:, 4:8])
        nc.vector.tensor_add(E3[:, 8:12], A_rem[:, 0:4], A_rem[:, 4:8])
        LR3 = persist.tile([128, 12], f32, tag="LR3")
        nc.scalar.activation(LR3[:], E3[:], AF.Prelu, alpha=NEG_SLOPE)
        EXP3 = persist.tile([128, 12], f32, tag="EXP3")
        nc.scalar.activation(EXP3[:], LR3[:], AF.Exp)

        # remote-dst softmax (2 edges) and h_rem (batched broadcast ops)
        asm = persist.tile([128, 16], f32, tag="asm")  # a_rl | a_lr | a_rr | R
        Zr = persist.tile([128, 4], f32, tag="Zr")
        nc.vector.tensor_add(Zr[:], EXP3[:, 4:8], EXP3[:, 8:12])
        Rr = persist.tile([128, 4], f32, tag="Rr")
        nc.vector.reciprocal(Rr[:], Zr[:])
        nc.vector.tensor_mul(asm[:, 4:8], EXP3[:, 4:8], Rr[:])
        nc.vector.tensor_mul(asm[:, 8:12], EXP3[:, 8:12], Rr[:])

        x_shard_v = x_shard.rearrange("p (h c) -> p h c", h=HEADS)
        x_rem_v = x_rem.rearrange("p (h c) -> p h c", h=HEADS)
        hrem_t1 = persist.tile([128, OUT], f32, tag="hrem_t1")
        nc.vector.tensor_mul(
            hrem_t1.rearrange("p (h c) -> p h c", h=HEADS),
            x_shard_v, asm[:, 4:8].to_broadcast((128, HEADS, HID)),
        )
        hrem_t2 = persist.tile([128, OUT], f32, tag="hrem_t2")
        nc.vector.tensor_mul(
            hrem_t2.rearrange("p (h c) -> p h c", h=HEADS),
            x_rem_v, asm[:, 8:12].to_broadcast((128, HEADS, HID)),
        )
        h_rem = persist.tile([128, OUT], f32, tag="h_rem")
        nc.vector.tensor_add(h_rem[:], hrem_t1[:], hrem_t2[:])
        nc.vector.tensor_add(h_rem[:], h_rem[:], sb_Bg[:])

        # remote half of the decoders (fills engine gaps of the head loop)
        hn_rem = layer_norm(h_rem, OUT, "hn")
        ha1, hb1 = transpose_h(hn_rem, 1)
        run_decoders_nt(1, ha1, hb1)

        # ---- batched per-head epilogue ----
        # Zf = psagg[:, :, 64] + exp(e_rl);  R = 1/Zf
        Zf = small.tile([128, HEADS], f32, tag="Zf")
        nc.vector.tensor_add(Zf[:], psagg[:, :, HID], EXP3[:, 0:4])
        nc.vector.reciprocal(asm[:, 12:16], Zf[:])
        nc.vector.tensor_mul(asm[:, 0:4], EXP3[:, 0:4], asm[:, 12:16])
        # h_loc = (agg + exp(e_rl)*x_rem) * R + bgat
        xt1 = persist.tile([128, OUT], f32, tag="xt1")
        xt1_v = xt1.rearrange("p (h c) -> p h c", h=HEADS)
        nc.vector.tensor_mul(
            xt1_v, x_rem_v, EXP3[:, 0:4].to_broadcast((128, HEADS, HID))
        )
        hpre = persist.tile([128, OUT], f32, tag="hpre")
        hpre_v = hpre.rearrange("p (h c) -> p h c", h=HEADS)
        nc.vector.tensor_add(hpre_v, psagg[:, :, 0:HID], xt1_v)
        h_loc = persist.tile([128, OUT], f32, tag="h_loc")
        nc.vector.tensor_mul(
            h_loc.rearrange("p (h c) -> p h c", h=HEADS),
            hpre_v, asm[:, 12:16].to_broadcast((128, HEADS, HID)),
        )
        nc.vector.tensor_add(h_loc[:], h_loc[:], sb_Bg[:])
        nc.sync.dma_start(asmall[:], asm[:])

        # ---- local half of decoders ----
        hn_loc = layer_norm(h_loc, OUT, "hn")
        ha0, hb0 = transpose_h(hn_loc, 0)
        run_decoders_nt(0, ha0, hb0)

        nc.sync.dma_start(rec_l.rearrange("(a p) n -> p a n", p=128), rec_sb["l"][:])
        nc.sync.dma_start(rec_r.rearrange("(a p) n -> p a n", p=128), rec_sb["r"][:])

    nc.compile()
    return nc


_NC_CACHE = {}


def _get_nc():
    if "nc" not in _NC_CACHE:
        _NC_CACHE["nc"] = _build_nc()
    return _NC_CACHE["nc"]


# ---------------------------------------------------------------------------
# Host side


def _expected_edge_index():
    local = np.arange(L)
    s = np.repeat(local, L)
    d = np.tile(local, L)
    m = s != d
    src = np.concatenate([s[m], local, L + local, np.arange(N)])
    dst = np.concatenate([d[m], L + local, local, np.arange(N)])
    return np.stack([src, dst])


def _np_reference_fallback(node_features, gat, norm, dec_local, dec_remote,
                           node_types, edge_index):
    """Pure-numpy replica of the reference; used only if the edge structure
    is not the expected deterministic pattern."""
    x = node_features @ gat["W"]
    xh = x.reshape(N, HEADS, HID)
    a_src = (xh * gat["att_src"]).sum(-1)
    a_dst = (xh * gat["att_dst"]).sum(-1)
    src, dst = edge_index[0], edge_index[1]
    e = a_src[src] + a_dst[dst]
    e = np.where(e >= 0, e, NEG_SLOPE * e).astype(np.float32)
    emax = np.full((N, HEADS), -np.inf, np.float32)
    np.maximum.at(emax, dst, e)
    ex = np.exp(e - emax[dst])
    zs = np.zeros((N, HEADS), np.float32)
    np.add.at(zs, dst, ex)
    alpha = ex / zs[dst]
    msg = xh[src] * alpha[:, :, None]
    out = np.zeros((N, HEADS, HID), np.float32)
    np.add.at(out, dst, msg)
    out = out.reshape(N, OUT) + gat["bias"]

    def ln(v, g, b):
        m = v.mean(-1, keepdims=True)
        var = ((v - m) ** 2).mean(-1, keepdims=True)
        return (v - m) / np.sqrt(var + EPS) * g + b

    h = ln(out, norm["g"], norm["b"])

    def dec(v, p):
        t = ln(np.maximum(v @ p["W1"] + p["b1"], 0), p["g1"], p["b1n"])
        t = ln(np.maximum(t @ p["W2"] + p["b2"], 0), p["g2"], p["b2n"])
        return t @ p["W3"] + p["b3"]

    rl = dec(h, dec_local)
    rr = dec(h, dec_remote)
    rec = np.where((node_types == 1)[:, None], rl, rr)
    return rec.astype(np.float32), edge_index, alpha.astype(np.float32)


def _to_np(v):
    return {k: _to_np(x) for k, x in v.items()} if isinstance(v, dict) else np.asarray(v)


def kernel(node_features, gat, norm, dec_local, dec_remote, node_types,
           edge_index, trace=False):
    _ensure_axon_hooks_stub()
    node_features = np.asarray(node_features, np.float32)
    gat, norm = _to_np(gat), _to_np(norm)
    dec_local, dec_remote = _to_np(dec_local), _to_np(dec_remote)
    node_types_np = np.asarray(node_types)
    edge_index_np = np.asarray(edge_index)

    if not np.array_equal(edge_index_np.astype(np.int64), _expected_edge_index()):
        return _np_reference_fallback(
            node_features, gat, norm, dec_local, dec_remote,
            node_types_np, edge_index_np,
        )

    from concourse.bass_utils import run_bass_kernel_spmd

    f32 = np.float32
    W = gat["W"].astype(f32)
    att_src = gat["att_src"].astype(f32)
    att_dst = gat["att_dst"].astype(f32)
    Wsrc = (W.reshape(MAPPED, HEADS, HID) * att_src[None]).sum(-1)
    Wdst = (W.reshape(MAPPED, HEADS, HID) * att_dst[None]).sum(-1)
    Wg_ext = np.ascontiguousarray(np.concatenate([W, Wsrc, Wdst], axis=1))
    nfT = np.ascontiguousarray(node_features.T)

    # fold LN affine params into the following linear layer; biases
    # pre-broadcast to [128, w]
    def fold(dec):
        g, b = norm["g"].astype(f32), norm["b"].astype(f32)
        W1 = g[:, None] * dec["W1"]
        b1 = dec["b1"] + b @ dec["W1"]
        W2 = dec["g1"][:, None] * dec["W2"]
        b2 = dec["b2"] + dec["b1n"] @ dec["W2"]
        W3 = dec["g2"][:, None] * dec["W3"]
        b3 = dec["b3"] + dec["b2n"] @ dec["W3"]

        def bc(v):
            return np.ascontiguousarray(
                np.broadcast_to(v.reshape(1, -1).astype(f32), (128, v.size))
            )

        return (
            np.ascontiguousarray(W1, f32), bc(b1),
            np.ascontiguousarray(W2, f32), bc(b2),
            np.ascontiguousarray(W3, f32), bc(b3),
        )

    # dec_all (combined l|r): W1lr(512, rearranged) b1lr(256) W2lr(128)
    # b2lr(128) W3lr-pad(256) b3lr(256) Bg(256)
    Fl = fold(dec_local)
    Fr = fold(dec_remote)
    W1lr = np.concatenate([Fl[0], Fr[0]], axis=1)          # [256, 256]
    W1lr = W1lr.reshape(2, 128, 256).transpose(1, 0, 2).reshape(128, 512)
    b1lr = np.concatenate([Fl[1], Fr[1]], axis=1)          # [128, 256]
    W2lr = np.concatenate([Fl[2], Fr[2]], axis=1)          # [128, 128]
    b2lr = np.concatenate([Fl[3], Fr[3]], axis=1)          # [128, 128]
    W3p = np.zeros((128, 256), f32)
    W3p[0:64, 0:128] = Fl[4]
    W3p[0:64, 128:256] = Fr[4]
    b3lr = np.concatenate([Fl[5], Fr[5]], axis=1)          # [128, 256]
    Bg = np.broadcast_to(gat["bias"].reshape(1, -1).astype(f32), (128, OUT))
    dec_all = np.ascontiguousarray(
        np.concatenate([W1lr, b1lr, W2lr, b2lr, W3p, b3lr, Bg], axis=1), f32
    )

    in_maps = []
    for c in range(NCORES):
        nf_all = np.concatenate(
            [
                nfT,
                nfT[:, c * SH : (c + 1) * SH],
                nfT[:, L + c * SH : L + (c + 1) * SH],
                Wg_ext,
            ],
            axis=1,
        )
        in_maps.append({"nf_all": np.ascontiguousarray(nf_all), "dec_all": dec_all})

    nc = _get_nc()
    res = run_bass_kernel_spmd(nc, in_maps, core_ids=list(range(NCORES)), trace=trace)
    _NC_CACHE["last_results"] = res
    outs = res.results

    # ---- host unshard / assembly ----
    # alphaT_full[s, d, h] = exp(e) * R
    alphaT_full = np.empty((L, L, HEADS), np.float32)
    a_rl = np.empty((L, HEADS), np.float32)
    a_lr = np.empty((L, HEADS), np.float32)
    a_rr = np.empty((L, HEADS), np.float32)
    for c in range(NCORES):
        sl = slice(c * SH, (c + 1) * SH)
        sm = outs[c]["asmall"]
        a_rl[sl], a_lr[sl], a_rr[sl] = sm[:, 0:4], sm[:, 4:8], sm[:, 8:12]
        R = sm[:, 12:16]  # [d, h]
        blk = outs[c]["alpha_t"].reshape(128, HEADS, NLOC_T, SH)  # [p, h, t, d]
        # -> [s=t*128+p, d, h]
        blk = np.transpose(blk, (2, 0, 3, 1)).reshape(L, SH, HEADS)
        alphaT_full[:, sl, :] = blk * R[None, :, :]

    per_edge = alphaT_full.reshape(L * L, HEADS)
    mask = ~np.eye(L, dtype=bool).reshape(-1)
    clique = per_edge[mask]
    idx = np.arange(L)
    self_local = alphaT_full[idx, idx, :]
    alpha = np.concatenate(
        [clique, a_lr, a_rl, self_local, a_rr], axis=0
    ).astype(np.float32)

    rec_local = np.empty((N, IN_DIM), np.float32)
    rec_remote = np.empty((N, IN_DIM), np.float32)
    for c in range(NCORES):
        sl = slice(c * SH, (c + 1) * SH)
        slr = slice(L + c * SH, L + (c + 1) * SH)
        rec_local[sl] = outs[c]["rec_l"][0:SH]
        rec_local[slr] = outs[c]["rec_l"][SH : 2 * SH]
        rec_remote[sl] = outs[c]["rec_r"][0:SH]
        rec_remote[slr] = outs[c]["rec_r"][SH : 2 * SH]
    reconstructed = np.where((node_types_np == 1)[:, None], rec_local, rec_remote)

    return reconstructed.astype(np.float32), edge_index_np, alpha


# revision 22
# speedup vs baseline: 1.3190x; 1.0535x over previous
"""Self-contained Trainium2 (Bass/Tile) kernel for the DeviceGAT problem.

Computes, on 8 NeuronCores, the GAT layer + LayerNorm + two decoder MLPs of
reference.py, exploiting the deterministic graph structure:
  - edges = dense local clique (1024 local nodes, no self edges)
            + local->remote pairs (i -> L+i)
            + remote->local pairs (L+i -> i)
            + self loops for all 2048 nodes
  - so each local dst d has in-edges from all 1024 local nodes (incl. itself
    via the self loop) plus remote node L+d; each remote dst L+i has in-edges
    {i, L+i}.

Sharding: destination rows are sharded 8 ways (128 local dst rows + the
matching 128 remote dst rows per core).  Node features / params replicated.

Scores are built directly in transposed [src, dst] layout (a_dst broadcast by
a tiny selector matmul, a_src as the per-partition activation bias), so the
attention needs NO on-chip transposes.  exp(scores) is written to HBM
unnormalized together with the per-(dst, head) softmax reciprocal R; the host
applies R while permuting the dense block into the reference per-edge order.
The aggregation matmul contracts src on the partition axis (exb^T @ [x | 1])
in bf16, producing both the weighted message sum and the softmax denominator
for all four heads in one PSUM tile; per-head epilogue work is batched into a
handful of broadcast-AP vector ops.
"""

import os
import sys
import types
import numpy as np

# ---------------------------------------------------------------------------
# Problem constants (from the reference problem definition; deterministic).
L = 1024
N = 2 * L
MAPPED = 32
HID = 64
HEADS = 4
OUT = HID * HEADS        # 256
IN_DIM = 128
NEG_SLOPE = 0.2
EPS = 1e-5
NCORES = 8
SH = L // NCORES         # 128 dst rows per core
NLOC_T = L // 128        # 8 local node tiles


def _ensure_axon_hooks_stub():
    """run_bass_kernel_spmd(trace=True) imports antenv.axon_hooks; provide a
    graceful stub when the image lacks it so tracing degrades instead of
    crashing.  (Harness runs trace=False and never hits this, but be safe.)"""
    try:
        import antenv.axon_hooks  # noqa: F401
        return
    except Exception:
        pass
    try:
        import antenv
    except Exception:
        antenv = types.ModuleType("antenv")
        sys.modules["antenv"] = antenv
    mod = types.ModuleType("antenv.axon_hooks")
    mod._HOOK = None

    def set_axon_ntff_profile_hook(hook):
        mod._HOOK = hook

    def get_axon_ntff_profile_hook():
        if mod._HOOK is not None:
            return mod._HOOK
        so = "/opt/axon/libaxon_pjrt.so"
        if os.path.exists(so):
            import contextlib
            import ctypes

            try:
                lib = ctypes.CDLL(so)
            except OSError:
                return None
            if not hasattr(lib, "axon_start_nrt_profile"):
                return None
            lib.axon_start_nrt_profile.argtypes = [
                ctypes.POINTER(ctypes.c_int64),
                ctypes.c_size_t,
            ]
            lib.axon_start_nrt_profile.restype = ctypes.c_int64
            lib.axon_stop_nrt_profile.argtypes = [ctypes.c_char_p]
            lib.axon_stop_nrt_profile.restype = ctypes.c_int64

            @contextlib.contextmanager
            def _hook(output_dir, device_ids):
                import jax

                jax.devices()
                if device_ids:
                    ids = (ctypes.c_int64 * len(device_ids))(*device_ids)
                    rc = lib.axon_start_nrt_profile(ids, len(device_ids))
                else:
                    rc = lib.axon_start_nrt_profile(None, 0)
                if rc != 0:
                    raise RuntimeError(f"axon_start_nrt_profile rc={rc}")
                try:
                    yield
                finally:
                    n = lib.axon_stop_nrt_profile(str(output_dir).encode())
                    print(f"profile: {n} file(s) in {output_dir}", file=sys.stderr)

            mod._HOOK = _hook
            return mod._HOOK
        return None

    mod.set_axon_ntff_profile_hook = set_axon_ntff_profile_hook
    mod.get_axon_ntff_profile_hook = get_axon_ntff_profile_hook
    sys.modules["antenv.axon_hooks"] = mod


# ---------------------------------------------------------------------------
# Bass kernel builder


def _build_nc():
    from contextlib import ExitStack

    import concourse.bass as bass
    import concourse.tile as tile
    from concourse import bacc, mybir
    from concourse.masks import make_identity

    f32 = mybir.dt.float32
    bf16 = mybir.dt.bfloat16
    AF = mybir.ActivationFunctionType
    MUL = mybir.AluOpType.mult
    ADD = mybir.AluOpType.add
    WEXT = OUT + 2 * HEADS  # 264: [Wg | Wsrc | Wdst]

    nc = bacc.Bacc(
        trn_type="TRN2", target_bir_lowering=False, debug=False, num_devices=NCORES
    )

    # ---- I/O ----
    # nf_all: [32, N + SH + SH + WEXT] = nfT | nfT_loc | nfT_rem | Wg_ext
    NFC = N + 2 * SH + WEXT
    nf_all = nc.dram_tensor("nf_all", [MAPPED, NFC], f32, kind="ExternalInput")
    # dec_all: [128, 1792] = for each dec: W1(256) b1(128) W2(64) b2(64)
    # W3pad(128) b3(128); then Bg(256)
    DCC = 2 * (256 + 128 + 64 + 64 + 128 + 128) + OUT
    dec_all = nc.dram_tensor("dec_all", [128, DCC], f32, kind="ExternalInput")

    # dense unnormalized exp(scores), transposed layout, raw per-head blocks
    # [p, t, d]: src node s = t*128+p, dst column d
    alpha_t = nc.dram_tensor("alpha_t", [128, HEADS * L], f32, kind="ExternalOutput")
    # a_rl | a_lr | a_rr | R  (R = per-(d,h) softmax reciprocal)
    asmall = nc.dram_tensor("asmall", [SH, 16], f32, kind="ExternalOutput")
    rec_l = nc.dram_tensor("rec_l", [2 * SH, IN_DIM], f32, kind="ExternalOutput")
    rec_r = nc.dram_tensor("rec_r", [2 * SH, IN_DIM], f32, kind="ExternalOutput")

    with tile.TileContext(nc) as tc, ExitStack() as ctx:
        consts = ctx.enter_context(tc.tile_pool(name="consts", bufs=1))
        big = ctx.enter_context(tc.tile_pool(name="big", bufs=3))
        scp = ctx.enter_context(tc.tile_pool(name="scp", bufs=4))
        persist = ctx.enter_context(tc.tile_pool(name="persist", bufs=1))
        small = ctx.enter_context(tc.tile_pool(name="small", bufs=2))
        psB = ctx.enter_context(tc.tile_pool(name="psB", bufs=2, space="PSUM"))
        psG = ctx.enter_context(tc.tile_pool(name="psG", bufs=1, space="PSUM"))
        psT = ctx.enter_context(tc.tile_pool(name="psT", bufs=2, space="PSUM"))
        psX = ctx.enter_context(tc.tile_pool(name="psX", bufs=2, space="PSUM"))

        # ---- load constants (2 big DMAs) ----
        sb_nf = consts.tile([MAPPED, NFC], f32)
        nc.sync.dma_start(sb_nf[:], nf_all[:])
        sb_nfT = sb_nf[:, 0:N]
        sb_nfT_loc = sb_nf[:, N : N + SH]
        sb_nfT_rem = sb_nf[:, N + SH : N + 2 * SH]
        sb_Wg = sb_nf[:, N + 2 * SH : NFC]
        sb_dec = consts.tile([128, DCC], f32)
        nc.sync.dma_start(sb_dec[:], dec_all[:])
        # combined l|r decoder params
        dW1 = sb_dec[:, 0:512].rearrange("p (a n) -> p a n", a=2)  # [128,2,256]
        db1 = sb_dec[:, 512:768]
        dW2 = sb_dec[:, 768:896]      # cols 0:64 = W2l, 64:128 = W2r
        db2 = sb_dec[:, 896:1024]
        dW3 = sb_dec[0:64, 1024:1280]  # cols 0:128 = W3l, 128:256 = W3r
        db3 = sb_dec[:, 1280:1536]

        eps_col = consts.tile([128, 1], f32)
        nc.vector.memset(eps_col[:], EPS)
        ident = consts.tile([128, 128], f32)
        make_identity(nc, ident[:])
        # head-selector: sel4[k, h*128+m] = (k == h); used as k=4 lhsT to
        # broadcast row h of a [4, x] tile across 128 partitions
        sel4 = consts.tile([HEADS, HEADS * 128], f32)
        nc.gpsimd.memset(sel4[:], 0.0)
        sel4v = sel4.rearrange("p (a b) -> p a b", a=HEADS)
        nc.gpsimd.affine_select(
            out=sel4v,
            in_=sel4v,
            compare_op=mybir.AluOpType.not_equal,
            fill=1.0,
            base=0,
            pattern=[[-1, HEADS], [0, 128]],
            channel_multiplier=1,
        )
        sb_Bg = sb_dec[:, 1536 : 1536 + OUT]

        # ---- projections on PE: x|a_src|a_dst per node tile ----
        # shard rows first so the special-edge path unblocks early
        ps = psX.tile([128, WEXT], f32, tag="psx", name="ps_rem")
        nc.tensor.matmul(ps[:], sb_nfT_rem[:], sb_Wg[:])
        x_rem = persist.tile([128, OUT], f32, tag="xrem")
        nc.scalar.copy(x_rem[:], ps[:, 0:OUT])
        A_rem = persist.tile([128, 2 * HEADS], f32, tag="arem")
        nc.vector.tensor_copy(A_rem[:], ps[:, OUT:WEXT])

        ps = psX.tile([128, WEXT], f32, tag="psx", name="ps_loc")
        nc.tensor.matmul(ps[:], sb_nfT_loc[:], sb_Wg[:])
        x_shard = persist.tile([128, OUT], f32, tag="xshard")
        nc.vector.tensor_copy(x_shard[:], ps[:, 0:OUT])
        A_loc = persist.tile([128, 2 * HEADS], f32, tag="aloc")
        nc.scalar.copy(A_loc[:], ps[:, OUT:WEXT])

        # a_dst of the shard as rows: [4, 128]
        psd = psX.tile([128, WEXT], f32, tag="psx", name="psd")
        nc.tensor.matmul(
            psd[0:HEADS, 0:SH], sb_Wg[:, OUT + HEADS : WEXT], sb_nfT_loc[:]
        )
        a_dstT = consts.tile([HEADS, SH], f32)
        nc.scalar.copy(a_dstT[:], psd[0:HEADS, 0:SH])

        # ---- special-edge scores: e_rl | e_lr | e_rr  [128, 12] ----
        E3 = persist.tile([128, 12], f32, tag="E3")
        nc.vector.tensor_add(E3[:, 0:4], A_rem[:, 0:4], A_loc[:, 4:8])
        nc.vector.tensor_add(E3[:, 4:8], A_loc[:, 0:4], A_rem[:, 4:8])
        nc.vector.tensor_add(E3[:, 8:12], A_rem[:, 0:4], A_rem[:, 4:8])
        LR3 = persist.tile([128, 12], f32, tag="LR3")
        nc.scalar.activation(LR3[:], E3[:], AF.Prelu, alpha=NEG_SLOPE)
        EXP3 = persist.tile([128, 12], f32, tag="EXP3")
        nc.scalar.activation(EXP3[:], LR3[:], AF.Exp)

        # remote-dst softmax (2 edges) and h_rem (batched broadcast ops)
        asm = persist.tile([128, 16], f32, tag="asm")  # a_rl | a_lr | a_rr | R
        Zr = persist.tile([128, 4], f32, tag="Zr")
        nc.vector.tensor_add(Zr[:], EXP3[:, 4:8], EXP3[:, 8:12])
        Rr = persist.tile([128, 4], f32, tag="Rr")
        nc.vector.reciprocal(Rr[:], Zr[:])
        nc.vector.tensor_mul(asm[:, 4:8], EXP3[:, 4:8], Rr[:])
        nc.vector.tensor_mul(asm[:, 8:12], EXP3[:, 8:12], Rr[:])

        x_shard_v = x_shard.rearrange("p (h c) -> p h c", h=HEADS)
        x_rem_v = x_rem.rearrange("p (h c) -> p h c", h=HEADS)
        hrem_t1 = persist.tile([128, OUT], f32, tag="hrem_t1")
        nc.vector.tensor_mul(
            hrem_t1.rearrange("p (h c) -> p h c", h=HEADS),
            x_shard_v, asm[:, 4:8].to_broadcast((128, HEADS, HID)),
        )
        hrem_t2 = persist.tile([128, OUT], f32, tag="hrem_t2")
        nc.vector.tensor_mul(
            hrem_t2.rearrange("p (h c) -> p h c", h=HEADS),
            x_rem_v, asm[:, 8:12].to_broadcast((128, HEADS, HID)),
        )
        h_rem = persist.tile([128, OUT], f32, tag="h_rem")
        nc.vector.tensor_add(h_rem[:], hrem_t1[:], hrem_t2[:])
        nc.vector.tensor_add(h_rem[:], h_rem[:], sb_Bg[:])

        # local node tiles
        xbf_all = persist.tile([128, NLOC_T, HEADS, HID + 1], bf16, tag="xbf")
        nc.gpsimd.memset(xbf_all[:], 1.0)
        A_src = persist.tile([128, NLOC_T, HEADS], f32, tag="asrc")
        for t in range(NLOC_T):
            ps = psX.tile([128, WEXT], f32, tag="psx")
            nc.tensor.matmul(ps[:], sb_nfT[:, t * 128 : (t + 1) * 128], sb_Wg[:])
            nc.vector.tensor_copy(
                xbf_all[:, t, :, 0:HID],
                ps[:, 0:OUT].rearrange("p (h c) -> p h c", h=HEADS),
            )
            nc.scalar.copy(A_src[:, t, :], ps[:, OUT : OUT + HEADS])

        # ---- layernorm helper (stats DVE, sqrt ACT, apply DVE) ----
        BNS = nc.vector.BN_STATS_DIM
        BNA = nc.vector.BN_AGGR_DIM

        def layer_norm(x_t, width, tagp, groups=1):
            g, w = groups, width // groups
            xv = x_t[:, 0:width].rearrange("p (g w) -> p g w", g=g)
            st = small.tile([128, g, BNS], f32, tag="bnst")
            mv = small.tile([128, g, BNA], f32, tag="bnmv")
            for i in range(g):
                nc.vector.bn_stats(st[:, i, :], xv[:, i, :])
                nc.vector.bn_aggr(mv[:, i, :], st[:, i, :])
            sdev = small.tile([128, g], f32, tag="sdev")
            nc.scalar.activation(sdev[:], mv[:, :, 1], AF.Sqrt, bias=eps_col[:])
            rstd = small.tile([128, g], f32, tag="rstd")
            nc.vector.reciprocal(rstd[:], sdev[:])
            negm = small.tile([128, g], f32, tag="negm")
            nc.vector.tensor_scalar(negm[:], mv[:, :, 0], -1.0, None, op0=MUL)
            out_t = big.tile([128, width], f32, tag=tagp)
            ov = out_t.rearrange("p (g w) -> p g w", g=g)
            nc.vector.tensor_add(ov, xv, negm[:].to_broadcast((128, g, w)))
            nc.vector.tensor_mul(ov, ov, rstd[:].to_broadcast((128, g, w)))
            return out_t

        # ---- decoder machinery (f32) ----
        rec_sb = {
            "l": persist.tile([128, 2, IN_DIM], f32, tag="recl", name="recl"),
            "r": persist.tile([128, 2, IN_DIM], f32, tag="recr", name="recr"),
        }

        def transpose_h(hn_t, nt):
            psTt = psT.tile([128, 4, 128], f32, tag="psT")
            nc.tensor.transpose(psTt[:, 0, :], hn_t[:, 0:128], ident[:])
            nc.tensor.transpose(psTt[:, 1, :], hn_t[:, 128:256], ident[:])
            ha = big.tile([128, 128], f32, tag=f"hTa{nt}", name=f"hTa{nt}")
            hb = big.tile([128, 128], f32, tag=f"hTb{nt}", name=f"hTb{nt}")
            if nt == 0:
                nc.scalar.copy(ha[:], psTt[:, 0, :])
                nc.vector.tensor_copy(hb[:], psTt[:, 1, :])
            else:
                nc.vector.tensor_copy(ha[:], psTt[:, 0, :])
                nc.scalar.copy(hb[:], psTt[:, 1, :])
            return ha, hb

        def run_decoders_nt(nt, ha, hb):
            # both decoders (l|r) processed jointly in the free dim
            ps1 = psX.tile([128, WEXT], f32, tag="psx")
            nc.tensor.matmul(ps1[:, 0:256], ha[:], dW1[:, 0, :],
                             start=True, stop=False)
            nc.tensor.matmul(ps1[:, 0:256], hb[:], dW1[:, 1, :],
                             start=False, stop=True)
            s1 = big.tile([128, 256], f32, tag="s1")
            nc.vector.tensor_add(s1[:], ps1[:, 0:256], db1[:])
            r1 = big.tile([128, 256], f32, tag="r1")
            nc.scalar.activation(r1[:], s1[:], AF.Relu)
            n1 = layer_norm(r1, 256, "n1", groups=2)
            psn = psT.tile([128, 4, 128], f32, tag="psT")
            nc.tensor.transpose(psn[:, 0, :], n1[:, 0:128], ident[:])
            nc.tensor.transpose(psn[:, 1, :], n1[:, 128:256], ident[:])
            n1lT = big.tile([128, 128], f32, tag="n1lT")
            n1rT = big.tile([128, 128], f32, tag="n1rT")
            if nt == 0:
                nc.scalar.copy(n1lT[:], psn[:, 0, :])
                nc.vector.tensor_copy(n1rT[:], psn[:, 1, :])
            else:
                nc.vector.tensor_copy(n1lT[:], psn[:, 0, :])
                nc.scalar.copy(n1rT[:], psn[:, 1, :])

            ps2 = psX.tile([128, WEXT], f32, tag="psx")
            nc.tensor.matmul(ps2[:, 0:64], n1lT[:], dW2[:, 0:64],
                             start=True, stop=True)
            nc.tensor.matmul(ps2[:, 64:128], n1rT[:], dW2[:, 64:128],
                             start=True, stop=True)
            s2 = big.tile([128, 128], f32, tag="s2")
            nc.vector.tensor_add(s2[:], ps2[:, 0:128], db2[:])
            r2 = big.tile([128, 128], f32, tag="r2")
            nc.scalar.activation(r2[:], s2[:], AF.Relu)
            n2 = layer_norm(r2, 128, "n2", groups=2)
            psn2 = psT.tile([128, 4, 128], f32, tag="psT")
            nc.tensor.transpose(psn2[0:64, 0, :], n2[:, 0:64], ident[:])
            nc.tensor.transpose(psn2[0:64, 1, :], n2[:, 64:128], ident[:])
            n2lT = big.tile([64, 128], f32, tag="n2lT")
            n2rT = big.tile([64, 128], f32, tag="n2rT")
            if nt == 0:
                nc.vector.tensor_copy(n2lT[:], psn2[0:64, 0, :])
                nc.scalar.copy(n2rT[:], psn2[0:64, 1, :])
            else:
                nc.scalar.copy(n2lT[:], psn2[0:64, 0, :])
                nc.vector.tensor_copy(n2rT[:], psn2[0:64, 1, :])

            ps3 = psX.tile([128, WEXT], f32, tag="psx")
            nc.tensor.matmul(ps3[:, 0:128], n2lT[:], dW3[:, 0:128],
                             start=True, stop=True)
            nc.tensor.matmul(ps3[:, 128:256], n2rT[:], dW3[:, 128:256],
                             start=True, stop=True)
            nc.vector.tensor_add(rec_sb["l"][:, nt, :], ps3[:, 0:128], db3[:, 0:128])
            nc.vector.tensor_add(rec_sb["r"][:, nt, :], ps3[:, 128:256], db3[:, 128:256])

        # ---- dense attention per head (transposed [s, d] layout) ----
        # all 4 a_dst broadcasts in one PSUM bank
        psBh = psB.tile([128, HEADS, SH], f32, tag="Bh")
        for h in range(HEADS):
            nc.tensor.matmul(
                psBh[:, h, :], sel4[:, h * 128 : (h + 1) * 128], a_dstT[:]
            )
        # S[p, t, d] = a_src[t*128+p, h] + a_dst[d] for all heads up front, so
        # the DVE stream never stalls behind ACT
        S_hs = []
        for h in range(HEADS):
            S_h = scp.tile([128, NLOC_T, SH], f32, tag="eT", name=f"S{h}")
            bh_slice = psBh[:, h, :]
            bh_bcast = bass.AP(
                tensor=bh_slice.tensor,
                offset=bh_slice.offset,
                ap=[bh_slice.ap[0], [0, NLOC_T], bh_slice.ap[1]],
            )
            nc.vector.tensor_add(
                S_h[:], bh_bcast, A_src[:, :, h].to_broadcast((128, NLOC_T, SH))
            )
            S_hs.append(S_h)

        # all 4 heads accumulate into one PSUM tile: [:, h, 0:64] = msg sum,
        # [:, h, 64] = sum(exp)
        psagg = psG.tile([128, HEADS, HID + 1], f32, tag="agg")
        for h in range(HEADS):
            S_h = S_hs[h]
            ex_T = big.tile([128, NLOC_T, SH], f32, tag="exT")
            eflat = S_h.rearrange("p a b -> p (a b)")
            nc.scalar.activation(eflat, eflat, AF.Prelu, alpha=NEG_SLOPE)
            nc.scalar.activation(
                ex_T.rearrange("p a b -> p (a b)"), eflat, AF.Exp
            )
            # unnormalized exp block straight to HBM (host normalizes)
            nc.sync.dma_start(
                alpha_t[:, h * L : (h + 1) * L],
                ex_T.rearrange("p a b -> p (a b)"),
            )
            exb = big.tile([128, NLOC_T, SH], bf16, tag="exb")
            nc.vector.tensor_copy(exb[:], ex_T[:])

            # aggregation + Z in one accumulation: exb[s,:]^T @ [x_h | 1]
            for t in range(NLOC_T):
                nc.tensor.matmul(
                    psagg[:, h, :],
                    exb[:, t, :],
                    xbf_all[:, t, h, :],
                    start=(t == 0),
                    stop=(t == NLOC_T - 1),
                )

        # remote half of the decoders (fills engine gaps of the head loop)
        hn_rem = layer_norm(h_rem, OUT, "hn")
        ha1, hb1 = transpose_h(hn_rem, 1)
        run_decoders_nt(1, ha1, hb1)
        nc.sync.dma_start(rec_l[SH : 2 * SH, :], rec_sb["l"][:, 1, :])
        nc.sync.dma_start(rec_r[SH : 2 * SH, :], rec_sb["r"][:, 1, :])

        # ---- batched per-head epilogue ----
        # Zf = psagg[:, :, 64] + exp(e_rl);  R = 1/Zf
        Zf = small.tile([128, HEADS], f32, tag="Zf")
        nc.vector.tensor_add(Zf[:], psagg[:, :, HID], EXP3[:, 0:4])
        nc.vector.reciprocal(asm[:, 12:16], Zf[:])
        nc.vector.tensor_mul(asm[:, 0:4], EXP3[:, 0:4], asm[:, 12:16])
        # h_loc = (agg + exp(e_rl)*x_rem) * R + bgat
        xt1 = persist.tile([128, OUT], f32, tag="xt1")
        xt1_v = xt1.rearrange("p (h c) -> p h c", h=HEADS)
        nc.vector.tensor_mul(
            xt1_v, x_rem_v, EXP3[:, 0:4].to_broadcast((128, HEADS, HID))
        )
        hpre = persist.tile([128, OUT], f32, tag="hpre")
        hpre_v = hpre.rearrange("p (h c) -> p h c", h=HEADS)
        nc.vector.tensor_add(hpre_v, psagg[:, :, 0:HID], xt1_v)
        h_loc = persist.tile([128, OUT], f32, tag="h_loc")
        nc.vector.tensor_mul(
            h_loc.rearrange("p (h c) -> p h c", h=HEADS),
            hpre_v, asm[:, 12:16].to_broadcast((128, HEADS, HID)),
        )
        nc.vector.tensor_add(h_loc[:], h_loc[:], sb_Bg[:])
        nc.sync.dma_start(asmall[:], asm[:])

        # ---- local half of decoders ----
        hn_loc = layer_norm(h_loc, OUT, "hn")
        ha0, hb0 = transpose_h(hn_loc, 0)
        run_decoders_nt(0, ha0, hb0)

        nc.sync.dma_start(rec_l[0:SH, :], rec_sb["l"][:, 0, :])
        nc.sync.dma_start(rec_r[0:SH, :], rec_sb["r"][:, 0, :])

    nc.compile()
    return nc


_NC_CACHE = {}


def _get_nc():
    if "nc" not in _NC_CACHE:
        _NC_CACHE["nc"] = _build_nc()
    return _NC_CACHE["nc"]


# ---------------------------------------------------------------------------
# Host side


def _expected_edge_index():
    local = np.arange(L)
    s = np.repeat(local, L)
    d = np.tile(local, L)
    m = s != d
    src = np.concatenate([s[m], local, L + local, np.arange(N)])
    dst = np.concatenate([d[m], L + local, local, np.arange(N)])
    return np.stack([src, dst])


def _np_reference_fallback(node_features, gat, norm, dec_local, dec_remote,
                           node_types, edge_index):
    """Pure-numpy replica of the reference; used only if the edge structure
    is not the expected deterministic pattern."""
    x = node_features @ gat["W"]
    xh = x.reshape(N, HEADS, HID)
    a_src = (xh * gat["att_src"]).sum(-1)
    a_dst = (xh * gat["att_dst"]).sum(-1)
    src, dst = edge_index[0], edge_index[1]
    e = a_src[src] + a_dst[dst]
    e = np.where(e >= 0, e, NEG_SLOPE * e).astype(np.float32)
    emax = np.full((N, HEADS), -np.inf, np.float32)
    np.maximum.at(emax, dst, e)
    ex = np.exp(e - emax[dst])
    zs = np.zeros((N, HEADS), np.float32)
    np.add.at(zs, dst, ex)
    alpha = ex / zs[dst]
    msg = xh[src] * alpha[:, :, None]
    out = np.zeros((N, HEADS, HID), np.float32)
    np.add.at(out, dst, msg)
    out = out.reshape(N, OUT) + gat["bias"]

    def ln(v, g, b):
        m = v.mean(-1, keepdims=True)
        var = ((v - m) ** 2).mean(-1, keepdims=True)
        return (v - m) / np.sqrt(var + EPS) * g + b

    h = ln(out, norm["g"], norm["b"])

    def dec(v, p):
        t = ln(np.maximum(v @ p["W1"] + p["b1"], 0), p["g1"], p["b1n"])
        t = ln(np.maximum(t @ p["W2"] + p["b2"], 0), p["g2"], p["b2n"])
        return t @ p["W3"] + p["b3"]

    rl = dec(h, dec_local)
    rr = dec(h, dec_remote)
    rec = np.where((node_types == 1)[:, None], rl, rr)
    return rec.astype(np.float32), edge_index, alpha.astype(np.float32)


def _to_np(v):
    return {k: _to_np(x) for k, x in v.items()} if isinstance(v, dict) else np.asarray(v)


def kernel(node_features, gat, norm, dec_local, dec_remote, node_types,
           edge_index, trace=False):
    _ensure_axon_hooks_stub()
    node_features = np.asarray(node_features, np.float32)
    gat, norm = _to_np(gat), _to_np(norm)
    dec_local, dec_remote = _to_np(dec_local), _to_np(dec_remote)
    node_types_np = np.asarray(node_types)
    edge_index_np = np.asarray(edge_index)

    if not np.array_equal(edge_index_np.astype(np.int64), _expected_edge_index()):
        return _np_reference_fallback(
            node_features, gat, norm, dec_local, dec_remote,
            node_types_np, edge_index_np,
        )

    from concourse.bass_utils import run_bass_kernel_spmd

    f32 = np.float32
    W = gat["W"].astype(f32)
    att_src = gat["att_src"].astype(f32)
    att_dst = gat["att_dst"].astype(f32)
    Wsrc = (W.reshape(MAPPED, HEADS, HID) * att_src[None]).sum(-1)
    Wdst = (W.reshape(MAPPED, HEADS, HID) * att_dst[None]).sum(-1)
    Wg_ext = np.ascontiguousarray(np.concatenate([W, Wsrc, Wdst], axis=1))
    nfT = np.ascontiguousarray(node_features.T)

    # fold LN affine params into the following linear layer; biases
    # pre-broadcast to [128, w]
    def fold(dec):
        g, b = norm["g"].astype(f32), norm["b"].astype(f32)
        W1 = g[:, None] * dec["W1"]
        b1 = dec["b1"] + b @ dec["W1"]
        W2 = dec["g1"][:, None] * dec["W2"]
        b2 = dec["b2"] + dec["b1n"] @ dec["W2"]
        W3 = dec["g2"][:, None] * dec["W3"]
        b3 = dec["b3"] + dec["b2n"] @ dec["W3"]

        def bc(v):
            return np.ascontiguousarray(
                np.broadcast_to(v.reshape(1, -1).astype(f32), (128, v.size))
            )

        return (
            np.ascontiguousarray(W1, f32), bc(b1),
            np.ascontiguousarray(W2, f32), bc(b2),
            np.ascontiguousarray(W3, f32), bc(b3),
        )

    # dec_all (combined l|r): W1lr(512, rearranged) b1lr(256) W2lr(128)
    # b2lr(128) W3lr-pad(256) b3lr(256) Bg(256)
    Fl = fold(dec_local)
    Fr = fold(dec_remote)
    W1lr = np.concatenate([Fl[0], Fr[0]], axis=1)          # [256, 256]
    W1lr = W1lr.reshape(2, 128, 256).transpose(1, 0, 2).reshape(128, 512)
    b1lr = np.concatenate([Fl[1], Fr[1]], axis=1)          # [128, 256]
    W2lr = np.concatenate([Fl[2], Fr[2]], axis=1)          # [128, 128]
    b2lr = np.concatenate([Fl[3], Fr[3]], axis=1)          # [128, 128]
    W3p = np.zeros((128, 256), f32)
    W3p[0:64, 0:128] = Fl[4]
    W3p[0:64, 128:256] = Fr[4]
    b3lr = np.concatenate([Fl[5], Fr[5]], axis=1)          # [128, 256]
    Bg = np.broadcast_to(gat["bias"].reshape(1, -1).astype(f32), (128, OUT))
    dec_all = np.ascontiguousarray(
        np.concatenate([W1lr, b1lr, W2lr, b2lr, W3p, b3lr, Bg], axis=1), f32
    )

    in_maps = []
    for c in range(NCORES):
        nf_all = np.concatenate(
            [
                nfT,
                nfT[:, c * SH : (c + 1) * SH],
                nfT[:, L + c * SH : L + (c + 1) * SH],
                Wg_ext,
            ],
            axis=1,
        )
        in_maps.append({"nf_all": np.ascontiguousarray(nf_all), "dec_all": dec_all})

    nc = _get_nc()
    res = run_bass_kernel_spmd(nc, in_maps, core_ids=list(range(NCORES)), trace=trace)
    _NC_CACHE["last_results"] = res
    outs = res.results

    # ---- host unshard / assembly ----
    # alphaT_full[s, d, h] = exp(e) * R
    alphaT_full = np.empty((L, L, HEADS), np.float32)
    a_rl = np.empty((L, HEADS), np.float32)
    a_lr = np.empty((L, HEADS), np.float32)
    a_rr = np.empty((L, HEADS), np.float32)
    for c in range(NCORES):
        sl = slice(c * SH, (c + 1) * SH)
        sm = outs[c]["asmall"]
        a_rl[sl], a_lr[sl], a_rr[sl] = sm[:, 0:4], sm[:, 4:8], sm[:, 8:12]
        R = sm[:, 12:16]  # [d, h]
        blk = outs[c]["alpha_t"].reshape(128, HEADS, NLOC_T, SH)  # [p, h, t, d]
        # -> [s=t*128+p, d, h]
        blk = np.transpose(blk, (2, 0, 3, 1)).reshape(L, SH, HEADS)
        alphaT_full[:, sl, :] = blk * R[None, :, :]

    per_edge = alphaT_full.reshape(L * L, HEADS)
    mask = ~np.eye(L, dtype=bool).reshape(-1)
    clique = per_edge[mask]
    idx = np.arange(L)
    self_local = alphaT_full[idx, idx, :]
    alpha = np.concatenate(
        [clique, a_lr, a_rl, self_local, a_rr], axis=0
    ).astype(np.float32)

    rec_local = np.empty((N, IN_DIM), np.float32)
    rec_remote = np.empty((N, IN_DIM), np.float32)
    for c in range(NCORES):
        sl = slice(c * SH, (c + 1) * SH)
        slr = slice(L + c * SH, L + (c + 1) * SH)
        rec_local[sl] = outs[c]["rec_l"][0:SH]
        rec_local[slr] = outs[c]["rec_l"][SH : 2 * SH]
        rec_remote[sl] = outs[c]["rec_r"][0:SH]
        rec_remote[slr] = outs[c]["rec_r"][SH : 2 * SH]
    reconstructed = np.where((node_types_np == 1)[:, None], rec_local, rec_remote)

    return reconstructed.astype(np.float32), edge_index_np, alpha


# revision 24
# speedup vs baseline: 1.3271x; 1.0061x over previous
"""Self-contained Trainium2 (Bass/Tile) kernel for the DeviceGAT problem.

Computes, on 8 NeuronCores, the GAT layer + LayerNorm + two decoder MLPs of
reference.py, exploiting the deterministic graph structure:
  - edges = dense local clique (1024 local nodes, no self edges)
            + local->remote pairs (i -> L+i)
            + remote->local pairs (L+i -> i)
            + self loops for all 2048 nodes
  - so each local dst d has in-edges from all 1024 local nodes (incl. itself
    via the self loop) plus remote node L+d; each remote dst L+i has in-edges
    {i, L+i}.

Sharding: destination rows are sharded 8 ways (128 local dst rows + the
matching 128 remote dst rows per core).  Node features / params replicated.

Scores are built directly in transposed [src, dst] layout (a_dst broadcast by
a tiny selector matmul, a_src as the per-partition activation bias), so the
attention needs NO on-chip transposes.  exp(scores) is written to HBM
unnormalized together with the per-(dst, head) softmax reciprocal R; the host
applies R while permuting the dense block into the reference per-edge order.
The aggregation matmul contracts src on the partition axis (exb^T @ [x | 1])
in bf16, producing both the weighted message sum and the softmax denominator
for all four heads in one PSUM tile; per-head epilogue work is batched into a
handful of broadcast-AP vector ops.
"""

import os
import sys
import types
import numpy as np

# ---------------------------------------------------------------------------
# Problem constants (from the reference problem definition; deterministic).
L = 1024
N = 2 * L
MAPPED = 32
HID = 64
HEADS = 4
OUT = HID * HEADS        # 256
IN_DIM = 128
NEG_SLOPE = 0.2
EPS = 1e-5
NCORES = 8
SH = L // NCORES         # 128 dst rows per core
NLOC_T = L // 128        # 8 local node tiles


def _ensure_axon_hooks_stub():
    """run_bass_kernel_spmd(trace=True) imports antenv.axon_hooks; provide a
    graceful stub when the image lacks it so tracing degrades instead of
    crashing.  (Harness runs trace=False and never hits this, but be safe.)"""
    try:
        import antenv.axon_hooks  # noqa: F401
        return
    except Exception:
        pass
    try:
        import antenv
    except Exception:
        antenv = types.ModuleType("antenv")
        sys.modules["antenv"] = antenv
    mod = types.ModuleType("antenv.axon_hooks")
    mod._HOOK = None

    def set_axon_ntff_profile_hook(hook):
        mod._HOOK = hook

    def get_axon_ntff_profile_hook():
        if mod._HOOK is not None:
            return mod._HOOK
        so = "/opt/axon/libaxon_pjrt.so"
        if os.path.exists(so):
            import contextlib
            import ctypes

            try:
                lib = ctypes.CDLL(so)
            except OSError:
                return None
            if not hasattr(lib, "axon_start_nrt_profile"):
                return None
            lib.axon_start_nrt_profile.argtypes = [
                ctypes.POINTER(ctypes.c_int64),
                ctypes.c_size_t,
            ]
            lib.axon_start_nrt_profile.restype = ctypes.c_int64
            lib.axon_stop_nrt_profile.argtypes = [ctypes.c_char_p]
            lib.axon_stop_nrt_profile.restype = ctypes.c_int64

            @contextlib.contextmanager
            def _hook(output_dir, device_ids):
                import jax

                jax.devices()
                if device_ids:
                    ids = (ctypes.c_int64 * len(device_ids))(*device_ids)
                    rc = lib.axon_start_nrt_profile(ids, len(device_ids))
                else:
                    rc = lib.axon_start_nrt_profile(None, 0)
                if rc != 0:
                    raise RuntimeError(f"axon_start_nrt_profile rc={rc}")
                try:
                    yield
                finally:
                    n = lib.axon_stop_nrt_profile(str(output_dir).encode())
                    print(f"profile: {n} file(s) in {output_dir}", file=sys.stderr)

            mod._HOOK = _hook
            return mod._HOOK
        return None

    mod.set_axon_ntff_profile_hook = set_axon_ntff_profile_hook
    mod.get_axon_ntff_profile_hook = get_axon_ntff_profile_hook
    sys.modules["antenv.axon_hooks"] = mod


# ---------------------------------------------------------------------------
# Bass kernel builder


def _build_nc():
    from contextlib import ExitStack

    import concourse.bass as bass
    import concourse.tile as tile
    from concourse import bacc, mybir
    from concourse.masks import make_identity

    f32 = mybir.dt.float32
    bf16 = mybir.dt.bfloat16
    AF = mybir.ActivationFunctionType
    MUL = mybir.AluOpType.mult
    ADD = mybir.AluOpType.add
    MAX = mybir.AluOpType.max
    WEXT = OUT + 2 * HEADS  # 264: [Wg | Wsrc | Wdst]

    nc = bacc.Bacc(
        trn_type="TRN2", target_bir_lowering=False, debug=False, num_devices=NCORES
    )

    # ---- I/O ----
    # nf_all: [32, N + SH + SH + WEXT] = nfT | nfT_loc | nfT_rem | Wg_ext
    NFC = N + 2 * SH + WEXT
    nf_all = nc.dram_tensor("nf_all", [MAPPED, NFC], f32, kind="ExternalInput")
    # dec_all: [128, 1792] = for each dec: W1(256) b1(128) W2(64) b2(64)
    # W3pad(128) b3(128); then Bg(256)
    DCC = 2 * (256 + 128 + 64 + 64 + 128 + 128) + OUT
    dec_all = nc.dram_tensor("dec_all", [128, DCC], f32, kind="ExternalInput")

    # dense unnormalized exp(scores), transposed layout, raw per-head blocks
    # [p, t, d]: src node s = t*128+p, dst column d
    alpha_t = nc.dram_tensor("alpha_t", [128, HEADS * L], f32, kind="ExternalOutput")
    # a_rl | a_lr | a_rr | R  (R = per-(d,h) softmax reciprocal)
    asmall = nc.dram_tensor("asmall", [SH, 16], f32, kind="ExternalOutput")
    rec_l = nc.dram_tensor("rec_l", [2 * SH, IN_DIM], f32, kind="ExternalOutput")
    rec_r = nc.dram_tensor("rec_r", [2 * SH, IN_DIM], f32, kind="ExternalOutput")

    with tile.TileContext(nc) as tc, ExitStack() as ctx:
        consts = ctx.enter_context(tc.tile_pool(name="consts", bufs=1))
        big = ctx.enter_context(tc.tile_pool(name="big", bufs=3))
        scp = ctx.enter_context(tc.tile_pool(name="scp", bufs=4))
        persist = ctx.enter_context(tc.tile_pool(name="persist", bufs=1))
        small = ctx.enter_context(tc.tile_pool(name="small", bufs=2))
        psB = ctx.enter_context(tc.tile_pool(name="psB", bufs=2, space="PSUM"))
        psG = ctx.enter_context(tc.tile_pool(name="psG", bufs=1, space="PSUM"))
        psT = ctx.enter_context(tc.tile_pool(name="psT", bufs=2, space="PSUM"))
        psX = ctx.enter_context(tc.tile_pool(name="psX", bufs=2, space="PSUM"))

        # ---- load constants (2 big DMAs) ----
        sb_nf = consts.tile([MAPPED, NFC], f32)
        nc.sync.dma_start(sb_nf[:], nf_all[:])
        sb_nfT = sb_nf[:, 0:N]
        sb_nfT_loc = sb_nf[:, N : N + SH]
        sb_nfT_rem = sb_nf[:, N + SH : N + 2 * SH]
        sb_Wg = sb_nf[:, N + 2 * SH : NFC]
        sb_dec = consts.tile([128, DCC], f32)
        nc.sync.dma_start(sb_dec[:], dec_all[:])
        # combined l|r decoder params
        dW1 = sb_dec[:, 0:512].rearrange("p (a n) -> p a n", a=2)  # [128,2,256]
        db1 = sb_dec[:, 512:768]
        dW2 = sb_dec[:, 768:896]      # cols 0:64 = W2l, 64:128 = W2r
        db2 = sb_dec[:, 896:1024]
        dW3 = sb_dec[0:64, 1024:1280]  # cols 0:128 = W3l, 128:256 = W3r
        db3 = sb_dec[:, 1280:1536]

        eps_col = consts.tile([128, 1], f32)
        nc.vector.memset(eps_col[:], EPS)
        ident = consts.tile([128, 128], f32)
        make_identity(nc, ident[:])
        # head-selector: sel4[k, h*128+m] = (k == h); used as k=4 lhsT to
        # broadcast row h of a [4, x] tile across 128 partitions
        sel4 = consts.tile([HEADS, HEADS * 128], f32)
        nc.gpsimd.memset(sel4[:], 0.0)
        sel4v = sel4.rearrange("p (a b) -> p a b", a=HEADS)
        nc.gpsimd.affine_select(
            out=sel4v,
            in_=sel4v,
            compare_op=mybir.AluOpType.not_equal,
            fill=1.0,
            base=0,
            pattern=[[-1, HEADS], [0, 128]],
            channel_multiplier=1,
        )
        sb_Bg = sb_dec[:, 1536 : 1536 + OUT]

        # ---- projections on PE: x|a_src|a_dst per node tile ----
        # shard rows first so the special-edge path unblocks early
        ps = psX.tile([128, WEXT], f32, tag="psx", name="ps_rem")
        nc.tensor.matmul(ps[:], sb_nfT_rem[:], sb_Wg[:])
        x_rem = persist.tile([128, OUT], f32, tag="xrem")
        nc.scalar.copy(x_rem[:], ps[:, 0:OUT])
        A_rem = persist.tile([128, 2 * HEADS], f32, tag="arem")
        nc.vector.tensor_copy(A_rem[:], ps[:, OUT:WEXT])

        ps = psX.tile([128, WEXT], f32, tag="psx", name="ps_loc")
        nc.tensor.matmul(ps[:], sb_nfT_loc[:], sb_Wg[:])
        x_shard = persist.tile([128, OUT], f32, tag="xshard")
        nc.vector.tensor_copy(x_shard[:], ps[:, 0:OUT])
        A_loc = persist.tile([128, 2 * HEADS], f32, tag="aloc")
        nc.scalar.copy(A_loc[:], ps[:, OUT:WEXT])

        # a_dst of the shard as rows: [4, 128]
        psd = psX.tile([128, WEXT], f32, tag="psx", name="psd")
        nc.tensor.matmul(
            psd[0:HEADS, 0:SH], sb_Wg[:, OUT + HEADS : WEXT], sb_nfT_loc[:]
        )
        a_dstT = consts.tile([HEADS, SH], f32)
        nc.scalar.copy(a_dstT[:], psd[0:HEADS, 0:SH])

        # ---- special-edge scores: e_rl | e_lr | e_rr  [128, 12] ----
        E3 = persist.tile([128, 12], f32, tag="E3")
        nc.vector.tensor_add(E3[:, 0:4], A_rem[:, 0:4], A_loc[:, 4:8])
        nc.vector.tensor_add(E3[:, 4:8], A_loc[:, 0:4], A_rem[:, 4:8])
        nc.vector.tensor_add(E3[:, 8:12], A_rem[:, 0:4], A_rem[:, 4:8])
        LR3 = persist.tile([128, 12], f32, tag="LR3")
        nc.scalar.activation(LR3[:], E3[:], AF.Prelu, alpha=NEG_SLOPE)
        EXP3 = persist.tile([128, 12], f32, tag="EXP3")
        nc.scalar.activation(EXP3[:], LR3[:], AF.Exp)

        # remote-dst softmax (2 edges) and h_rem (batched broadcast ops)
        asm = persist.tile([128, 16], f32, tag="asm")  # a_rl | a_lr | a_rr | R
        Zr = persist.tile([128, 4], f32, tag="Zr")
        nc.vector.tensor_add(Zr[:], EXP3[:, 4:8], EXP3[:, 8:12])
        Rr = persist.tile([128, 4], f32, tag="Rr")
        nc.vector.reciprocal(Rr[:], Zr[:])
        nc.vector.tensor_mul(asm[:, 4:8], EXP3[:, 4:8], Rr[:])
        nc.vector.tensor_mul(asm[:, 8:12], EXP3[:, 8:12], Rr[:])

        x_shard_v = x_shard.rearrange("p (h c) -> p h c", h=HEADS)
        x_rem_v = x_rem.rearrange("p (h c) -> p h c", h=HEADS)
        hrem_t1 = persist.tile([128, OUT], f32, tag="hrem_t1")
        nc.vector.tensor_mul(
            hrem_t1.rearrange("p (h c) -> p h c", h=HEADS),
            x_shard_v, asm[:, 4:8].to_broadcast((128, HEADS, HID)),
        )
        hrem_t2 = persist.tile([128, OUT], f32, tag="hrem_t2")
        nc.vector.tensor_mul(
            hrem_t2.rearrange("p (h c) -> p h c", h=HEADS),
            x_rem_v, asm[:, 8:12].to_broadcast((128, HEADS, HID)),
        )
        h_rem = persist.tile([128, OUT], f32, tag="h_rem")
        nc.vector.tensor_add(h_rem[:], hrem_t1[:], hrem_t2[:])
        nc.vector.tensor_add(h_rem[:], h_rem[:], sb_Bg[:])

        # local node tiles
        xbf_all = persist.tile([128, NLOC_T, HEADS, HID + 1], bf16, tag="xbf")
        nc.gpsimd.memset(xbf_all[:], 1.0)
        A_src = persist.tile([128, NLOC_T, HEADS], f32, tag="asrc")
        for t in range(NLOC_T):
            ps = psX.tile([128, WEXT], f32, tag="psx")
            nc.tensor.matmul(ps[:], sb_nfT[:, t * 128 : (t + 1) * 128], sb_Wg[:])
            nc.vector.tensor_copy(
                xbf_all[:, t, :, 0:HID],
                ps[:, 0:OUT].rearrange("p (h c) -> p h c", h=HEADS),
            )
            nc.scalar.copy(A_src[:, t, :], ps[:, OUT : OUT + HEADS])

        # ---- layernorm helper (stats DVE, sqrt ACT, apply DVE) ----
        BNS = nc.vector.BN_STATS_DIM
        BNA = nc.vector.BN_AGGR_DIM

        def layer_norm(x_t, width, tagp, groups=1):
            g, w = groups, width // groups
            xv = x_t[:, 0:width].rearrange("p (g w) -> p g w", g=g)
            st = small.tile([128, g, BNS], f32, tag="bnst")
            mv = small.tile([128, g, BNA], f32, tag="bnmv")
            for i in range(g):
                nc.vector.bn_stats(st[:, i, :], xv[:, i, :])
                nc.vector.bn_aggr(mv[:, i, :], st[:, i, :])
            sdev = small.tile([128, g], f32, tag="sdev")
            nc.scalar.activation(sdev[:], mv[:, :, 1], AF.Sqrt, bias=eps_col[:])
            rstd = small.tile([128, g], f32, tag="rstd")
            nc.vector.reciprocal(rstd[:], sdev[:])
            negm = small.tile([128, g], f32, tag="negm")
            nc.vector.tensor_scalar(negm[:], mv[:, :, 0], -1.0, None, op0=MUL)
            out_t = big.tile([128, width], f32, tag=tagp)
            ov = out_t.rearrange("p (g w) -> p g w", g=g)
            nc.vector.tensor_add(ov, xv, negm[:].to_broadcast((128, g, w)))
            nc.vector.tensor_mul(ov, ov, rstd[:].to_broadcast((128, g, w)))
            return out_t

        # ---- decoder machinery (f32) ----
        rec_sb = {
            "l": persist.tile([128, 2, IN_DIM], f32, tag="recl", name="recl"),
            "r": persist.tile([128, 2, IN_DIM], f32, tag="recr", name="recr"),
        }

        def transpose_h(hn_t, nt):
            psTt = psT.tile([128, 4, 128], f32, tag="psT")
            nc.tensor.transpose(psTt[:, 0, :], hn_t[:, 0:128], ident[:])
            nc.tensor.transpose(psTt[:, 1, :], hn_t[:, 128:256], ident[:])
            ha = big.tile([128, 128], f32, tag=f"hTa{nt}", name=f"hTa{nt}")
            hb = big.tile([128, 128], f32, tag=f"hTb{nt}", name=f"hTb{nt}")
            if nt == 0:
                nc.scalar.copy(ha[:], psTt[:, 0, :])
                nc.vector.tensor_copy(hb[:], psTt[:, 1, :])
            else:
                nc.vector.tensor_copy(ha[:], psTt[:, 0, :])
                nc.scalar.copy(hb[:], psTt[:, 1, :])
            return ha, hb

        def dec_stage1(nt, ha, hb):
            # both decoders (l|r) processed jointly in the free dim
            ps1 = psX.tile([128, WEXT], f32, tag="psx")
            nc.tensor.matmul(ps1[:, 0:256], ha[:], dW1[:, 0, :],
                             start=True, stop=False)
            nc.tensor.matmul(ps1[:, 0:256], hb[:], dW1[:, 1, :],
                             start=False, stop=True)
            s1 = big.tile([128, 256], f32, tag="s1")
            nc.vector.tensor_add(s1[:], ps1[:, 0:256], db1[:])
            r1 = big.tile([128, 256], f32, tag="r1")
            nc.vector.tensor_scalar(r1[:], s1[:], 0.0, None, op0=MAX)
            n1 = layer_norm(r1, 256, "n1", groups=2)
            psn = psT.tile([128, 4, 128], f32, tag="psT")
            nc.tensor.transpose(psn[:, 0, :], n1[:, 0:128], ident[:])
            nc.tensor.transpose(psn[:, 1, :], n1[:, 128:256], ident[:])
            n1lT = big.tile([128, 128], f32, tag="n1lT")
            n1rT = big.tile([128, 128], f32, tag="n1rT")
            if nt == 0:
                nc.scalar.copy(n1lT[:], psn[:, 0, :])
                nc.vector.tensor_copy(n1rT[:], psn[:, 1, :])
            else:
                nc.vector.tensor_copy(n1lT[:], psn[:, 0, :])
                nc.scalar.copy(n1rT[:], psn[:, 1, :])
            return n1lT, n1rT

        def dec_stage2(nt, n1lT, n1rT):
            ps2 = psX.tile([128, WEXT], f32, tag="psx")
            nc.tensor.matmul(ps2[:, 0:64], n1lT[:], dW2[:, 0:64],
                             start=True, stop=True)
            nc.tensor.matmul(ps2[:, 64:128], n1rT[:], dW2[:, 64:128],
                             start=True, stop=True)
            s2 = big.tile([128, 128], f32, tag="s2")
            nc.vector.tensor_add(s2[:], ps2[:, 0:128], db2[:])
            r2 = big.tile([128, 128], f32, tag="r2")
            nc.vector.tensor_scalar(r2[:], s2[:], 0.0, None, op0=MAX)
            n2 = layer_norm(r2, 128, "n2", groups=2)
            psn2 = psT.tile([128, 4, 128], f32, tag="psT")
            nc.tensor.transpose(psn2[0:64, 0, :], n2[:, 0:64], ident[:])
            nc.tensor.transpose(psn2[0:64, 1, :], n2[:, 64:128], ident[:])
            n2lT = big.tile([64, 128], f32, tag="n2lT")
            n2rT = big.tile([64, 128], f32, tag="n2rT")
            if nt == 0:
                nc.vector.tensor_copy(n2lT[:], psn2[0:64, 0, :])
                nc.scalar.copy(n2rT[:], psn2[0:64, 1, :])
            else:
                nc.scalar.copy(n2lT[:], psn2[0:64, 0, :])
                nc.vector.tensor_copy(n2rT[:], psn2[0:64, 1, :])
            return n2lT, n2rT

        def dec_stage3(nt, n2lT, n2rT):
            ps3 = psX.tile([128, WEXT], f32, tag="psx")
            nc.tensor.matmul(ps3[:, 0:128], n2lT[:], dW3[:, 0:128],
                             start=True, stop=True)
            nc.tensor.matmul(ps3[:, 128:256], n2rT[:], dW3[:, 128:256],
                             start=True, stop=True)
            nc.vector.tensor_add(rec_sb["l"][:, nt, :], ps3[:, 0:128], db3[:, 0:128])
            nc.vector.tensor_add(rec_sb["r"][:, nt, :], ps3[:, 128:256], db3[:, 128:256])

        def run_decoders_nt(nt, ha, hb):
            n1lT, n1rT = dec_stage1(nt, ha, hb)
            n2lT, n2rT = dec_stage2(nt, n1lT, n1rT)
            dec_stage3(nt, n2lT, n2rT)

        # ---- dense attention per head (transposed [s, d] layout) ----
        # all 4 a_dst broadcasts in one PSUM bank
        psBh = psB.tile([128, HEADS, SH], f32, tag="Bh")
        for h in range(HEADS):
            nc.tensor.matmul(
                psBh[:, h, :], sel4[:, h * 128 : (h + 1) * 128], a_dstT[:]
            )
        # S[p, t, d] = a_src[t*128+p, h] + a_dst[d] for all heads up front, so
        # the DVE stream never stalls behind ACT
        S_hs = []
        for h in range(HEADS):
            S_h = scp.tile([128, NLOC_T, SH], f32, tag="eT", name=f"S{h}")
            bh_slice = psBh[:, h, :]
            bh_bcast = bass.AP(
                tensor=bh_slice.tensor,
                offset=bh_slice.offset,
                ap=[bh_slice.ap[0], [0, NLOC_T], bh_slice.ap[1]],
            )
            nc.vector.tensor_add(
                S_h[:], bh_bcast, A_src[:, :, h].to_broadcast((128, NLOC_T, SH))
            )
            S_hs.append(S_h)

        # all 4 heads accumulate into one PSUM tile: [:, h, 0:64] = msg sum,
        # [:, h, 64] = sum(exp)
        psagg = psG.tile([128, HEADS, HID + 1], f32, tag="agg")
        nt1_state = {}

        def nt1_piece(step):
            # spread the remote-half decoder chain across head iterations
            if step == 0:
                hn_rem = layer_norm(h_rem, OUT, "hn")
                nt1_state["h"] = transpose_h(hn_rem, 1)
            elif step == 1:
                nt1_state["n1"] = dec_stage1(1, *nt1_state["h"])
            elif step == 2:
                nt1_state["n2"] = dec_stage2(1, *nt1_state["n1"])
            elif step == 3:
                dec_stage3(1, *nt1_state["n2"])
                nc.sync.dma_start(rec_l[SH : 2 * SH, :], rec_sb["l"][:, 1, :])
                nc.sync.dma_start(rec_r[SH : 2 * SH, :], rec_sb["r"][:, 1, :])

        for h in range(HEADS):
            S_h = S_hs[h]
            ex_T = big.tile([128, NLOC_T, SH], f32, tag="exT")
            eflat = S_h.rearrange("p a b -> p (a b)")
            nc.scalar.activation(eflat, eflat, AF.Prelu, alpha=NEG_SLOPE)
            nc.scalar.activation(
                ex_T.rearrange("p a b -> p (a b)"), eflat, AF.Exp
            )
            # unnormalized exp block straight to HBM (host normalizes)
            nc.sync.dma_start(
                alpha_t[:, h * L : (h + 1) * L],
                ex_T.rearrange("p a b -> p (a b)"),
            )
            exb = big.tile([128, NLOC_T, SH], bf16, tag="exb")
            nc.vector.tensor_copy(exb[:], ex_T[:])

            # aggregation + Z in one accumulation: exb[s,:]^T @ [x_h | 1]
            for t in range(NLOC_T):
                nc.tensor.matmul(
                    psagg[:, h, :],
                    exb[:, t, :],
                    xbf_all[:, t, h, :],
                    start=(t == 0),
                    stop=(t == NLOC_T - 1),
                )
            nt1_piece(h)

        # ---- batched per-head epilogue ----
        # Zf = psagg[:, :, 64] + exp(e_rl);  R = 1/Zf
        Zf = small.tile([128, HEADS], f32, tag="Zf")
        nc.vector.tensor_add(Zf[:], psagg[:, :, HID], EXP3[:, 0:4])
        nc.vector.reciprocal(asm[:, 12:16], Zf[:])
        nc.vector.tensor_mul(asm[:, 0:4], EXP3[:, 0:4], asm[:, 12:16])
        # h_loc = (agg + exp(e_rl)*x_rem) * R + bgat
        xt1 = persist.tile([128, OUT], f32, tag="xt1")
        xt1_v = xt1.rearrange("p (h c) -> p h c", h=HEADS)
        nc.vector.tensor_mul(
            xt1_v, x_rem_v, EXP3[:, 0:4].to_broadcast((128, HEADS, HID))
        )
        hpre = persist.tile([128, OUT], f32, tag="hpre")
        hpre_v = hpre.rearrange("p (h c) -> p h c", h=HEADS)
        nc.vector.tensor_add(hpre_v, psagg[:, :, 0:HID], xt1_v)
        h_loc = persist.tile([128, OUT], f32, tag="h_loc")
        nc.vector.tensor_mul(
            h_loc.rearrange("p (h c) -> p h c", h=HEADS),
            hpre_v, asm[:, 12:16].to_broadcast((128, HEADS, HID)),
        )
        nc.vector.tensor_add(h_loc[:], h_loc[:], sb_Bg[:])
        nc.sync.dma_start(asmall[:], asm[:])

        # ---- local half of decoders ----
        hn_loc = layer_norm(h_loc, OUT, "hn")
        ha0, hb0 = transpose_h(hn_loc, 0)
        run_decoders_nt(0, ha0, hb0)

        nc.sync.dma_start(rec_l[0:SH, :], rec_sb["l"][:, 0, :])
        nc.sync.dma_start(rec_r[0:SH, :], rec_sb["r"][:, 0, :])

    nc.compile()
    return nc


_NC_CACHE = {}


def _get_nc():
    if "nc" not in _NC_CACHE:
        _NC_CACHE["nc"] = _build_nc()
    return _NC_CACHE["nc"]


# ---------------------------------------------------------------------------
# Host side


def _expected_edge_index():
    local = np.arange(L)
    s = np.repeat(local, L)
    d = np.tile(local, L)
    m = s != d
    src = np.concatenate([s[m], local, L + local, np.arange(N)])
    dst = np.concatenate([d[m], L + local, local, np.arange(N)])
    return np.stack([src, dst])


def _np_reference_fallback(node_features, gat, norm, dec_local, dec_remote,
                           node_types, edge_index):
    """Pure-numpy replica of the reference; used only if the edge structure
    is not the expected deterministic pattern."""
    x = node_features @ gat["W"]
    xh = x.reshape(N, HEADS, HID)
    a_src = (xh * gat["att_src"]).sum(-1)
    a_dst = (xh * gat["att_dst"]).sum(-1)
    src, dst = edge_index[0], edge_index[1]
    e = a_src[src] + a_dst[dst]
    e = np.where(e >= 0, e, NEG_SLOPE * e).astype(np.float32)
    emax = np.full((N, HEADS), -np.inf, np.float32)
    np.maximum.at(emax, dst, e)
    ex = np.exp(e - emax[dst])
    zs = np.zeros((N, HEADS), np.float32)
    np.add.at(zs, dst, ex)
    alpha = ex / zs[dst]
    msg = xh[src] * alpha[:, :, None]
    out = np.zeros((N, HEADS, HID), np.float32)
    np.add.at(out, dst, msg)
    out = out.reshape(N, OUT) + gat["bias"]

    def ln(v, g, b):
        m = v.mean(-1, keepdims=True)
        var = ((v - m) ** 2).mean(-1, keepdims=True)
        return (v - m) / np.sqrt(var + EPS) * g + b

    h = ln(out, norm["g"], norm["b"])

    def dec(v, p):
        t = ln(np.maximum(v @ p["W1"] + p["b1"], 0), p["g1"], p["b1n"])
        t = ln(np.maximum(t @ p["W2"] + p["b2"], 0), p["g2"], p["b2n"])
        return t @ p["W3"] + p["b3"]

    rl = dec(h, dec_local)
    rr = dec(h, dec_remote)
    rec = np.where((node_types == 1)[:, None], rl, rr)
    return rec.astype(np.float32), edge_index, alpha.astype(np.float32)


def _to_np(v):
    return {k: _to_np(x) for k, x in v.items()} if isinstance(v, dict) else np.asarray(v)


def kernel(node_features, gat, norm, dec_local, dec_remote, node_types,
           edge_index, trace=False):
    _ensure_axon_hooks_stub()
    node_features = np.asarray(node_features, np.float32)
    gat, norm = _to_np(gat), _to_np(norm)
    dec_local, dec_remote = _to_np(dec_local), _to_np(dec_remote)
    node_types_np = np.asarray(node_types)
    edge_index_np = np.asarray(edge_index)

    if not np.array_equal(edge_index_np.astype(np.int64), _expected_edge_index()):
        return _np_reference_fallback(
            node_features, gat, norm, dec_local, dec_remote,
            node_types_np, edge_index_np,
        )

    from concourse.bass_utils import run_bass_kernel_spmd

    f32 = np.float32
    W = gat["W"].astype(f32)
    att_src = gat["att_src"].astype(f32)
    att_dst = gat["att_dst"].astype(f32)
    Wsrc = (W.reshape(MAPPED, HEADS, HID) * att_src[None]).sum(-1)
    Wdst = (W.reshape(MAPPED, HEADS, HID) * att_dst[None]).sum(-1)
    Wg_ext = np.ascontiguousarray(np.concatenate([W, Wsrc, Wdst], axis=1))
    nfT = np.ascontiguousarray(node_features.T)

    # fold LN affine params into the following linear layer; biases
    # pre-broadcast to [128, w]
    def fold(dec):
        g, b = norm["g"].astype(f32), norm["b"].astype(f32)
        W1 = g[:, None] * dec["W1"]
        b1 = dec["b1"] + b @ dec["W1"]
        W2 = dec["g1"][:, None] * dec["W2"]
        b2 = dec["b2"] + dec["b1n"] @ dec["W2"]
        W3 = dec["g2"][:, None] * dec["W3"]
        b3 = dec["b3"] + dec["b2n"] @ dec["W3"]

        def bc(v):
            return np.ascontiguousarray(
                np.broadcast_to(v.reshape(1, -1).astype(f32), (128, v.size))
            )

        return (
            np.ascontiguousarray(W1, f32), bc(b1),
            np.ascontiguousarray(W2, f32), bc(b2),
            np.ascontiguousarray(W3, f32), bc(b3),
        )

    # dec_all (combined l|r): W1lr(512, rearranged) b1lr(256) W2lr(128)
    # b2lr(128) W3lr-pad(256) b3lr(256) Bg(256)
    Fl = fold(dec_local)
    Fr = fold(dec_remote)
    W1lr = np.concatenate([Fl[0], Fr[0]], axis=1)          # [256, 256]
    W1lr = W1lr.reshape(2, 128, 256).transpose(1, 0, 2).reshape(128, 512)
    b1lr = np.concatenate([Fl[1], Fr[1]], axis=1)          # [128, 256]
    W2lr = np.concatenate([Fl[2], Fr[2]], axis=1)          # [128, 128]
    b2lr = np.concatenate([Fl[3], Fr[3]], axis=1)          # [128, 128]
    W3p = np.zeros((128, 256), f32)
    W3p[0:64, 0:128] = Fl[4]
    W3p[0:64, 128:256] = Fr[4]
    b3lr = np.concatenate([Fl[5], Fr[5]], axis=1)          # [128, 256]
    Bg = np.broadcast_to(gat["bias"].reshape(1, -1).astype(f32), (128, OUT))
    dec_all = np.ascontiguousarray(
        np.concatenate([W1lr, b1lr, W2lr, b2lr, W3p, b3lr, Bg], axis=1), f32
    )

    in_maps = []
    for c in range(NCORES):
        nf_all = np.concatenate(
            [
                nfT,
                nfT[:, c * SH : (c + 1) * SH],
                nfT[:, L + c * SH : L + (c + 1) * SH],
                Wg_ext,
            ],
            axis=1,
        )
        in_maps.append({"nf_all": np.ascontiguousarray(nf_all), "dec_all": dec_all})

    nc = _get_nc()
    res = run_bass_kernel_spmd(nc, in_maps, core_ids=list(range(NCORES)), trace=trace)
    _NC_CACHE["last_results"] = res
    outs = res.results

    # ---- host unshard / assembly ----
    # alphaT_full[s, d, h] = exp(e) * R
    alphaT_full = np.empty((L, L, HEADS), np.float32)
    a_rl = np.empty((L, HEADS), np.float32)
    a_lr = np.empty((L, HEADS), np.float32)
    a_rr = np.empty((L, HEADS), np.float32)
    for c in range(NCORES):
        sl = slice(c * SH, (c + 1) * SH)
        sm = outs[c]["asmall"]
        a_rl[sl], a_lr[sl], a_rr[sl] = sm[:, 0:4], sm[:, 4:8], sm[:, 8:12]
        R = sm[:, 12:16]  # [d, h]
        blk = outs[c]["alpha_t"].reshape(128, HEADS, NLOC_T, SH)  # [p, h, t, d]
        # -> [s=t*128+p, d, h]
        blk = np.transpose(blk, (2, 0, 3, 1)).reshape(L, SH, HEADS)
        alphaT_full[:, sl, :] = blk * R[None, :, :]

    per_edge = alphaT_full.reshape(L * L, HEADS)
    mask = ~np.eye(L, dtype=bool).reshape(-1)
    clique = per_edge[mask]
    idx = np.arange(L)
    self_local = alphaT_full[idx, idx, :]
    alpha = np.concatenate(
        [clique, a_lr, a_rl, self_local, a_rr], axis=0
    ).astype(np.float32)

    rec_local = np.empty((N, IN_DIM), np.float32)
    rec_remote = np.empty((N, IN_DIM), np.float32)
    for c in range(NCORES):
        sl = slice(c * SH, (c + 1) * SH)
        slr = slice(L + c * SH, L + (c + 1) * SH)
        rec_local[sl] = outs[c]["rec_l"][0:SH]
        rec_local[slr] = outs[c]["rec_l"][SH : 2 * SH]
        rec_remote[sl] = outs[c]["rec_r"][0:SH]
        rec_remote[slr] = outs[c]["rec_r"][SH : 2 * SH]
    reconstructed = np.where((node_types_np == 1)[:, None], rec_local, rec_remote)

    return reconstructed.astype(np.float32), edge_index_np, alpha


# revision 26
# speedup vs baseline: 1.4048x; 1.0585x over previous
"""Self-contained Trainium2 (Bass/Tile) kernel for the DeviceGAT problem.

Computes, on 8 NeuronCores, the GAT layer + LayerNorm + two decoder MLPs of
reference.py, exploiting the deterministic graph structure:
  - edges = dense local clique (1024 local nodes, no self edges)
            + local->remote pairs (i -> L+i)
            + remote->local pairs (L+i -> i)
            + self loops for all 2048 nodes
  - so each local dst d has in-edges from all 1024 local nodes (incl. itself
    via the self loop) plus remote node L+d; each remote dst L+i has in-edges
    {i, L+i}.

Sharding: destination rows are sharded 8 ways (128 local dst rows + the
matching 128 remote dst rows per core).  Node features / params replicated.

Scores are built directly in transposed [src, dst] layout (a_dst broadcast by
a tiny selector matmul, a_src as the per-partition activation bias), so the
attention needs NO on-chip transposes.  exp(scores) is written to HBM
unnormalized together with the per-(dst, head) softmax reciprocal R; the host
applies R while permuting the dense block into the reference per-edge order.
The aggregation matmul contracts src on the partition axis (exb^T @ [x | 1])
in bf16, producing both the weighted message sum and the softmax denominator
for all four heads in one PSUM tile; per-head epilogue work is batched into a
handful of broadcast-AP vector ops.
"""

import os
import sys
import types
import numpy as np

# ---------------------------------------------------------------------------
# Problem constants (from the reference problem definition; deterministic).
L = 1024
N = 2 * L
MAPPED = 32
HID = 64
HEADS = 4
OUT = HID * HEADS        # 256
IN_DIM = 128
NEG_SLOPE = 0.2
EPS = 1e-5
NCORES = 8
SH = L // NCORES         # 128 dst rows per core
NLOC_T = L // 128        # 8 local node tiles


def _ensure_axon_hooks_stub():
    """run_bass_kernel_spmd(trace=True) imports antenv.axon_hooks; provide a
    graceful stub when the image lacks it so tracing degrades instead of
    crashing.  (Harness runs trace=False and never hits this, but be safe.)"""
    try:
        import antenv.axon_hooks  # noqa: F401
        return
    except Exception:
        pass
    try:
        import antenv
    except Exception:
        antenv = types.ModuleType("antenv")
        sys.modules["antenv"] = antenv
    mod = types.ModuleType("antenv.axon_hooks")
    mod._HOOK = None

    def set_axon_ntff_profile_hook(hook):
        mod._HOOK = hook

    def get_axon_ntff_profile_hook():
        if mod._HOOK is not None:
            return mod._HOOK
        so = "/opt/axon/libaxon_pjrt.so"
        if os.path.exists(so):
            import contextlib
            import ctypes

            try:
                lib = ctypes.CDLL(so)
            except OSError:
                return None
            if not hasattr(lib, "axon_start_nrt_profile"):
                return None
            lib.axon_start_nrt_profile.argtypes = [
                ctypes.POINTER(ctypes.c_int64),
                ctypes.c_size_t,
            ]
            lib.axon_start_nrt_profile.restype = ctypes.c_int64
            lib.axon_stop_nrt_profile.argtypes = [ctypes.c_char_p]
            lib.axon_stop_nrt_profile.restype = ctypes.c_int64

            @contextlib.contextmanager
            def _hook(output_dir, device_ids):
                import jax

                jax.devices()
                if device_ids:
                    ids = (ctypes.c_int64 * len(device_ids))(*device_ids)
                    rc = lib.axon_start_nrt_profile(ids, len(device_ids))
                else:
                    rc = lib.axon_start_nrt_profile(None, 0)
                if rc != 0:
                    raise RuntimeError(f"axon_start_nrt_profile rc={rc}")
                try:
                    yield
                finally:
                    n = lib.axon_stop_nrt_profile(str(output_dir).encode())
                    print(f"profile: {n} file(s) in {output_dir}", file=sys.stderr)

            mod._HOOK = _hook
            return mod._HOOK
        return None

    mod.set_axon_ntff_profile_hook = set_axon_ntff_profile_hook
    mod.get_axon_ntff_profile_hook = get_axon_ntff_profile_hook
    sys.modules["antenv.axon_hooks"] = mod


# ---------------------------------------------------------------------------
# Bass kernel builder


def _build_nc():
    from contextlib import ExitStack

    import concourse.bass as bass
    import concourse.tile as tile
    from concourse import bacc, mybir
    from concourse.masks import make_identity

    f32 = mybir.dt.float32
    bf16 = mybir.dt.bfloat16
    AF = mybir.ActivationFunctionType
    MUL = mybir.AluOpType.mult
    ADD = mybir.AluOpType.add
    MAX = mybir.AluOpType.max
    WEXT = OUT + 2 * HEADS  # 264: [Wg | Wsrc | Wdst]

    nc = bacc.Bacc(
        trn_type="TRN2", target_bir_lowering=False, debug=False, num_devices=NCORES
    )

    # ---- I/O ----
    # nf_all: [32, N + SH + SH + WEXT] = nfT | nfT_loc | nfT_rem | Wg_ext
    NFC = N + 2 * SH + WEXT
    nf_all = nc.dram_tensor("nf_all", [MAPPED, NFC], f32, kind="ExternalInput")
    # dec_all: [128, 1792] = for each dec: W1(256) b1(128) W2(64) b2(64)
    # W3pad(128) b3(128); then Bg(256)
    DCC = 2 * (256 + 128 + 64 + 64 + 128 + 128) + OUT
    dec_all = nc.dram_tensor("dec_all", [128, DCC], f32, kind="ExternalInput")

    # dense unnormalized exp(scores), transposed layout, raw per-head blocks
    # [p, t, d]: src node s = t*128+p, dst column d
    alpha_t = nc.dram_tensor("alpha_t", [128, HEADS * L], f32, kind="ExternalOutput")
    # a_rl | a_lr | a_rr | R  (R = per-(d,h) softmax reciprocal)
    asmall = nc.dram_tensor("asmall", [SH, 16], f32, kind="ExternalOutput")
    rec_l = nc.dram_tensor("rec_l", [2 * SH, IN_DIM], f32, kind="ExternalOutput")
    rec_r = nc.dram_tensor("rec_r", [2 * SH, IN_DIM], f32, kind="ExternalOutput")

    with tile.TileContext(nc) as tc, ExitStack() as ctx:
        consts = ctx.enter_context(tc.tile_pool(name="consts", bufs=1))
        big = ctx.enter_context(tc.tile_pool(name="big", bufs=3))
        scp = ctx.enter_context(tc.tile_pool(name="scp", bufs=4))
        persist = ctx.enter_context(tc.tile_pool(name="persist", bufs=1))
        small = ctx.enter_context(tc.tile_pool(name="small", bufs=2))
        psB = ctx.enter_context(tc.tile_pool(name="psB", bufs=1, space="PSUM"))
        psG = ctx.enter_context(tc.tile_pool(name="psG", bufs=1, space="PSUM"))
        psT = ctx.enter_context(tc.tile_pool(name="psT", bufs=2, space="PSUM"))
        psX = ctx.enter_context(tc.tile_pool(name="psX", bufs=4, space="PSUM"))

        # ---- load constants ----
        # host layout: [nfT_loc | nfT_rem | Wg_ext | nfT]; the small head chunk
        # is DMA'd first so the shard projections unblock immediately
        SMALL_C = 2 * SH + WEXT
        sb_nf = consts.tile([MAPPED, NFC], f32)
        nc.sync.dma_start(sb_nf[:, 0:SMALL_C], nf_all[:, 0:SMALL_C])
        nc.sync.dma_start(sb_nf[:, SMALL_C:NFC], nf_all[:, SMALL_C:NFC])
        sb_nfT_loc = sb_nf[:, 0:SH]
        sb_nfT_rem = sb_nf[:, SH : 2 * SH]
        sb_Wg = sb_nf[:, 2 * SH : SMALL_C]
        sb_nfT = sb_nf[:, SMALL_C:NFC]
        sb_dec = consts.tile([128, DCC], f32)
        nc.sync.dma_start(sb_dec[:], dec_all[:])
        # combined l|r decoder params
        dW1 = sb_dec[:, 0:512].rearrange("p (a n) -> p a n", a=2)  # [128,2,256]
        db1 = sb_dec[:, 512:768]
        dW2 = sb_dec[:, 768:896]      # cols 0:64 = W2l, 64:128 = W2r
        db2 = sb_dec[:, 896:1024]
        dW3 = sb_dec[0:64, 1024:1280]  # cols 0:128 = W3l, 128:256 = W3r
        db3 = sb_dec[:, 1280:1536]

        eps_col = consts.tile([128, 1], f32)
        nc.vector.memset(eps_col[:], EPS)
        ident = consts.tile([128, 128], f32)
        make_identity(nc, ident[:])
        # head-selector: sel4[k, h*128+m] = (k == h); used as k=4 lhsT to
        # broadcast row h of a [4, x] tile across 128 partitions
        sel4 = consts.tile([HEADS, HEADS * 128], f32)
        nc.gpsimd.memset(sel4[:], 0.0)
        sel4v = sel4.rearrange("p (a b) -> p a b", a=HEADS)
        nc.gpsimd.affine_select(
            out=sel4v,
            in_=sel4v,
            compare_op=mybir.AluOpType.not_equal,
            fill=1.0,
            base=0,
            pattern=[[-1, HEADS], [0, 128]],
            channel_multiplier=1,
        )
        sb_Bg = sb_dec[:, 1536 : 1536 + OUT]

        # ---- projections on PE: x|a_src|a_dst per node tile ----
        # shard rows first so the special-edge path unblocks early
        ps = psX.tile([128, WEXT], f32, tag="psx", name="ps_rem")
        nc.tensor.matmul(ps[:], sb_nfT_rem[:], sb_Wg[:])
        x_rem = persist.tile([128, OUT], f32, tag="xrem")
        nc.scalar.copy(x_rem[:], ps[:, 0:OUT])
        A_rem = persist.tile([128, 2 * HEADS], f32, tag="arem")
        nc.vector.tensor_copy(A_rem[:], ps[:, OUT:WEXT])

        ps = psX.tile([128, WEXT], f32, tag="psx", name="ps_loc")
        nc.tensor.matmul(ps[:], sb_nfT_loc[:], sb_Wg[:])
        x_shard = persist.tile([128, OUT], f32, tag="xshard")
        nc.vector.tensor_copy(x_shard[:], ps[:, 0:OUT])
        A_loc = persist.tile([128, 2 * HEADS], f32, tag="aloc")
        nc.scalar.copy(A_loc[:], ps[:, OUT:WEXT])

        # a_dst of the shard as rows: [4, 128]
        psd = psX.tile([128, WEXT], f32, tag="psx", name="psd")
        nc.tensor.matmul(
            psd[0:HEADS, 0:SH], sb_Wg[:, OUT + HEADS : WEXT], sb_nfT_loc[:]
        )
        a_dstT = consts.tile([HEADS, SH], f32)
        nc.scalar.copy(a_dstT[:], psd[0:HEADS, 0:SH])

        # ---- special-edge scores: e_rl | e_lr | e_rr  [128, 12] ----
        E3 = persist.tile([128, 12], f32, tag="E3")
        nc.vector.tensor_add(E3[:, 0:4], A_rem[:, 0:4], A_loc[:, 4:8])
        nc.vector.tensor_add(E3[:, 4:8], A_loc[:, 0:4], A_rem[:, 4:8])
        nc.vector.tensor_add(E3[:, 8:12], A_rem[:, 0:4], A_rem[:, 4:8])
        LR3 = persist.tile([128, 12], f32, tag="LR3")
        nc.scalar.activation(LR3[:], E3[:], AF.Prelu, alpha=NEG_SLOPE)
        EXP3 = persist.tile([128, 12], f32, tag="EXP3")
        nc.scalar.activation(EXP3[:], LR3[:], AF.Exp)

        # remote-dst softmax (2 edges) and h_rem (batched broadcast ops)
        asm = persist.tile([128, 16], f32, tag="asm")  # a_rl | a_lr | a_rr | R
        Zr = persist.tile([128, 4], f32, tag="Zr")
        nc.vector.tensor_add(Zr[:], EXP3[:, 4:8], EXP3[:, 8:12])
        Rr = persist.tile([128, 4], f32, tag="Rr")
        nc.vector.reciprocal(Rr[:], Zr[:])
        nc.vector.tensor_mul(asm[:, 4:8], EXP3[:, 4:8], Rr[:])
        nc.vector.tensor_mul(asm[:, 8:12], EXP3[:, 8:12], Rr[:])

        x_shard_v = x_shard.rearrange("p (h c) -> p h c", h=HEADS)
        x_rem_v = x_rem.rearrange("p (h c) -> p h c", h=HEADS)
        hrem_t1 = persist.tile([128, OUT], f32, tag="hrem_t1")
        nc.vector.tensor_mul(
            hrem_t1.rearrange("p (h c) -> p h c", h=HEADS),
            x_shard_v, asm[:, 4:8].to_broadcast((128, HEADS, HID)),
        )
        hrem_t2 = persist.tile([128, OUT], f32, tag="hrem_t2")
        nc.vector.tensor_mul(
            hrem_t2.rearrange("p (h c) -> p h c", h=HEADS),
            x_rem_v, asm[:, 8:12].to_broadcast((128, HEADS, HID)),
        )
        h_rem = persist.tile([128, OUT], f32, tag="h_rem")
        nc.vector.tensor_add(h_rem[:], hrem_t1[:], hrem_t2[:])
        nc.vector.tensor_add(h_rem[:], h_rem[:], sb_Bg[:])

        # local node tiles
        xbf_all = persist.tile([128, NLOC_T, HEADS, HID + 1], bf16, tag="xbf")
        nc.gpsimd.memset(xbf_all[:], 1.0)
        A_src = persist.tile([128, NLOC_T, HEADS], f32, tag="asrc")
        for t in range(NLOC_T):
            ps = psX.tile([128, WEXT], f32, tag="psx")
            nc.tensor.matmul(ps[:], sb_nfT[:, t * 128 : (t + 1) * 128], sb_Wg[:])
            nc.vector.tensor_copy(
                xbf_all[:, t, :, 0:HID],
                ps[:, 0:OUT].rearrange("p (h c) -> p h c", h=HEADS),
            )
            nc.scalar.copy(A_src[:, t, :], ps[:, OUT : OUT + HEADS])

        # ---- layernorm helper (stats DVE, sqrt ACT, apply DVE) ----
        BNS = nc.vector.BN_STATS_DIM
        BNA = nc.vector.BN_AGGR_DIM

        def layer_norm(x_t, width, tagp, groups=1):
            g, w = groups, width // groups
            xv = x_t[:, 0:width].rearrange("p (g w) -> p g w", g=g)
            st = small.tile([128, g, BNS], f32, tag="bnst")
            mv = small.tile([128, g, BNA], f32, tag="bnmv")
            for i in range(g):
                nc.vector.bn_stats(st[:, i, :], xv[:, i, :])
                nc.vector.bn_aggr(mv[:, i, :], st[:, i, :])
            sdev = small.tile([128, g], f32, tag="sdev")
            nc.scalar.activation(sdev[:], mv[:, :, 1], AF.Sqrt, bias=eps_col[:])
            rstd = small.tile([128, g], f32, tag="rstd")
            nc.vector.reciprocal(rstd[:], sdev[:])
            negm = small.tile([128, g], f32, tag="negm")
            nc.vector.tensor_scalar(negm[:], mv[:, :, 0], -1.0, None, op0=MUL)
            out_t = big.tile([128, width], f32, tag=tagp)
            ov = out_t.rearrange("p (g w) -> p g w", g=g)
            nc.vector.tensor_add(ov, xv, negm[:].to_broadcast((128, g, w)))
            nc.vector.tensor_mul(ov, ov, rstd[:].to_broadcast((128, g, w)))
            return out_t

        # ---- decoder machinery (f32) ----
        rec_sb = {
            "l": persist.tile([128, 2, IN_DIM], f32, tag="recl", name="recl"),
            "r": persist.tile([128, 2, IN_DIM], f32, tag="recr", name="recr"),
        }

        def transpose_h(hn_t, nt):
            psTt = psT.tile([128, 4, 128], f32, tag="psT")
            nc.tensor.transpose(psTt[:, 0, :], hn_t[:, 0:128], ident[:])
            nc.tensor.transpose(psTt[:, 1, :], hn_t[:, 128:256], ident[:])
            ha = big.tile([128, 128], f32, tag=f"hTa{nt}", name=f"hTa{nt}")
            hb = big.tile([128, 128], f32, tag=f"hTb{nt}", name=f"hTb{nt}")
            if nt == 0:
                nc.scalar.copy(ha[:], psTt[:, 0, :])
                nc.vector.tensor_copy(hb[:], psTt[:, 1, :])
            else:
                nc.vector.tensor_copy(ha[:], psTt[:, 0, :])
                nc.scalar.copy(hb[:], psTt[:, 1, :])
            return ha, hb

        def dec_stage1(nt, ha, hb):
            # both decoders (l|r) processed jointly in the free dim
            ps1 = psX.tile([128, WEXT], f32, tag="psx")
            nc.tensor.matmul(ps1[:, 0:256], ha[:], dW1[:, 0, :],
                             start=True, stop=False)
            nc.tensor.matmul(ps1[:, 0:256], hb[:], dW1[:, 1, :],
                             start=False, stop=True)
            s1 = big.tile([128, 256], f32, tag="s1")
            nc.vector.tensor_add(s1[:], ps1[:, 0:256], db1[:])
            r1 = big.tile([128, 256], f32, tag="r1")
            nc.vector.tensor_scalar(r1[:], s1[:], 0.0, None, op0=MAX)
            n1 = layer_norm(r1, 256, "n1", groups=2)
            psn = psT.tile([128, 4, 128], f32, tag="psT")
            nc.tensor.transpose(psn[:, 0, :], n1[:, 0:128], ident[:])
            nc.tensor.transpose(psn[:, 1, :], n1[:, 128:256], ident[:])
            n1lT = big.tile([128, 128], f32, tag="n1lT")
            n1rT = big.tile([128, 128], f32, tag="n1rT")
            if nt == 0:
                nc.scalar.copy(n1lT[:], psn[:, 0, :])
                nc.vector.tensor_copy(n1rT[:], psn[:, 1, :])
            else:
                nc.vector.tensor_copy(n1lT[:], psn[:, 0, :])
                nc.scalar.copy(n1rT[:], psn[:, 1, :])
            return n1lT, n1rT

        def dec_stage2(nt, n1lT, n1rT):
            ps2 = psX.tile([128, WEXT], f32, tag="psx")
            nc.tensor.matmul(ps2[:, 0:64], n1lT[:], dW2[:, 0:64],
                             start=True, stop=True)
            nc.tensor.matmul(ps2[:, 64:128], n1rT[:], dW2[:, 64:128],
                             start=True, stop=True)
            s2 = big.tile([128, 128], f32, tag="s2")
            nc.vector.tensor_add(s2[:], ps2[:, 0:128], db2[:])
            r2 = big.tile([128, 128], f32, tag="r2")
            nc.vector.tensor_scalar(r2[:], s2[:], 0.0, None, op0=MAX)
            n2 = layer_norm(r2, 128, "n2", groups=2)
            psn2 = psT.tile([128, 4, 128], f32, tag="psT")
            nc.tensor.transpose(psn2[0:64, 0, :], n2[:, 0:64], ident[:])
            nc.tensor.transpose(psn2[0:64, 1, :], n2[:, 64:128], ident[:])
            n2lT = big.tile([64, 128], f32, tag="n2lT")
            n2rT = big.tile([64, 128], f32, tag="n2rT")
            if nt == 0:
                nc.vector.tensor_copy(n2lT[:], psn2[0:64, 0, :])
                nc.scalar.copy(n2rT[:], psn2[0:64, 1, :])
            else:
                nc.scalar.copy(n2lT[:], psn2[0:64, 0, :])
                nc.vector.tensor_copy(n2rT[:], psn2[0:64, 1, :])
            return n2lT, n2rT

        def dec_stage3(nt, n2lT, n2rT):
            ps3 = psX.tile([128, WEXT], f32, tag="psx")
            nc.tensor.matmul(ps3[:, 0:128], n2lT[:], dW3[:, 0:128],
                             start=True, stop=True)
            nc.tensor.matmul(ps3[:, 128:256], n2rT[:], dW3[:, 128:256],
                             start=True, stop=True)
            nc.vector.tensor_add(rec_sb["l"][:, nt, :], ps3[:, 0:128], db3[:, 0:128])
            nc.vector.tensor_add(rec_sb["r"][:, nt, :], ps3[:, 128:256], db3[:, 128:256])

        def run_decoders_nt(nt, ha, hb):
            n1lT, n1rT = dec_stage1(nt, ha, hb)
            n2lT, n2rT = dec_stage2(nt, n1lT, n1rT)
            dec_stage3(nt, n2lT, n2rT)

        # ---- dense attention per head (transposed [s, d] layout) ----
        # all 4 a_dst broadcasts in one PSUM bank
        psBh = psB.tile([128, HEADS, SH], f32, tag="Bh")
        for h in range(HEADS):
            nc.tensor.matmul(
                psBh[:, h, :], sel4[:, h * 128 : (h + 1) * 128], a_dstT[:]
            )
        # S[p, t, d] = a_src[t*128+p, h] + a_dst[d] for all heads up front, so
        # the DVE stream never stalls behind ACT
        S_hs = []
        for h in range(HEADS):
            S_h = scp.tile([128, NLOC_T, SH], f32, tag="eT", name=f"S{h}")
            bh_slice = psBh[:, h, :]
            bh_bcast = bass.AP(
                tensor=bh_slice.tensor,
                offset=bh_slice.offset,
                ap=[bh_slice.ap[0], [0, NLOC_T], bh_slice.ap[1]],
            )
            nc.vector.tensor_add(
                S_h[:], bh_bcast, A_src[:, :, h].to_broadcast((128, NLOC_T, SH))
            )
            S_hs.append(S_h)

        # all 4 heads accumulate into one PSUM tile: [:, h, 0:64] = msg sum,
        # [:, h, 64] = sum(exp)
        psagg = psG.tile([128, HEADS, HID + 1], f32, tag="agg")
        nt1_state = {}

        def nt1_piece(step):
            # spread the remote-half decoder chain across head iterations
            if step == 0:
                hn_rem = layer_norm(h_rem, OUT, "hn")
                nt1_state["h"] = transpose_h(hn_rem, 1)
            elif step == 1:
                nt1_state["n1"] = dec_stage1(1, *nt1_state["h"])
            elif step == 2:
                nt1_state["n2"] = dec_stage2(1, *nt1_state["n1"])
            elif step == 3:
                dec_stage3(1, *nt1_state["n2"])
                nc.sync.dma_start(rec_l[SH : 2 * SH, :], rec_sb["l"][:, 1, :])
                nc.sync.dma_start(rec_r[SH : 2 * SH, :], rec_sb["r"][:, 1, :])

        for h in range(HEADS):
            S_h = S_hs[h]
            ex_T = big.tile([128, NLOC_T, SH], f32, tag="exT")
            eflat = S_h.rearrange("p a b -> p (a b)")
            nc.scalar.activation(eflat, eflat, AF.Prelu, alpha=NEG_SLOPE)
            nc.scalar.activation(
                ex_T.rearrange("p a b -> p (a b)"), eflat, AF.Exp
            )
            # unnormalized exp block straight to HBM (host normalizes)
            nc.sync.dma_start(
                alpha_t[:, h * L : (h + 1) * L],
                ex_T.rearrange("p a b -> p (a b)"),
            )
            exb = big.tile([128, NLOC_T, SH], bf16, tag="exb")
            nc.vector.tensor_copy(exb[:], ex_T[:])

            # aggregation + Z in one accumulation: exb[s,:]^T @ [x_h | 1]
            for t in range(NLOC_T):
                nc.tensor.matmul(
                    psagg[:, h, :],
                    exb[:, t, :],
                    xbf_all[:, t, h, :],
                    start=(t == 0),
                    stop=(t == NLOC_T - 1),
                )
            nt1_piece(h)

        # ---- batched per-head epilogue ----
        # Zf = psagg[:, :, 64] + exp(e_rl);  R = 1/Zf
        Zf = small.tile([128, HEADS], f32, tag="Zf")
        nc.vector.tensor_add(Zf[:], psagg[:, :, HID], EXP3[:, 0:4])
        nc.vector.reciprocal(asm[:, 12:16], Zf[:])
        nc.vector.tensor_mul(asm[:, 0:4], EXP3[:, 0:4], asm[:, 12:16])
        # h_loc = (agg + exp(e_rl)*x_rem) * R + bgat
        xt1 = persist.tile([128, OUT], f32, tag="xt1")
        xt1_v = xt1.rearrange("p (h c) -> p h c", h=HEADS)
        nc.vector.tensor_mul(
            xt1_v, x_rem_v, EXP3[:, 0:4].to_broadcast((128, HEADS, HID))
        )
        hpre = persist.tile([128, OUT], f32, tag="hpre")
        hpre_v = hpre.rearrange("p (h c) -> p h c", h=HEADS)
        nc.vector.tensor_add(hpre_v, psagg[:, :, 0:HID], xt1_v)
        h_loc = persist.tile([128, OUT], f32, tag="h_loc")
        nc.vector.tensor_mul(
            h_loc.rearrange("p (h c) -> p h c", h=HEADS),
            hpre_v, asm[:, 12:16].to_broadcast((128, HEADS, HID)),
        )
        nc.vector.tensor_add(h_loc[:], h_loc[:], sb_Bg[:])
        nc.sync.dma_start(asmall[:], asm[:])

        # ---- local half of decoders ----
        hn_loc = layer_norm(h_loc, OUT, "hn")
        ha0, hb0 = transpose_h(hn_loc, 0)
        run_decoders_nt(0, ha0, hb0)

        nc.sync.dma_start(rec_l[0:SH, :], rec_sb["l"][:, 0, :])
        nc.sync.dma_start(rec_r[0:SH, :], rec_sb["r"][:, 0, :])

    nc.compile()
    return nc


_NC_CACHE = {}


def _get_nc():
    if "nc" not in _NC_CACHE:
        _NC_CACHE["nc"] = _build_nc()
    return _NC_CACHE["nc"]


# ---------------------------------------------------------------------------
# Host side


def _expected_edge_index():
    local = np.arange(L)
    s = np.repeat(local, L)
    d = np.tile(local, L)
    m = s != d
    src = np.concatenate([s[m], local, L + local, np.arange(N)])
    dst = np.concatenate([d[m], L + local, local, np.arange(N)])
    return np.stack([src, dst])


def _np_reference_fallback(node_features, gat, norm, dec_local, dec_remote,
                           node_types, edge_index):
    """Pure-numpy replica of the reference; used only if the edge structure
    is not the expected deterministic pattern."""
    x = node_features @ gat["W"]
    xh = x.reshape(N, HEADS, HID)
    a_src = (xh * gat["att_src"]).sum(-1)
    a_dst = (xh * gat["att_dst"]).sum(-1)
    src, dst = edge_index[0], edge_index[1]
    e = a_src[src] + a_dst[dst]
    e = np.where(e >= 0, e, NEG_SLOPE * e).astype(np.float32)
    emax = np.full((N, HEADS), -np.inf, np.float32)
    np.maximum.at(emax, dst, e)
    ex = np.exp(e - emax[dst])
    zs = np.zeros((N, HEADS), np.float32)
    np.add.at(zs, dst, ex)
    alpha = ex / zs[dst]
    msg = xh[src] * alpha[:, :, None]
    out = np.zeros((N, HEADS, HID), np.float32)
    np.add.at(out, dst, msg)
    out = out.reshape(N, OUT) + gat["bias"]

    def ln(v, g, b):
        m = v.mean(-1, keepdims=True)
        var = ((v - m) ** 2).mean(-1, keepdims=True)
        return (v - m) / np.sqrt(var + EPS) * g + b

    h = ln(out, norm["g"], norm["b"])

    def dec(v, p):
        t = ln(np.maximum(v @ p["W1"] + p["b1"], 0), p["g1"], p["b1n"])
        t = ln(np.maximum(t @ p["W2"] + p["b2"], 0), p["g2"], p["b2n"])
        return t @ p["W3"] + p["b3"]

    rl = dec(h, dec_local)
    rr = dec(h, dec_remote)
    rec = np.where((node_types == 1)[:, None], rl, rr)
    return rec.astype(np.float32), edge_index, alpha.astype(np.float32)


def _to_np(v):
    return {k: _to_np(x) for k, x in v.items()} if isinstance(v, dict) else np.asarray(v)


def kernel(node_features, gat, norm, dec_local, dec_remote, node_types,
           edge_index, trace=False):
    _ensure_axon_hooks_stub()
    node_features = np.asarray(node_features, np.float32)
    gat, norm = _to_np(gat), _to_np(norm)
    dec_local, dec_remote = _to_np(dec_local), _to_np(dec_remote)
    node_types_np = np.asarray(node_types)
    edge_index_np = np.asarray(edge_index)

    if not np.array_equal(edge_index_np.astype(np.int64), _expected_edge_index()):
        return _np_reference_fallback(
            node_features, gat, norm, dec_local, dec_remote,
            node_types_np, edge_index_np,
        )

    from concourse.bass_utils import run_bass_kernel_spmd

    f32 = np.float32
    W = gat["W"].astype(f32)
    att_src = gat["att_src"].astype(f32)
    att_dst = gat["att_dst"].astype(f32)
    Wsrc = (W.reshape(MAPPED, HEADS, HID) * att_src[None]).sum(-1)
    Wdst = (W.reshape(MAPPED, HEADS, HID) * att_dst[None]).sum(-1)
    Wg_ext = np.ascontiguousarray(np.concatenate([W, Wsrc, Wdst], axis=1))
    nfT = np.ascontiguousarray(node_features.T)

    # fold LN affine params into the following linear layer; biases
    # pre-broadcast to [128, w]
    def fold(dec):
        g, b = norm["g"].astype(f32), norm["b"].astype(f32)
        W1 = g[:, None] * dec["W1"]
        b1 = dec["b1"] + b @ dec["W1"]
        W2 = dec["g1"][:, None] * dec["W2"]
        b2 = dec["b2"] + dec["b1n"] @ dec["W2"]
        W3 = dec["g2"][:, None] * dec["W3"]
        b3 = dec["b3"] + dec["b2n"] @ dec["W3"]

        def bc(v):
            return np.ascontiguousarray(
                np.broadcast_to(v.reshape(1, -1).astype(f32), (128, v.size))
            )

        return (
            np.ascontiguousarray(W1, f32), bc(b1),
            np.ascontiguousarray(W2, f32), bc(b2),
            np.ascontiguousarray(W3, f32), bc(b3),
        )

    # dec_all (combined l|r): W1lr(512, rearranged) b1lr(256) W2lr(128)
    # b2lr(128) W3lr-pad(256) b3lr(256) Bg(256)
    Fl = fold(dec_local)
    Fr = fold(dec_remote)
    W1lr = np.concatenate([Fl[0], Fr[0]], axis=1)          # [256, 256]
    W1lr = W1lr.reshape(2, 128, 256).transpose(1, 0, 2).reshape(128, 512)
    b1lr = np.concatenate([Fl[1], Fr[1]], axis=1)          # [128, 256]
    W2lr = np.concatenate([Fl[2], Fr[2]], axis=1)          # [128, 128]
    b2lr = np.concatenate([Fl[3], Fr[3]], axis=1)          # [128, 128]
    W3p = np.zeros((128, 256), f32)
    W3p[0:64, 0:128] = Fl[4]
    W3p[0:64, 128:256] = Fr[4]
    b3lr = np.concatenate([Fl[5], Fr[5]], axis=1)          # [128, 256]
    Bg = np.broadcast_to(gat["bias"].reshape(1, -1).astype(f32), (128, OUT))
    dec_all = np.ascontiguousarray(
        np.concatenate([W1lr, b1lr, W2lr, b2lr, W3p, b3lr, Bg], axis=1), f32
    )

    in_maps = []
    for c in range(NCORES):
        nf_all = np.concatenate(
            [
                nfT[:, c * SH : (c + 1) * SH],
                nfT[:, L + c * SH : L + (c + 1) * SH],
                Wg_ext,
                nfT,
            ],
            axis=1,
        )
        in_maps.append({"nf_all": np.ascontiguousarray(nf_all), "dec_all": dec_all})

    nc = _get_nc()
    res = run_bass_kernel_spmd(nc, in_maps, core_ids=list(range(NCORES)), trace=trace)
    _NC_CACHE["last_results"] = res
    outs = res.results

    # ---- host unshard / assembly ----
    # alphaT_full[s, d, h] = exp(e) * R
    alphaT_full = np.empty((L, L, HEADS), np.float32)
    a_rl = np.empty((L, HEADS), np.float32)
    a_lr = np.empty((L, HEADS), np.float32)
    a_rr = np.empty((L, HEADS), np.float32)
    for c in range(NCORES):
        sl = slice(c * SH, (c + 1) * SH)
        sm = outs[c]["asmall"]
        a_rl[sl], a_lr[sl], a_rr[sl] = sm[:, 0:4], sm[:, 4:8], sm[:, 8:12]
        R = sm[:, 12:16]  # [d, h]
        blk = outs[c]["alpha_t"].reshape(128, HEADS, NLOC_T, SH)  # [p, h, t, d]
        # -> [s=t*128+p, d, h]
        blk = np.transpose(blk, (2, 0, 3, 1)).reshape(L, SH, HEADS)
        alphaT_full[:, sl, :] = blk * R[None, :, :]

    per_edge = alphaT_full.reshape(L * L, HEADS)
    mask = ~np.eye(L, dtype=bool).reshape(-1)
    clique = per_edge[mask]
    idx = np.arange(L)
    self_local = alphaT_full[idx, idx, :]
    alpha = np.concatenate(
        [clique, a_lr, a_rl, self_local, a_rr], axis=0
    ).astype(np.float32)

    rec_local = np.empty((N, IN_DIM), np.float32)
    rec_remote = np.empty((N, IN_DIM), np.float32)
    for c in range(NCORES):
        sl = slice(c * SH, (c + 1) * SH)
        slr = slice(L + c * SH, L + (c + 1) * SH)
        rec_local[sl] = outs[c]["rec_l"][0:SH]
        rec_local[slr] = outs[c]["rec_l"][SH : 2 * SH]
        rec_remote[sl] = outs[c]["rec_r"][0:SH]
        rec_remote[slr] = outs[c]["rec_r"][SH : 2 * SH]
    reconstructed = np.where((node_types_np == 1)[:, None], rec_local, rec_remote)

    return reconstructed.astype(np.float32), edge_index_np, alpha
